# revision 34
# baseline (speedup 1.0000x reference)
"""GATNet (3x GATConv graph branch + 1D-CNN protein branch + fusion MLP) on 8
Trainium2 NeuronCores via Bass/Tile.

v2 pipeline:
  - L1 x@W replicated on every core (tiny) -> h1_full in natural node order;
    no AllGather for layer 1.
  - Layers 2/3: per-dst-block interleave of [message-pass block m] ->
    [next-layer matmul block m] -> [per-block AllGather of that block's h].
    h2/h3_full use AG-native row order (m*1024 + core*128 + r) with gather
    indices remapped on host, so each 128-row block AllGather lands
    contiguously and communication pipelines behind compute.
  - Message pass per block: per-tile indirect gathers into one SBUF strip,
    block-batched score chain (add / prelu / exp), single broadcast-AP
    multiply for per-head scaling, S^T one-hot matmuls accumulate numerator
    + denominator (ex column first), fused scalar_tensor_tensor epilogue
    (x*rec + bias), batched activation, then transpose tiles for the next
    matmul's lhsT.
  - CNN branch: embedding done on host (E shipped as input); stages
    interleaved into the GAT phases. Large head weights in bf16.

Self-contained: hardcodes all shapes; builds the per-call edge structure into
the traced program, compiles and runs via run_bass_kernel_spmd.
"""
import numpy as np
import ml_dtypes

import concourse.bass as bass
import concourse.mybir as mybir
import concourse.tile as tile
from concourse.bass_utils import run_bass_kernel_spmd
from concourse.masks import make_identity
from concourse.tile import add_dep_helper

NCORES = 8
N_NODES = 10240
N_GRAPHS = 256
NPC = N_NODES // NCORES          # 1280 nodes/core
GPC = N_GRAPHS // NCORES         # 32 graphs/core
NPG = N_NODES // N_GRAPHS        # 40 nodes/graph
BPC = NPC // 128                 # 10 dst blocks/core
NBLK = N_NODES // 128            # 80 global blocks
SEQ = 1000
VOCAB = 26
EMB = 128
NEG_SLOPE = 0.2

F32 = mybir.dt.float32
BF16 = mybir.dt.bfloat16
I32 = mybir.dt.int32
AX = mybir.AxisListType
OP = mybir.AluOpType
ACT = mybir.ActivationFunctionType

# (F_in, F_out, heads)
LAYERS = [(78, 780, 10), (780, 1560, 2), (1560, 3120, 1)]
# x@W psum chunk lists over [fo + 2H] aug cols (h | a_s | a_d)
MM_CHUNKS = [[512, 288], [512, 512, 512, 28], [512] * 6 + [50]]
# message-pass psum chunks over gs cols [ex H | h fo]
MP_CHUNKS = [[512, 278], [512, 512, 512, 26], [512] * 6 + [49]]

bf = lambda a: np.ascontiguousarray(a).astype(ml_dtypes.bfloat16)
f32 = lambda a: np.ascontiguousarray(a, dtype=np.float32)
cdiv = lambda a, b: -(-a // b)


# ------------------------------------------------------------------ walrus patch
def _split_sync_waits(nc, max_keep=1):
    for f in nc.m.functions:
        for bb in f.blocks:
            out, changed = [], False
            for ins in bb.instructions:
                si = ins.sync_info
                waits = list(si.on_wait) if si is not None and si.on_wait else []
                if len(waits) > max_keep:
                    extra, keep = waits[:-max_keep], waits[-max_keep:]
                    for i in range(0, len(extra), max_keep):
                        out.append(mybir.InstNoOp(
                            name=f"WSPLIT-{nc.next_id()}", engine=ins.engine,
                            bass_nofuse=True,
                            sync_info=mybir.SyncInfo(on_wait=extra[i:i + max_keep],
                                                     on_update=[])))
                    si.on_wait = keep
                    changed = True
                out.append(ins)
            if changed:
                bb.instructions[:] = out


# ------------------------------------------------------------------ host prep
def _edge_structure(edge_index):
    src, dst = edge_index[0].astype(np.int64), edge_index[1].astype(np.int64)
    loop = np.arange(N_NODES, dtype=np.int64)
    s_all = np.concatenate([src, loop])
    d_all = np.concatenate([dst, loop])
    order = np.argsort(d_all, kind="stable")
    s_s, d_s = s_all[order], d_all[order]

    bounds = np.searchsorted(d_s, np.arange(0, N_NODES + 1, 128))
    cnt = bounds[1:] - bounds[:-1]
    tiles_needed = -(-cnt // 128)
    T_blocks = [int(tiles_needed.reshape(NCORES, BPC)[:, p].max()) for p in range(BPC)]
    t_off = np.cumsum([0] + T_blocks)
    T_tot = int(t_off[-1])

    src_nat = np.zeros((NCORES, T_tot, 128), np.int32)
    S = np.zeros((NCORES, T_tot, 128, 128), np.float32)
    for c in range(NCORES):
        for p_ in range(BPC):
            blk = c * BPC + p_
            e0, e1 = int(bounds[blk]), int(bounds[blk + 1])
            m = e1 - e0
            ti = np.arange(m) // 128 + t_off[p_]
            ei = np.arange(m) % 128
            src_nat[c, ti, ei] = s_s[e0:e1]
            S[c, ti, ei, d_s[e0:e1] - 128 * blk] = 1.0
    # half-AG row order: half h = loc//640; row = h*5120 + core*640 + loc%640
    sv = src_nat.astype(np.int64)
    loc = sv % NPC
    src_ag = ((loc // 640) * 5120 + (sv // NPC) * 640 + loc % 640).astype(np.int32)
    ST = np.ascontiguousarray(np.swapaxes(S, 2, 3))
    natT = np.ascontiguousarray(np.swapaxes(src_nat, 1, 2))  # [8,128,T_tot]
    agT = np.ascontiguousarray(np.swapaxes(src_ag, 1, 2))
    return T_blocks, natT, agT, bf(S), f32(ST)


def _aug_w(W, a_s, a_d, H):
    """[W | W@as_blk | W@ad_blk] with as_blk[f,h] = a_s[h, f - h*FH]."""
    fi, fo = W.shape
    FH = fo // H
    was = np.zeros((fi, H), np.float32)
    wad = np.zeros((fi, H), np.float32)
    for h in range(H):
        was[:, h] = W[:, h * FH:(h + 1) * FH] @ a_s[h]
        wad[:, h] = W[:, h * FH:(h + 1) * FH] @ a_d[h]
    return np.concatenate([W, was, wad], axis=1), wad


def _host_prep(inputs):
    ii = {k: np.asarray(v) for k, v in inputs.items()}
    T_blocks, natT, agT, S, ST = _edge_structure(ii["edge_index"])

    xT = np.ascontiguousarray(np.swapaxes(f32(ii["x"]), 0, 1))   # [78, 10240]

    W_aug, wads, bias_rep = [], [], []
    for i, (fi, fo, H) in enumerate(LAYERS):
        Wa, wad = _aug_w(f32(ii[f"W{i+1}"]), f32(ii[f"as{i+1}"]),
                         f32(ii[f"ad{i+1}"]), H)
        W_aug.append(bf(Wa))
        wads.append(bf(wad))
        b = f32(ii[f"b{i+1}"]).reshape(1, -1)
        bias_rep.append(bf(np.broadcast_to(b, (128, fo))))

    cw1 = f32(ii["cw1"])
    cw1f = np.zeros((125, 8, 2, 128), np.float32)
    for sc in range(8):
        for ks in range(2):
            blk = cw1[:, sc * 125:(sc + 1) * 125, ks * 4:(ks + 1) * 4]
            cw1f[:, sc, ks, :] = blk.transpose(1, 2, 0).reshape(125, 128)
    cwT = lambda w: np.ascontiguousarray(np.transpose(f32(ii[w]), (1, 2, 0)))

    w1xt = np.ascontiguousarray(
        f32(ii["fc1_xt_w"]).reshape(128, 33, 1024).transpose(1, 0, 2))

    emb = f32(ii["emb_xt"])
    rep = lambda a, n: np.ascontiguousarray(
        np.broadcast_to(f32(a).reshape(1, -1), (n, f32(a).size)))

    shared = {
        "W1": W_aug[0], "W2": W_aug[1], "W3": W_aug[2],
        "br1": bias_rep[0], "br2": bias_rep[1], "br3": bias_rep[2],
        "wad1": wads[0],
        "fc_g1_w": bf(ii["fc_g1_w"]), "fc_g1_b": rep(ii["fc_g1_b"], GPC),
        "fc_g2_w": f32(ii["fc_g2_w"]), "fc_g2_b": rep(ii["fc_g2_b"], GPC),
        "cw1f": bf(cw1f), "cb1": f32(ii["cb1"]).reshape(-1, 1),
        "cw2T": bf(cwT("cw2")), "cb2": f32(ii["cb2"]).reshape(-1, 1),
        "cw3T": bf(cwT("cw3")), "cb3": f32(ii["cb3"]).reshape(-1, 1),
        "cw4T": bf(cwT("cw4")), "cb4": f32(ii["cb4"]).reshape(-1, 1),
        "w1xt": bf(w1xt), "fc1_xt_b": rep(ii["fc1_xt_b"], GPC),
        "fc2_xt_w": f32(ii["fc2_xt_w"]), "fc2_xt_b": rep(ii["fc2_xt_b"], GPC),
        "fc1_w": f32(ii["fc1_w"]), "fc1_b": rep(ii["fc1_b"], GPC),
        "fc2_w": f32(ii["fc2_w"]), "fc2_b": rep(ii["fc2_b"], GPC),
        "out_w": f32(ii["out_w"]),
    }
    in_maps = []
    for c in range(NCORES):
        m = dict(shared)
        m["xT"] = bf(xT)                                  # full, replicated
        m["xTl"] = bf(xT[:, c * NPC:(c + 1) * NPC])       # local slice
        m["enat"] = natT[c]
        m["eag"] = agT[c]
        m["S"] = S[c]
        m["ST"] = ST[c]
        # host-embedded CNN input: E[grp, sc, s, bloc*128+e]
        tgt_c = ii["target"][c * GPC:(c + 1) * GPC].astype(np.int64)  # [32,1000]
        E_full = emb[tgt_c]                               # [32, 1000, 128]
        Eh = E_full.reshape(8, 4, 8, 125, 128).transpose(0, 2, 3, 1, 4)
        m["Eh"] = bf(Eh.reshape(8, 8, 125, 512))
        in_maps.append(m)
    out_b = float(np.asarray(ii["out_b"]).reshape(-1)[0])
    return T_blocks, in_maps, out_b


# ------------------------------------------------------------------ program
class P:
    pass


def _aug_cols(li):
    fo, H = LAYERS[li][1], LAYERS[li][2]
    return fo + 2 * H + 2        # h | a_s(f32 as 2H bf16) | ones | pad


def build_program(T_blocks, taps=()):
    T_tot = sum(T_blocks)
    nc = bass.Bass()
    p = P()
    p.nc = nc
    p.T_blocks = T_blocks
    p.taps = set(taps)
    p.tap_tensors = {}

    dp = lambda name, shape, dt: nc.declare_dram_parameter(name, list(shape), dt,
                                                           isOutput=False)
    p.xT = dp("xT", [78, N_NODES], BF16)
    p.xTl = dp("xTl", [78, NPC], BF16)
    p.W = [dp(f"W{i+1}", [LAYERS[i][0], LAYERS[i][1] + 2 * LAYERS[i][2]], BF16)
           for i in range(3)]
    p.br = [dp(f"br{i+1}", [128, LAYERS[i][1]], BF16) for i in range(3)]
    p.wad1 = dp("wad1", [78, LAYERS[0][2]], BF16)
    p.enat = dp("enat", [128, T_tot], I32)
    p.eag = dp("eag", [128, T_tot], I32)
    p.S = dp("S", [T_tot, 128, 128], BF16)
    p.ST = dp("ST", [T_tot, 128, 128], F32)
    p.fc_g1_w = dp("fc_g1_w", [3120, 1024], BF16)
    p.fc_g1_b = dp("fc_g1_b", [GPC, 1024], F32)
    p.fc_g2_w = dp("fc_g2_w", [1024, 128], F32)
    p.fc_g2_b = dp("fc_g2_b", [GPC, 128], F32)
    p.Eh = dp("Eh", [8, 8, 125, 512], BF16)
    p.cw1f = dp("cw1f", [125, 8, 2, 128], BF16)
    p.cb1 = dp("cb1", [32, 1], F32)
    p.cw2T = dp("cw2T", [32, 8, 64], BF16)
    p.cb2 = dp("cb2", [64, 1], F32)
    p.cw3T = dp("cw3T", [64, 8, 96], BF16)
    p.cb3 = dp("cb3", [96, 1], F32)
    p.cw4T = dp("cw4T", [96, 8, 128], BF16)
    p.cb4 = dp("cb4", [128, 1], F32)
    p.w1xt = dp("w1xt", [33, 128, 1024], BF16)
    p.fc1_xt_b = dp("fc1_xt_b", [GPC, 1024], F32)
    p.fc2_xt_w = dp("fc2_xt_w", [1024, 128], F32)
    p.fc2_xt_b = dp("fc2_xt_b", [GPC, 128], F32)
    p.fc1_w = dp("fc1_w", [256, 1024], F32)
    p.fc1_b = dp("fc1_b", [GPC, 1024], F32)
    p.fc2_w = dp("fc2_w", [1024, 256], F32)
    p.fc2_b = dp("fc2_b", [GPC, 256], F32)
    p.out_w = dp("out_w", [256, 1], F32)
    p.out = nc.declare_dram_parameter("out", [GPC, 1], F32, isOutput=True)

    # h1_full natural order (replicated compute, no AG); h2/h3 AG-native order
    p.h_full = [nc.dram_tensor("h1_full", [N_NODES, _aug_cols(0)], BF16)]
    for i in (1, 2):
        p.h_full.append(nc.dram_tensor(f"h{i+1}_full", [N_NODES, _aug_cols(i)],
                                       BF16, addr_space="Shared"))
    p.h_loc = [None,
               nc.dram_tensor("h2_loc", [NPC, _aug_cols(1)], BF16),
               nc.dram_tensor("h3_loc", [NPC, _aug_cols(2)], BF16)]

    def tap(name, shape, dt=F32):
        if name in p.taps:
            t = nc.declare_dram_parameter("tap_" + name, list(shape), dt,
                                          isOutput=True)
            p.tap_tensors[name] = t
            return t
        return None
    p.tap = tap

    with tile.TileContext(nc) as tc:
        p.tc = tc
        _cp_cm = tc.tile_pool(name="const", bufs=1)
        const_pool = _cp_cm.__enter__()
        p.ident = const_pool.tile([128, 128], BF16)
        make_identity(nc, p.ident[:])
        p.head_pool = const_pool

        p.cnn = _cnn_make(p)
        _gat(p)
        _fusion(p)
        _cp_cm.__exit__(None, None, None)

    _split_sync_waits(nc)
    return nc, p


# ---------------- layer-1 replicated matmul ----------------
def _l1_mm(p):
    """Every core computes h1 for ALL nodes -> h1_full (natural order)."""
    nc, tc = p.nc, p.tc
    fi, fo, H = LAYERS[0]
    cols = _aug_cols(0)
    chunks = MM_CHUNKS[0]
    offs = [int(v) for v in np.cumsum([0] + chunks)]
    h_writes = []
    with (
        tc.tile_pool(name="l1w", bufs=1) as wpool,
        tc.tile_pool(name="l1x", bufs=1) as xpool,
        tc.tile_pool(name="l1m", bufs=3) as mpool,
        tc.tile_pool(name="l1p", bufs=2, space="PSUM") as pspool,
    ):
        W_sb = wpool.tile([fi, fo + 2 * H], BF16, tag="W", name="W")
        nc.sync.dma_start(out=W_sb[:], in_=p.W[0][:])
        xf = xpool.tile([fi, N_NODES], BF16, tag="xf", name="xf")
        nc.sync.dma_start(out=xf[:], in_=p.xT[:])
        # local a_d via xTl @ wad1
        xl = xpool.tile([fi, NPC], BF16, tag="xl", name="xl")
        nc.sync.dma_start(out=xl[:], in_=p.xTl[:])
        wad = xpool.tile([fi, H], BF16, tag="wad", name="wad")
        nc.sync.dma_start(out=wad[:], in_=p.wad1[:])
        p.a_d1 = []
        with tc.tile_pool(name="l1adp", bufs=2, space="PSUM") as adps:
            for blk in range(BPC):
                ps = adps.tile([128, H], F32, tag="ad", name="ad")
                nc.tensor.matmul(ps[:], xl[:, blk * 128:(blk + 1) * 128], wad[:],
                                 start=True, stop=True)
                t = p.head_pool.tile([128, H], F32, tag=f"a_d1_{blk}",
                                     name=f"a_d1_{blk}")
                nc.vector.tensor_copy(out=t[:], in_=ps[:])
                p.a_d1.append(t)
        for m in range(NBLK):
            stage = mpool.tile([128, cols], BF16, tag="stage", name="stage",
                               bufs=4)
            for n in range(len(chunks)):
                lo, hi = offs[n], offs[n + 1]
                ps = pspool.tile([128, chunks[n]], F32, tag=f"mp{n}", name=f"mp{n}")
                nc.tensor.matmul(ps[:], xf[:, m * 128:(m + 1) * 128],
                                 W_sb[:, lo:hi], start=True, stop=True)
                if hi <= fo:
                    nc.scalar.copy(out=stage[:, lo:hi], in_=ps[:])
                else:
                    if lo < fo:
                        nc.vector.tensor_copy(out=stage[:, lo:fo],
                                              in_=ps[:, :fo - lo])
                    a_sf = mpool.tile([128, H], F32, tag="a_sf", name="a_sf")
                    nc.vector.tensor_copy(out=a_sf[:], in_=ps[:, fo - lo:fo - lo + H])
                    nc.vector.tensor_copy(
                        out=stage[:, fo:fo + 2 * H], in_=a_sf[:].bitcast(BF16))
            if m < 4:
                oc = fo + 2 * H
                nc.vector.memset(stage[:, oc:oc + 1], 1.0)
                nc.vector.memset(stage[:, oc + 1:cols], 0.0)
            w = nc.sync.dma_start(out=p.h_full[0][m * 128:(m + 1) * 128, :],
                                  in_=stage[:])
            h_writes.append(w)
    # fence: all h1_full writes done
    fence_sb = p.head_pool.tile([1, 2], BF16, tag="fence1", name="fence1")
    fence = nc.sync.dma_start(out=fence_sb[:], in_=p.h_full[0][0:1, 0:2])
    for w in h_writes:
        add_dep_helper(fence.ins, w.ins, reason="h1 fence")
    return fence


# ---------------- message pass for one dst block ----------------
def _mp_block(p, li, blk, pools, a_d_tile, fence, xT_out):
    """Message pass layer li for dst block blk. xT_out: either a list of
    full-width tiles (write cols blk*128..) or None -> allocate per-block
    [128,128] tiles in the mp pool and return them."""
    nc, tc = p.nc, p.tc
    fi, fo, H = LAYERS[li]
    FH = fo // H
    cols = _aug_cols(li)
    gsc = H + fo                     # gs cols: [ex H | h fo]
    chunks = MP_CHUNKS[li]
    offs = [int(v) for v in np.cumsum([0] + chunks)]
    t_off = np.cumsum([0] + p.T_blocks)
    Tb = p.T_blocks[blk]
    t0 = int(t_off[blk])
    n_kT = cdiv(fo, 128)
    mp, mpS = pools["mp"], pools["mpS"]
    eidx = p.eidx_ag if li == 2 else p.eidx_nat

    S_blk = mpS.tile([128, Tb * 128], BF16, tag="Sblk", name="Sblk", bufs=2)
    nc.sync.dma_start(
        out=S_blk[:].rearrange("p (t c) -> p t c", c=128),
        in_=p.S[t0:t0 + Tb].rearrange("t p c -> p t c"))
    ST_blk = mpS.tile([128, Tb * 128], F32, tag="STblk", name="STblk")
    nc.sync.dma_start(
        out=ST_blk[:].rearrange("p (t c) -> p t c", c=128),
        in_=p.ST[t0:t0 + Tb].rearrange("t p c -> p t c"))

    per_tile = li >= 1
    rows_bf = mp.tile([128, fo], BF16, tag="rows_bf", name="rows_bf", bufs=2)
    rows_f = mp.tile([128, fo], F32, tag="rows_f", name="rows_f") \
        if li == 0 else rows_bf
    rec = mp.tile([128, H], F32, tag="rec", name="rec", bufs=2)
    if per_tile:
        # per-tile pipeline (H == 1): gather -> score -> scale -> matmuls
        with (
            tc.tile_pool(name=f"ap{li}_{blk}", bufs=1, space="PSUM") as pa,
            tc.tile_pool(name=f"op{li}_{blk}", bufs=1, space="PSUM") as pp,
        ):
            n_ch = len(chunks)
            opsum = [pp.tile([128, chunks[n]], F32, tag=f"op{n}", name=f"op{n}")
                     for n in range(n_ch)]
            for i in range(Tb):
                t = t0 + i
                g_t = mp.tile([128, cols], BF16, tag="g_t", name="g_t", bufs=6)
                gi = nc.gpsimd.indirect_dma_start(
                    out=g_t[:], out_offset=None, in_=p.h_full[li][:],
                    in_offset=bass.IndirectOffsetOnAxis(ap=eidx[:, t:t + 1],
                                                        axis=0))
                add_dep_helper(gi.ins, fence.ins, reason="gather waits h_full")
                adg = pa.tile([128, H], F32, tag="adg", name="adg")
                nc.tensor.matmul(adg[:], ST_blk[:, i * 128:(i + 1) * 128],
                                 a_d_tile[:], start=True, stop=True)
                sc = mp.tile([128, H], F32, tag="sc_t", name="sc_t", bufs=6)
                nc.vector.tensor_tensor(out=sc[:],
                                        in0=g_t[:, fo:fo + 2 * H].bitcast(F32),
                                        in1=adg[:], op=OP.add)
                ex = mp.tile([128, H], F32, tag="ex_t", name="ex_t", bufs=6)
                nc.scalar.activation(ex[:], sc[:], ACT.Prelu, alpha=NEG_SLOPE)
                nc.scalar.activation(ex[:], ex[:], ACT.Exp)
                gs_t = mp.tile([128, gsc], BF16, tag="gs_t", name="gs_t", bufs=4)
                nc.vector.tensor_copy(out=gs_t[:, 0:H], in_=ex[:])
                for h in range(H):
                    eng = nc.vector if h % 2 == 0 else nc.scalar
                    if h % 2 == 0:
                        nc.vector.tensor_scalar(
                            out=gs_t[:, H + h * FH:H + (h + 1) * FH],
                            in0=g_t[:, h * FH:(h + 1) * FH],
                            scalar1=ex[:, h:h + 1], scalar2=None, op0=OP.mult)
                    else:
                        nc.scalar.mul(gs_t[:, H + h * FH:H + (h + 1) * FH],
                                      g_t[:, h * FH:(h + 1) * FH],
                                      ex[:, h:h + 1])
                for n in range(n_ch):
                    nc.tensor.matmul(
                        opsum[n][:], S_blk[:, i * 128:(i + 1) * 128],
                        gs_t[:, offs[n]:offs[n + 1]],
                        start=(i == 0), stop=(i == Tb - 1))
            nc.vector.tensor_scalar(out=rec[:], in0=opsum[0][:, 0:H],
                                    scalar1=1e-16, scalar2=None, op0=OP.add)
            nc.vector.reciprocal(rec[:], rec[:])
            for n in range(n_ch):
                lo, hi = max(offs[n], H), offs[n + 1]
                a, b = lo - H, hi - H
                h0, h1 = a // FH, cdiv(b, FH)
                for h in range(h0, h1):
                    s_lo, s_hi = max(a, h * FH), min(b, (h + 1) * FH)
                    nc.vector.scalar_tensor_tensor(
                        out=rows_f[:, s_lo:s_hi],
                        in0=opsum[n][:, s_lo + H - offs[n]:s_hi + H - offs[n]],
                        scalar=rec[:, h:h + 1], in1=p.br_sb[li][:, s_lo:s_hi],
                        op0=OP.mult, op1=OP.add)
    else:
        g_blk = mp.tile([128, Tb * cols], BF16, tag="g_blk", name="g_blk",
                        bufs=2)
        gv = g_blk[:].rearrange("p (t c) -> p t c", c=cols)
        gs_blk = mp.tile([128, Tb * gsc], BF16, tag="gs_blk", name="gs_blk",
                         bufs=2)
        gsv = gs_blk[:].rearrange("p (t c) -> p t c", c=gsc)
        sc_blk = mp.tile([128, Tb * H], F32, tag="sc_blk", name="sc_blk",
                         bufs=2)
        ex_blk = mp.tile([128, Tb * H], F32, tag="ex_blk", name="ex_blk",
                         bufs=2)
        with (
            tc.tile_pool(name=f"ap{li}_{blk}", bufs=1, space="PSUM") as pa,
        ):
            adg = pa.tile([128, Tb * H], F32, tag="adg", name="adg")
            for i in range(Tb):
                t = t0 + i
                gi = nc.gpsimd.indirect_dma_start(
                    out=g_blk[:, i * cols:(i + 1) * cols], out_offset=None,
                    in_=p.h_full[li][:],
                    in_offset=bass.IndirectOffsetOnAxis(ap=eidx[:, t:t + 1],
                                                        axis=0))
                add_dep_helper(gi.ins, fence.ins, reason="gather waits h_full")
                nc.tensor.matmul(adg[:, i * H:(i + 1) * H],
                                 ST_blk[:, i * 128:(i + 1) * 128], a_d_tile[:],
                                 start=True, stop=True)
            # score chain, batched over tiles
            nc.vector.tensor_tensor(
                out=sc_blk[:].rearrange("p (t h) -> p t h", h=H),
                in0=gv[:, :, fo:fo + 2 * H].bitcast(F32),
                in1=adg[:].rearrange("p (t h) -> p t h", h=H), op=OP.add)
            nc.scalar.activation(ex_blk[:], sc_blk[:], ACT.Prelu,
                                 alpha=NEG_SLOPE)
            nc.scalar.activation(ex_blk[:], ex_blk[:], ACT.Exp)
            exv = ex_blk[:].rearrange("p (t h) -> p t h", h=H)
            nc.vector.tensor_copy(out=gsv[:, :, 0:H], in_=exv)
            nc.vector.tensor_tensor(
                out=gsv[:, :, H:].rearrange("p t (h f) -> p t h f", h=H),
                in0=gv[:, :, 0:fo].rearrange("p t (h f) -> p t h f", h=H),
                in1=exv.unsqueeze(3).broadcast_to((128, Tb, H, FH)),
                op=OP.mult)
            with tc.tile_pool(name=f"op{li}_{blk}", bufs=1,
                              space="PSUM") as pp:
                opsum = [pp.tile([128, chunks[n]], F32, tag=f"op{n}",
                                 name=f"op{n}")
                         for n in range(len(chunks))]
                for i in range(Tb):
                    S_t = S_blk[:, i * 128:(i + 1) * 128]
                    for n in range(len(chunks)):
                        lo, hi = offs[n], offs[n + 1]
                        nc.tensor.matmul(
                            opsum[n][:], S_t,
                            gs_blk[:, i * gsc + lo:i * gsc + hi],
                            start=(i == 0), stop=(i == Tb - 1))
                nc.vector.tensor_scalar(out=rec[:], in0=opsum[0][:, 0:H],
                                        scalar1=1e-16, scalar2=None,
                                        op0=OP.add)
                nc.vector.reciprocal(rec[:], rec[:])
                for n in range(len(chunks)):
                    lo, hi = max(offs[n], H), offs[n + 1]
                    if lo >= hi:
                        continue
                    a, b = lo - H, hi - H
                    h0, h1 = a // FH, cdiv(b, FH)
                    for h in range(h0, h1):
                        s_lo, s_hi = max(a, h * FH), min(b, (h + 1) * FH)
                        nc.vector.scalar_tensor_tensor(
                            out=rows_f[:, s_lo:s_hi],
                            in0=opsum[n][:, s_lo + H - offs[n]:
                                          s_hi + H - offs[n]],
                            scalar=rec[:, h:h + 1],
                            in1=p.br_sb[li][:, s_lo:s_hi],
                            op0=OP.mult, op1=OP.add)
    # activation (batched, row-major)
    if li == 0:
        t1 = mp.tile([128, fo], F32, tag="elu1", name="elu1")
        nc.vector.tensor_scalar(out=t1[:], in0=rows_f[:], scalar1=0.0,
                                scalar2=None, op0=OP.min)
        nc.scalar.activation(t1[:], t1[:], ACT.Exp)
        nc.scalar.activation(rows_f[:], rows_f[:], ACT.Relu)
        nc.vector.scalar_tensor_tensor(out=rows_bf[:], in0=t1[:], scalar=-1.0,
                                       in1=rows_f[:], op0=OP.add, op1=OP.add)
    else:
        nc.scalar.activation(rows_bf[:], rows_bf[:], ACT.Relu)
    # transpose; either into full-width xT_out columns or per-block tiles
    if xT_out is None:
        xtb = [mp.tile([min(128, fo - j * 128), 128], BF16, tag=f"xtb{j}",
                       name=f"xtb{j}", bufs=2) for j in range(n_kT)]
        dst = lambda j: xtb[j][:]
    else:
        dst = lambda j: xT_out[j][:, blk * 128:(blk + 1) * 128]
    with tc.tile_pool(name=f"tp{li}_{blk}", bufs=2, space="PSUM") as ptp:
        for j in range(n_kT):
            kp = min(128, fo - j * 128)
            tp = ptp.tile([kp, 128], BF16, tag="tp", name="tp")
            nc.tensor.transpose(tp[:], rows_bf[:, j * 128:j * 128 + kp],
                                p.ident[:])
            if j % 2 == 0:
                nc.scalar.copy(out=dst(j), in_=tp[:])
            else:
                nc.vector.tensor_copy(out=dst(j), in_=tp[:])
    return None if xT_out is not None else xtb


# ---------------- next-layer matmul for one dst block ----------------
def _mm_block(p, li, blk, W_sb, xtb, pools, a_d_list):
    """x@W for layer li (1 or 2), rows of dst block blk (lhsT tiles xtb);
    writes h_loc rows. Returns the stage-write instr for the AG."""
    nc, tc = p.nc, p.tc
    fi, fo, H = LAYERS[li]
    cols = _aug_cols(li)
    n_k = cdiv(fi, 128)
    chunks = MM_CHUNKS[li]
    offs = [int(v) for v in np.cumsum([0] + chunks)]
    mp = pools["mm"]
    stage = mp.tile([128, cols], BF16, tag=f"stg{li}", name=f"stg{li}", bufs=2)
    with tc.tile_pool(name=f"mmp{li}_{blk}", bufs=4, space="PSUM") as pspool:
        for n in range(len(chunks)):
            lo, hi = offs[n], offs[n + 1]
            ps = pspool.tile([128, chunks[n]], F32, tag="mmps", name="mmps")
            for k in range(n_k):
                nc.tensor.matmul(
                    ps[:], xtb[k], W_sb[k][:, lo:hi],
                    start=(k == 0), stop=(k == n_k - 1))
            if hi <= fo:
                nc.scalar.copy(out=stage[:, lo:hi], in_=ps[:])
            else:
                if lo < fo:
                    nc.scalar.copy(out=stage[:, lo:fo], in_=ps[:, :fo - lo])
                a_sf = mp.tile([128, H], F32, tag="a_sf2", name="a_sf2")
                nc.vector.tensor_copy(out=a_sf[:], in_=ps[:, fo - lo:fo - lo + H])
                a_d = p.head_pool.tile([128, H], F32, tag=f"a_d{li}_{blk}",
                                       name=f"a_d{li}_{blk}")
                nc.vector.tensor_copy(out=a_d[:],
                                      in_=ps[:, fo - lo + H:fo - lo + 2 * H])
                a_d_list.append(a_d)
                nc.vector.tensor_copy(out=stage[:, fo:fo + 2 * H],
                                      in_=a_sf[:].bitcast(BF16))
    oc = fo + 2 * H
    nc.vector.memset(stage[:, oc:oc + 1], 1.0)
    nc.vector.memset(stage[:, oc + 1:cols], 0.0)
    w = nc.sync.dma_start(out=p.h_loc[li][blk * 128:(blk + 1) * 128, :],
                          in_=stage[:])
    return w


def _fire_ag(p, li, half, ws):
    """AllGather of one half of h_loc -> h_full rows (half-AG order)."""
    nc = p.nc
    cc = nc.gpsimd.collective_compute(
        "AllGather", OP.bypass, replica_groups=[list(range(NCORES))],
        ins=[p.h_loc[li][half * 640:(half + 1) * 640, :]],
        outs=[p.h_full[li][half * 5120:(half + 1) * 5120, :]])
    for w in ws:
        add_dep_helper(cc.ins, w.ins, reason="AG waits h_loc writes")
    return cc


# ---------------- GAT orchestration ----------------
def _gat(p):
    nc, tc = p.nc, p.tc
    T_tot = sum(p.T_blocks)

    mpc_cm = tc.tile_pool(name="mpc", bufs=1)
    mpc = mpc_cm.__enter__()
    p.eidx_nat = mpc.tile([128, T_tot], I32, tag="enat", name="enat")
    nc.sync.dma_start(out=p.eidx_nat[:], in_=p.enat[:])
    p.eidx_ag = mpc.tile([128, T_tot], I32, tag="eag", name="eag")
    nc.sync.dma_start(out=p.eidx_ag[:], in_=p.eag[:])
    p.br_sb = []
    for li in range(3):
        t = mpc.tile([128, LAYERS[li][1]], BF16, tag=f"br{li}", name=f"br{li}")
        nc.sync.dma_start(out=t[:], in_=p.br[li][:])
        p.br_sb.append(t)

    fence1 = _l1_mm(p)
    p.cnn["stage1_w"]()     # CNN weight loads early

    # ---- phase 1: MP L1 (all blocks) -> dense mm L2 with mid-pass AG ----
    a_d2 = []
    ccs2 = []
    xT2_cm = tc.tile_pool(name="xT2", bufs=1)
    xT2p = xT2_cm.__enter__()
    fo2 = LAYERS[0][1]
    n_kT2 = cdiv(fo2, 128)
    xT2 = [xT2p.tile([min(128, fo2 - j * 128), NPC], BF16, tag=f"xT2_{j}",
                     name=f"xT2_{j}") for j in range(n_kT2)]
    with (
        tc.tile_pool(name="w2", bufs=1) as w2pool,
        tc.tile_pool(name="mp1", bufs=1) as mp1,
        tc.tile_pool(name="mpS1", bufs=1) as mpS1,
        tc.tile_pool(name="mm1", bufs=1) as mm1,
    ):
        fi2, fo2b, H2 = LAYERS[1]
        W2_sb = []
        for k in range(cdiv(fi2, 128)):
            kp = min(128, fi2 - k * 128)
            t = w2pool.tile([kp, fo2b + 2 * H2], BF16, tag=f"W2_{k}",
                            name=f"W2_{k}")
            nc.sync.dma_start(out=t[:], in_=p.W[1][k * 128:k * 128 + kp, :])
            W2_sb.append(t)
        pools = {"mp": mp1, "mpS": mpS1, "mm": mm1}
        for blk in range(BPC):
            _mp_block(p, 0, blk, pools, p.a_d1[blk], fence1, xT2)
        ws = []
        for blk in range(BPC):
            xts = [xT2[k][:min(128, fo2 - k * 128), blk * 128:(blk + 1) * 128]
                   for k in range(n_kT2)]
            ws.append(_mm_block(p, 1, blk, W2_sb, xts, pools, a_d2))
        cc = nc.gpsimd.collective_compute(
            "AllGather", OP.bypass, replica_groups=[list(range(NCORES))],
            ins=[p.h_loc[1][:]], outs=[p.h_full[1][:]])
        for w in ws:
            add_dep_helper(cc.ins, w.ins, reason="AG2 waits h_loc writes")
        ccs2.append(cc)
        for g in range(8):
            p.cnn["stage1_grp"](g)
        p.cnn["stage2"]()
        p.cnn["stage3"]()
    xT2_cm.__exit__(None, None, None)
    p.cnn["cleanup"]()
    fence2_sb = p.head_pool.tile([1, 2], BF16, tag="fence2", name="fence2")
    fence2 = nc.sync.dma_start(out=fence2_sb[:], in_=p.h_full[1][0:1, 0:2])
    for cc in ccs2:
        add_dep_helper(fence2.ins, cc.ins, reason="h2 fence")

    # ---- phase 2: MP L2 (all blocks) -> dense mm L3 with mid-pass AG ----
    a_d3 = []
    ccs3 = []
    xT3_cm = tc.tile_pool(name="xT3", bufs=1)
    xT3p = xT3_cm.__enter__()
    fo3 = LAYERS[1][1]
    n_kT3 = cdiv(fo3, 128)
    xT3 = [xT3p.tile([min(128, fo3 - j * 128), NPC], BF16, tag=f"xT3_{j}",
                     name=f"xT3_{j}") for j in range(n_kT3)]
    with (
        tc.tile_pool(name="w3", bufs=1) as w3pool,
        tc.tile_pool(name="mp2", bufs=1) as mp2,
        tc.tile_pool(name="mpS2", bufs=1) as mpS2,
        tc.tile_pool(name="mm2", bufs=1) as mm2,
    ):
        fi3, fo3b, H3 = LAYERS[2]
        W3_sb = []
        for k in range(cdiv(fi3, 128)):
            kp = min(128, fi3 - k * 128)
            t = w3pool.tile([kp, fo3b + 2 * H3], BF16, tag=f"W3_{k}",
                            name=f"W3_{k}")
            nc.sync.dma_start(out=t[:], in_=p.W[2][k * 128:k * 128 + kp, :])
            W3_sb.append(t)
        pools = {"mp": mp2, "mpS": mpS2, "mm": mm2}
        for blk in range(BPC):
            _mp_block(p, 1, blk, pools, a_d2[blk], fence2, xT3)
        ws = []
        for blk in range(BPC):
            xts = [xT3[k][:min(128, fo3 - k * 128), blk * 128:(blk + 1) * 128]
                   for k in range(n_kT3)]
            ws.append(_mm_block(p, 2, blk, W3_sb, xts, pools, a_d3))
            if blk == 4:
                ccs3.append(_fire_ag(p, 2, 0, ws))
        ccs3.append(_fire_ag(p, 2, 1, ws[5:]))
    xT3_cm.__exit__(None, None, None)
    fence3_sb = p.head_pool.tile([1, 2], BF16, tag="fence3", name="fence3")
    fence3 = nc.sync.dma_start(out=fence3_sb[:], in_=p.h_full[2][0:1, 0:2])
    for cc in ccs3:
        add_dep_helper(fence3.ins, cc.ins, reason="h3 fence")

    # ---- phase 3: MP L3 (+ CNN stages 2/3) ----
    xT4_cm = tc.tile_pool(name="xT4", bufs=1)
    xT4p = xT4_cm.__enter__()
    fo4 = LAYERS[2][1]
    p.out3T = [xT4p.tile([min(128, fo4 - j * 128), NPC], BF16, tag=f"xT4_{j}",
                         name=f"xT4_{j}") for j in range(cdiv(fo4, 128))]
    with (
        tc.tile_pool(name="mp3", bufs=1) as mp3,
        tc.tile_pool(name="mpS3", bufs=1) as mpS3,
    ):
        pools = {"mp": mp3, "mpS": mpS3, "mm": None}
        for blk in range(BPC):
            _mp_block(p, 2, blk, pools, a_d3[blk], fence3, p.out3T)
    p.gat_cleanup = [xT4_cm, mpc_cm]

    t = p.tap("o3T", [fo4, NPC], BF16)
    if t is not None:
        for j in range(cdiv(fo4, 128)):
            kp = min(128, fo4 - j * 128)
            nc.sync.dma_start(out=t[j * 128:j * 128 + kp, :], in_=p.out3T[j][:])


def _dve_T(nc, dst, src, n):
    """dst[n, 32] = src[32, n].T via DVE 32x32 block transposes (f32)."""
    for i in range(n // 32):
        nc.vector.transpose(out=dst[32 * i:32 * (i + 1), :],
                            in_=src[:, 32 * i:32 * (i + 1)])


# ---------------- graph head ----------------
def _graph_head(p):
    nc, tc = p.nc, p.tc
    n_kT = len(p.out3T)
    with (
        tc.tile_pool(name="gh", bufs=2) as gh,
        tc.tile_pool(name="ghG", bufs=1) as ghG,
        tc.tile_pool(name="ghp", bufs=2, space="PSUM") as ghp,
    ):
        gT = [ghG.tile([min(128, 3120 - j * 128), GPC], BF16, tag=f"gT{j}",
                       name=f"gT{j}") for j in range(n_kT)]
        for j in range(n_kT):
            nc.vector.reduce_max(
                gT[j][:],
                p.out3T[j][:].rearrange("p (g n) -> p g n", n=NPG),
                axis=AX.X)
        g1 = ghG.tile([GPC, 1024], F32, tag="g1", name="g1")
        for n in range(2):
            ps = ghp.tile([GPC, 512], F32, tag="mm", name="mm")
            for j in range(n_kT):
                kp = min(128, 3120 - j * 128)
                w = gh.tile([kp, 512], BF16, tag="fg1w", name="fg1w")
                nc.sync.dma_start(out=w[:], in_=p.fc_g1_w[j * 128:j * 128 + kp,
                                                          n * 512:(n + 1) * 512])
                nc.tensor.matmul(ps[:], gT[j][:], w[:], start=(j == 0),
                                 stop=(j == n_kT - 1))
            nc.vector.tensor_copy(out=g1[:, n * 512:(n + 1) * 512], in_=ps[:])
        bb1 = gh.tile([GPC, 1024], F32, tag="ghbb", name="ghbb")
        nc.sync.dma_start(out=bb1[:], in_=p.fc_g1_b[:])
        nc.vector.tensor_tensor(out=g1[:], in0=g1[:], in1=bb1[:], op=OP.add)
        g1b = ghG.tile([GPC, 1024], F32, tag="g1b", name="g1b")
        nc.scalar.activation(g1b[:], g1[:], ACT.Relu)
        g1T = [ghG.tile([128, GPC], F32, tag=f"g1T{j}", name=f"g1T{j}")
               for j in range(8)]
        for j in range(8):
            _dve_T(nc, g1T[j], g1b[:, j * 128:(j + 1) * 128], 128)
        ps = ghp.tile([GPC, 128], F32, tag="mm", name="mm")
        for j in range(8):
            w = gh.tile([128, 128], F32, tag="fg2w", name="fg2w")
            nc.sync.dma_start(out=w[:], in_=p.fc_g2_w[j * 128:(j + 1) * 128, :])
            nc.tensor.matmul(ps[:], g1T[j][:], w[:], start=(j == 0), stop=(j == 7))
        p.g2 = p.head_pool.tile([GPC, 128], F32, tag="g2", name="g2")
        bb2 = gh.tile([GPC, 128], F32, tag="ghbb2", name="ghbb2")
        nc.sync.dma_start(out=bb2[:], in_=p.fc_g2_b[:])
        nc.vector.tensor_tensor(out=p.g2[:], in0=ps[:], in1=bb2[:], op=OP.add)
        t = p.tap("g2", [GPC, 128])
        if t is not None:
            nc.sync.dma_start(out=t[:], in_=p.g2[:])


# ---------------- CNN branch ----------------
def _cnn_make(p):
    """CNN branch split into stages interleaved into the GAT phases."""
    nc, tc = p.nc, p.tc
    st = {}

    def stage1_w():
        cn_cm = tc.tile_pool(name="cn", bufs=2)
        cnw_cm = tc.tile_pool(name="cnw", bufs=1)
        cny_cm = tc.tile_pool(name="cny", bufs=1)
        cn = cn_cm.__enter__()
        cnw = cnw_cm.__enter__()
        cny = cny_cm.__enter__()
        st["cms"] = [cny_cm, cnw_cm, cn_cm]
        st["cn"], st["cnw"], st["cny"] = cn, cnw, cny
        cw1f_sb = cnw.tile([125, 8, 2, 128], BF16, tag="cw1f", name="cw1f")
        nc.sync.dma_start(out=cw1f_sb[:], in_=p.cw1f[:])
        cw2_sb = cnw.tile([32, 8, 64], BF16, tag="cw2", name="cw2")
        nc.sync.dma_start(out=cw2_sb[:], in_=p.cw2T[:])
        cw3_sb = cnw.tile([64, 8, 96], BF16, tag="cw3", name="cw3")
        nc.sync.dma_start(out=cw3_sb[:], in_=p.cw3T[:])
        cw4_sb = cnw.tile([96, 8, 128], BF16, tag="cw4", name="cw4")
        nc.sync.dma_start(out=cw4_sb[:], in_=p.cw4T[:])
        cb = {}
        for nm, sh in [("cb1", 32), ("cb2", 64), ("cb3", 96), ("cb4", 128)]:
            cb[nm] = cnw.tile([sh, 1], F32, tag=nm, name=nm)
            nc.sync.dma_start(out=cb[nm][:], in_=getattr(p, nm)[:])
        st.update(cw1f=cw1f_sb, cw2=cw2_sb, cw3=cw3_sb, cw4=cw4_sb, cb=cb)
        y1 = cny.tile([32, GPC * 121], BF16, tag="y1", name="y1")
        st["y1"] = y1

    def stage1_grp(grp):
        cn, cb = st["cn"], st["cb"]
        cw1f_sb = st["cw1f"]
        y1 = st["y1"]
        with tc.tile_pool(name=f"cnp1_{grp}", bufs=1, space="PSUM") as cnp:
            pc = [cnp.tile([128, 512], F32, tag=f"pc{k}", name=f"pc{k}")
                  for k in range(2)]
            for sc in range(8):
                E = cn.tile([125, 512], BF16, tag="E", name="E")
                nc.sync.dma_start(out=E[:], in_=p.Eh[grp, sc])
                for ks in range(2):
                    nc.tensor.matmul(pc[ks][:], cw1f_sb[:, sc, ks, :], E[:],
                                     start=(sc == 0), stop=(sc == 7))
            acc = cn.tile([32, 4 * 121], F32, tag="c1acc", name="c1acc", bufs=1)
            accr = acc[:].rearrange("p (b t) -> p b t", b=4)
            firstop = True
            for ks in range(2):
                for kl in range(4):
                    k = ks * 4 + kl
                    src = pc[ks][:].rearrange("p (b j) -> p b j", b=4)[
                        kl * 32:(kl + 1) * 32, :, k:k + 121]
                    if firstop:
                        nc.vector.tensor_copy(out=accr, in_=src)
                        firstop = False
                    else:
                        nc.vector.tensor_tensor(out=accr, in0=accr, in1=src,
                                                op=OP.add)
            nc.scalar.activation(y1[:, grp * 4 * 121:(grp + 1) * 4 * 121],
                                 acc[:], ACT.Relu, bias=cb["cb1"][:32, :1])

    def stage2():
        cn, cny, cb = st["cn"], st["cny"], st["cb"]
        cw2_sb, cw3_sb, cw4_sb = st["cw2"], st["cw3"], st["cw4"]
        y1 = st["y1"]
        with tc.tile_pool(name="cnp2", bufs=2, space="PSUM") as cnp:
            y2 = cny.tile([64, GPC * 114], BF16, tag="y2", name="y2")
            for grp in range(8):
                ps = cnp.tile([64, 4 * 114], F32, tag="pc0", name="pc0")
                for k in range(8):
                    rhs = y1[:].rearrange("p (b t) -> p b t", t=121)[
                        :, grp * 4:(grp + 1) * 4, k:k + 114]
                    nc.tensor.matmul(ps[:], cw2_sb[:, k, :], rhs, start=(k == 0),
                                     stop=(k == 7))
                nc.scalar.activation(y2[:, grp * 4 * 114:(grp + 1) * 4 * 114],
                                     ps[:], ACT.Relu, bias=cb["cb2"][:, :1])
            y3 = cny.tile([96, GPC * 107], BF16, tag="y3", name="y3")
            for grp in range(8):
                ps = cnp.tile([96, 4 * 107], F32, tag="pc0", name="pc0")
                for k in range(8):
                    rhs = y2[:].rearrange("p (b t) -> p b t", t=114)[
                        :, grp * 4:(grp + 1) * 4, k:k + 107]
                    nc.tensor.matmul(ps[:], cw3_sb[:, k, :], rhs, start=(k == 0),
                                     stop=(k == 7))
                nc.scalar.activation(y3[:, grp * 4 * 107:(grp + 1) * 4 * 107],
                                     ps[:], ACT.Relu, bias=cb["cb3"][:, :1])
            yp = cny.tile([128, GPC * 33], BF16, tag="yp", name="yp")
            st["yp"] = yp
            for grp in range(8):
                ps = cnp.tile([128, 4 * 100], F32, tag="pc0", name="pc0")
                for k in range(8):
                    rhs = y3[:].rearrange("p (b t) -> p b t", t=107)[
                        :, grp * 4:(grp + 1) * 4, k:k + 100]
                    nc.tensor.matmul(ps[:], cw4_sb[:, k, :], rhs, start=(k == 0),
                                     stop=(k == 7))
                psr = ps[:].rearrange("p (b t) -> p b t", b=4)
                mx = cn.tile([128, 4 * 33], F32, tag="mx", name="mx")
                mxr = mx[:].rearrange("p (b t) -> p b t", b=4)
                nc.vector.tensor_copy(out=mxr, in_=psr[:, :, 0:99:3])
                nc.vector.tensor_tensor(out=mxr, in0=mxr, in1=psr[:, :, 1:100:3],
                                        op=OP.max)
                nc.vector.tensor_tensor(out=mxr, in0=mxr, in1=psr[:, :, 2:100:3],
                                        op=OP.max)
                nc.scalar.activation(yp[:, grp * 4 * 33:(grp + 1) * 4 * 33],
                                     mx[:], ACT.Relu, bias=cb["cb4"][:, :1])

    def stage3():
        cn, cny = st["cn"], st["cny"]
        yp = st["yp"]
        with tc.tile_pool(name="cnp3", bufs=2, space="PSUM") as cnp:
            xt1 = cny.tile([GPC, 1024], F32, tag="xt1", name="xt1")
            for n in range(2):
                ps = cnp.tile([GPC, 512], F32, tag="pc0", name="pc0")
                for t_ in range(33):
                    w = cn.tile([128, 512], BF16, tag="fx1w", name="fx1w", bufs=2)
                    nc.sync.dma_start(out=w[:],
                                      in_=p.w1xt[t_, :, n * 512:(n + 1) * 512])
                    lhs = yp[:].rearrange("p (b t) -> p t b", t=33)[:, t_, :]
                    nc.tensor.matmul(ps[:], lhs, w[:], start=(t_ == 0),
                                     stop=(t_ == 32))
                nc.vector.tensor_copy(out=xt1[:, n * 512:(n + 1) * 512], in_=ps[:])
            bb = cn.tile([GPC, 1024], F32, tag="fxbb", name="fxbb", bufs=1)
            nc.sync.dma_start(out=bb[:], in_=p.fc1_xt_b[:])
            nc.vector.tensor_tensor(out=xt1[:], in0=xt1[:], in1=bb[:], op=OP.add)
            nc.scalar.activation(xt1[:], xt1[:], ACT.Relu)
            xt1T = [cn.tile([128, GPC], F32, tag=f"xt1T{j}", name=f"xt1T{j}",
                            bufs=1)
                    for j in range(8)]
            for j in range(8):
                _dve_T(nc, xt1T[j], xt1[:, j * 128:(j + 1) * 128], 128)
            ps = cnp.tile([GPC, 128], F32, tag="pc0", name="pc0")
            for j in range(8):
                w = cn.tile([128, 128], F32, tag="fx2w", name="fx2w", bufs=2)
                nc.sync.dma_start(out=w[:], in_=p.fc2_xt_w[j * 128:(j + 1) * 128, :])
                nc.tensor.matmul(ps[:], xt1T[j][:], w[:], start=(j == 0),
                                 stop=(j == 7))
            p.xt2 = p.head_pool.tile([GPC, 128], F32, tag="xt2", name="xt2")
            bb2 = cn.tile([GPC, 128], F32, tag="fxbb2", name="fxbb2", bufs=1)
            nc.sync.dma_start(out=bb2[:], in_=p.fc2_xt_b[:])
            nc.vector.tensor_tensor(out=p.xt2[:], in0=ps[:], in1=bb2[:], op=OP.add)
            t = p.tap("xt2", [GPC, 128])
            if t is not None:
                nc.sync.dma_start(out=t[:], in_=p.xt2[:])

    def cleanup():
        for cm in st["cms"]:
            cm.__exit__(None, None, None)

    return {"stage1_w": stage1_w, "stage1_grp": stage1_grp, "stage2": stage2,
            "stage3": stage3, "cleanup": cleanup}


# ---------------- fusion ----------------
def _fusion(p):
    nc, tc = p.nc, p.tc
    _graph_head(p)
    for cm in p.gat_cleanup:
        cm.__exit__(None, None, None)
    with (
        tc.tile_pool(name="fu", bufs=2) as fu,
        tc.tile_pool(name="fup", bufs=2, space="PSUM") as fup,
    ):
        xcT = []
        for src_ in (p.g2, p.xt2):
            t = fu.tile([128, GPC], F32, tag=f"xcT{len(xcT)}", name=f"xcT{len(xcT)}")
            _dve_T(nc, t, src_[:], 128)
            xcT.append(t)
        c1 = fu.tile([GPC, 1024], F32, tag="c1", name="c1")
        for n in range(2):
            ps = fup.tile([GPC, 512], F32, tag="mm", name="mm")
            for j in range(2):
                w = fu.tile([128, 512], F32, tag="f1w", name="f1w")
                nc.sync.dma_start(out=w[:], in_=p.fc1_w[j * 128:(j + 1) * 128,
                                                        n * 512:(n + 1) * 512])
                nc.tensor.matmul(ps[:], xcT[j][:], w[:], start=(j == 0),
                                 stop=(j == 1))
            nc.vector.tensor_copy(out=c1[:, n * 512:(n + 1) * 512], in_=ps[:])
        bb = fu.tile([GPC, 1024], F32, tag="fbb", name="fbb")
        nc.sync.dma_start(out=bb[:], in_=p.fc1_b[:])
        nc.vector.tensor_tensor(out=c1[:], in0=c1[:], in1=bb[:], op=OP.add)
        c1b = fu.tile([GPC, 1024], F32, tag="c1b", name="c1b")
        nc.scalar.activation(c1b[:], c1[:], ACT.Relu)
        c1T = [fu.tile([128, GPC], F32, tag=f"c1T{j}", name=f"c1T{j}") for j in range(8)]
        for j in range(8):
            _dve_T(nc, c1T[j], c1b[:, j * 128:(j + 1) * 128], 128)
        ps = fup.tile([GPC, 256], F32, tag="mm", name="mm")
        for j in range(8):
            w = fu.tile([128, 256], F32, tag="f2w", name="f2w")
            nc.sync.dma_start(out=w[:], in_=p.fc2_w[j * 128:(j + 1) * 128, :])
            nc.tensor.matmul(ps[:], c1T[j][:], w[:], start=(j == 0), stop=(j == 7))
        c2 = fu.tile([GPC, 256], F32, tag="c2", name="c2")
        bb2 = fu.tile([GPC, 256], F32, tag="fbb2", name="fbb2")
        nc.sync.dma_start(out=bb2[:], in_=p.fc2_b[:])
        nc.vector.tensor_tensor(out=c2[:], in0=ps[:], in1=bb2[:], op=OP.add)
        c2b = fu.tile([GPC, 256], F32, tag="c2b", name="c2b")
        nc.scalar.activation(c2b[:], c2[:], ACT.Relu)
        c2T = []
        for j in range(2):
            t = fu.tile([128, GPC], F32, tag=f"c2T{j}", name=f"c2T{j}")
            _dve_T(nc, t, c2b[:, j * 128:(j + 1) * 128], 128)
            c2T.append(t)
        ow = fu.tile([128, 2], F32, tag="ow", name="ow")
        for j in range(2):
            nc.sync.dma_start(out=ow[:, j:j + 1], in_=p.out_w[j * 128:(j + 1) * 128, :])
        ps = fup.tile([GPC, 1], F32, tag="mm", name="mm")
        for j in range(2):
            nc.tensor.matmul(ps[:], c2T[j][:], ow[:, j:j + 1],
                             start=(j == 0), stop=(j == 1))
        o = fu.tile([GPC, 1], F32, tag="o", name="o")
        nc.vector.tensor_copy(out=o[:], in_=ps[:])
        nc.sync.dma_start(out=p.out[:], in_=o[:])


# ------------------------------------------------------------------ entry
def _build_and_run(inputs, taps=()):
    T_blocks, in_maps, out_b = _host_prep(inputs)
    nc, p = build_program(T_blocks, taps=taps)
    res = run_bass_kernel_spmd(nc, in_maps, list(range(NCORES)))
    return res, out_b, p


def kernel(**inputs) -> np.ndarray:
    res, out_b, _ = _build_and_run(inputs)
    out = np.concatenate([res.results[c]["out"] for c in range(NCORES)], axis=0)
    return (out + out_b).astype(np.float32)


# revision 35
# speedup vs baseline: 1.0710x; 1.0710x over previous
"""GATNet (3x GATConv graph branch + 1D-CNN protein branch + fusion MLP) on 8
Trainium2 NeuronCores via Bass/Tile.

v2 pipeline:
  - L1 x@W replicated on every core (tiny) -> h1_full in natural node order;
    no AllGather for layer 1.
  - Layers 2/3: per-dst-block interleave of [message-pass block m] ->
    [next-layer matmul block m] -> [per-block AllGather of that block's h].
    h2/h3_full use AG-native row order (m*1024 + core*128 + r) with gather
    indices remapped on host, so each 128-row block AllGather lands
    contiguously and communication pipelines behind compute.
  - Message pass per block: per-tile indirect gathers into one SBUF strip,
    block-batched score chain (add / prelu / exp), single broadcast-AP
    multiply for per-head scaling, S^T one-hot matmuls accumulate numerator
    + denominator (ex column first), fused scalar_tensor_tensor epilogue
    (x*rec + bias), batched activation, then transpose tiles for the next
    matmul's lhsT.
  - CNN branch: embedding done on host (E shipped as input); stages
    interleaved into the GAT phases. Large head weights in bf16.

Self-contained: hardcodes all shapes; builds the per-call edge structure into
the traced program, compiles and runs via run_bass_kernel_spmd.
"""
import numpy as np
import ml_dtypes

import concourse.bass as bass
import concourse.mybir as mybir
import concourse.tile as tile
from concourse.bass_utils import run_bass_kernel_spmd
from concourse.masks import make_identity
from concourse.tile import add_dep_helper

NCORES = 8
N_NODES = 10240
N_GRAPHS = 256
NPC = N_NODES // NCORES          # 1280 nodes/core
GPC = N_GRAPHS // NCORES         # 32 graphs/core
NPG = N_NODES // N_GRAPHS        # 40 nodes/graph
BPC = NPC // 128                 # 10 dst blocks/core
NBLK = N_NODES // 128            # 80 global blocks
SEQ = 1000
VOCAB = 26
EMB = 128
NEG_SLOPE = 0.2

F32 = mybir.dt.float32
BF16 = mybir.dt.bfloat16
I32 = mybir.dt.int32
AX = mybir.AxisListType
OP = mybir.AluOpType
ACT = mybir.ActivationFunctionType

# (F_in, F_out, heads)
LAYERS = [(78, 780, 10), (780, 1560, 2), (1560, 3120, 1)]
# x@W psum chunk lists over [fo + 2H] aug cols (h | a_s | a_d)
MM_CHUNKS = [[512, 288], [512, 512, 512, 28], [512] * 6 + [50]]
# message-pass psum chunks over gs cols [ex H | h fo]
MP_CHUNKS = [[512, 278], [512, 512, 512, 26], [512] * 6 + [49]]

bf = lambda a: np.ascontiguousarray(a).astype(ml_dtypes.bfloat16)
f32 = lambda a: np.ascontiguousarray(a, dtype=np.float32)
cdiv = lambda a, b: -(-a // b)


# ------------------------------------------------------------------ walrus patch
def _split_sync_waits(nc, max_keep=1):
    for f in nc.m.functions:
        for bb in f.blocks:
            out, changed = [], False
            for ins in bb.instructions:
                si = ins.sync_info
                waits = list(si.on_wait) if si is not None and si.on_wait else []
                if len(waits) > max_keep:
                    extra, keep = waits[:-max_keep], waits[-max_keep:]
                    for i in range(0, len(extra), max_keep):
                        out.append(mybir.InstNoOp(
                            name=f"WSPLIT-{nc.next_id()}", engine=ins.engine,
                            bass_nofuse=True,
                            sync_info=mybir.SyncInfo(on_wait=extra[i:i + max_keep],
                                                     on_update=[])))
                    si.on_wait = keep
                    changed = True
                out.append(ins)
            if changed:
                bb.instructions[:] = out


# ------------------------------------------------------------------ host prep
def _edge_structure(edge_index):
    src, dst = edge_index[0].astype(np.int64), edge_index[1].astype(np.int64)
    loop = np.arange(N_NODES, dtype=np.int64)
    s_all = np.concatenate([src, loop])
    d_all = np.concatenate([dst, loop])
    order = np.argsort(d_all, kind="stable")
    s_s, d_s = s_all[order], d_all[order]

    bounds = np.searchsorted(d_s, np.arange(0, N_NODES + 1, 128))
    cnt = bounds[1:] - bounds[:-1]
    tiles_needed = -(-cnt // 128)
    T_blocks = [int(tiles_needed.reshape(NCORES, BPC)[:, p].max()) for p in range(BPC)]
    t_off = np.cumsum([0] + T_blocks)
    T_tot = int(t_off[-1])

    src_nat = np.zeros((NCORES, T_tot, 128), np.int32)
    S = np.zeros((NCORES, T_tot, 128, 128), np.float32)
    for c in range(NCORES):
        for p_ in range(BPC):
            blk = c * BPC + p_
            e0, e1 = int(bounds[blk]), int(bounds[blk + 1])
            m = e1 - e0
            ti = np.arange(m) // 128 + t_off[p_]
            ei = np.arange(m) % 128
            src_nat[c, ti, ei] = s_s[e0:e1]
            S[c, ti, ei, d_s[e0:e1] - 128 * blk] = 1.0
    # half-AG row order: half h = loc//640; row = h*5120 + core*640 + loc%640
    sv = src_nat.astype(np.int64)
    loc = sv % NPC
    src_ag = ((loc // 640) * 5120 + (sv // NPC) * 640 + loc % 640).astype(np.int32)
    ST = np.ascontiguousarray(np.swapaxes(S, 2, 3))
    natT = np.ascontiguousarray(np.swapaxes(src_nat, 1, 2))  # [8,128,T_tot]
    agT = np.ascontiguousarray(np.swapaxes(src_ag, 1, 2))
    return T_blocks, natT, agT, bf(S), f32(ST)


def _aug_w(W, a_s, a_d, H):
    """[W | W@as_blk | W@ad_blk] with as_blk[f,h] = a_s[h, f - h*FH]."""
    fi, fo = W.shape
    FH = fo // H
    was = np.zeros((fi, H), np.float32)
    wad = np.zeros((fi, H), np.float32)
    for h in range(H):
        was[:, h] = W[:, h * FH:(h + 1) * FH] @ a_s[h]
        wad[:, h] = W[:, h * FH:(h + 1) * FH] @ a_d[h]
    return np.concatenate([W, was, wad], axis=1), wad


def _host_prep(inputs):
    ii = {k: np.asarray(v) for k, v in inputs.items()}
    T_blocks, natT, agT, S, ST = _edge_structure(ii["edge_index"])

    xT = np.ascontiguousarray(np.swapaxes(f32(ii["x"]), 0, 1))   # [78, 10240]

    W_aug, wads, bias_rep = [], [], []
    for i, (fi, fo, H) in enumerate(LAYERS):
        Wa, wad = _aug_w(f32(ii[f"W{i+1}"]), f32(ii[f"as{i+1}"]),
                         f32(ii[f"ad{i+1}"]), H)
        W_aug.append(bf(Wa))
        wads.append(bf(wad))
        b = f32(ii[f"b{i+1}"]).reshape(1, -1)
        bias_rep.append(bf(np.broadcast_to(b, (128, fo))))

    cw1 = f32(ii["cw1"])
    cw1f = np.zeros((125, 8, 2, 128), np.float32)
    for sc in range(8):
        for ks in range(2):
            blk = cw1[:, sc * 125:(sc + 1) * 125, ks * 4:(ks + 1) * 4]
            cw1f[:, sc, ks, :] = blk.transpose(1, 2, 0).reshape(125, 128)
    cwT = lambda w: np.ascontiguousarray(np.transpose(f32(ii[w]), (1, 2, 0)))

    w1xt = np.ascontiguousarray(
        f32(ii["fc1_xt_w"]).reshape(128, 33, 1024).transpose(1, 0, 2))

    emb = f32(ii["emb_xt"])
    rep = lambda a, n: np.ascontiguousarray(
        np.broadcast_to(f32(a).reshape(1, -1), (n, f32(a).size)))

    shared = {
        "W1": W_aug[0], "W2": W_aug[1], "W3": W_aug[2],
        "br1": bias_rep[0], "br2": bias_rep[1], "br3": bias_rep[2],
        "wad1": wads[0],
        "fc_g1_w": bf(ii["fc_g1_w"]), "fc_g1_b": rep(ii["fc_g1_b"], GPC),
        "fc_g2_w": f32(ii["fc_g2_w"]), "fc_g2_b": rep(ii["fc_g2_b"], GPC),
        "cw1f": bf(cw1f), "cb1": f32(ii["cb1"]).reshape(-1, 1),
        "cw2T": bf(cwT("cw2")), "cb2": f32(ii["cb2"]).reshape(-1, 1),
        "cw3T": bf(cwT("cw3")), "cb3": f32(ii["cb3"]).reshape(-1, 1),
        "cw4T": bf(cwT("cw4")), "cb4": f32(ii["cb4"]).reshape(-1, 1),
        "w1xt": bf(w1xt), "fc1_xt_b": rep(ii["fc1_xt_b"], GPC),
        "fc2_xt_w": f32(ii["fc2_xt_w"]), "fc2_xt_b": rep(ii["fc2_xt_b"], GPC),
        "fc1_w": f32(ii["fc1_w"]), "fc1_b": rep(ii["fc1_b"], GPC),
        "fc2_w": f32(ii["fc2_w"]), "fc2_b": rep(ii["fc2_b"], GPC),
        "out_w": f32(ii["out_w"]),
    }
    in_maps = []
    for c in range(NCORES):
        m = dict(shared)
        m["xT"] = bf(xT)                                  # full, replicated
        m["xTl"] = bf(xT[:, c * NPC:(c + 1) * NPC])       # local slice
        m["enat"] = natT[c]
        m["eag"] = agT[c]
        m["S"] = S[c]
        m["ST"] = ST[c]
        # host-embedded CNN input: E[grp, sc, s, bloc*128+e]
        tgt_c = ii["target"][c * GPC:(c + 1) * GPC].astype(np.int64)  # [32,1000]
        E_full = emb[tgt_c]                               # [32, 1000, 128]
        Eh = E_full.reshape(8, 4, 8, 125, 128).transpose(0, 2, 3, 1, 4)
        m["Eh"] = bf(Eh.reshape(8, 8, 125, 512))
        in_maps.append(m)
    out_b = float(np.asarray(ii["out_b"]).reshape(-1)[0])
    return T_blocks, in_maps, out_b


# ------------------------------------------------------------------ program
class P:
    pass


def _aug_cols(li):
    fo, H = LAYERS[li][1], LAYERS[li][2]
    return fo + 2 * H + 2        # h | a_s(f32 as 2H bf16) | ones | pad


def build_program(T_blocks, taps=()):
    T_tot = sum(T_blocks)
    nc = bass.Bass()
    p = P()
    p.nc = nc
    p.T_blocks = T_blocks
    p.taps = set(taps)
    p.tap_tensors = {}

    dp = lambda name, shape, dt: nc.declare_dram_parameter(name, list(shape), dt,
                                                           isOutput=False)
    p.xT = dp("xT", [78, N_NODES], BF16)
    p.xTl = dp("xTl", [78, NPC], BF16)
    p.W = [dp(f"W{i+1}", [LAYERS[i][0], LAYERS[i][1] + 2 * LAYERS[i][2]], BF16)
           for i in range(3)]
    p.br = [dp(f"br{i+1}", [128, LAYERS[i][1]], BF16) for i in range(3)]
    p.wad1 = dp("wad1", [78, LAYERS[0][2]], BF16)
    p.enat = dp("enat", [128, T_tot], I32)
    p.eag = dp("eag", [128, T_tot], I32)
    p.S = dp("S", [T_tot, 128, 128], BF16)
    p.ST = dp("ST", [T_tot, 128, 128], F32)
    p.fc_g1_w = dp("fc_g1_w", [3120, 1024], BF16)
    p.fc_g1_b = dp("fc_g1_b", [GPC, 1024], F32)
    p.fc_g2_w = dp("fc_g2_w", [1024, 128], F32)
    p.fc_g2_b = dp("fc_g2_b", [GPC, 128], F32)
    p.Eh = dp("Eh", [8, 8, 125, 512], BF16)
    p.cw1f = dp("cw1f", [125, 8, 2, 128], BF16)
    p.cb1 = dp("cb1", [32, 1], F32)
    p.cw2T = dp("cw2T", [32, 8, 64], BF16)
    p.cb2 = dp("cb2", [64, 1], F32)
    p.cw3T = dp("cw3T", [64, 8, 96], BF16)
    p.cb3 = dp("cb3", [96, 1], F32)
    p.cw4T = dp("cw4T", [96, 8, 128], BF16)
    p.cb4 = dp("cb4", [128, 1], F32)
    p.w1xt = dp("w1xt", [33, 128, 1024], BF16)
    p.fc1_xt_b = dp("fc1_xt_b", [GPC, 1024], F32)
    p.fc2_xt_w = dp("fc2_xt_w", [1024, 128], F32)
    p.fc2_xt_b = dp("fc2_xt_b", [GPC, 128], F32)
    p.fc1_w = dp("fc1_w", [256, 1024], F32)
    p.fc1_b = dp("fc1_b", [GPC, 1024], F32)
    p.fc2_w = dp("fc2_w", [1024, 256], F32)
    p.fc2_b = dp("fc2_b", [GPC, 256], F32)
    p.out_w = dp("out_w", [256, 1], F32)
    p.out = nc.declare_dram_parameter("out", [GPC, 1], F32, isOutput=True)

    # h1_full natural order (replicated compute, no AG); h2/h3 AG-native order
    p.h_full = [nc.dram_tensor("h1_full", [N_NODES, _aug_cols(0)], BF16)]
    for i in (1, 2):
        p.h_full.append(nc.dram_tensor(f"h{i+1}_full", [N_NODES, _aug_cols(i)],
                                       BF16, addr_space="Shared"))
    p.h_loc = [None,
               nc.dram_tensor("h2_loc", [NPC, _aug_cols(1)], BF16),
               nc.dram_tensor("h3_loc", [NPC, _aug_cols(2)], BF16)]

    def tap(name, shape, dt=F32):
        if name in p.taps:
            t = nc.declare_dram_parameter("tap_" + name, list(shape), dt,
                                          isOutput=True)
            p.tap_tensors[name] = t
            return t
        return None
    p.tap = tap

    with tile.TileContext(nc) as tc:
        p.tc = tc
        _cp_cm = tc.tile_pool(name="const", bufs=1)
        const_pool = _cp_cm.__enter__()
        p.ident = const_pool.tile([128, 128], BF16)
        make_identity(nc, p.ident[:])
        p.head_pool = const_pool

        p.cnn = _cnn_make(p)
        _gat(p)
        _fusion(p)
        _cp_cm.__exit__(None, None, None)

    _split_sync_waits(nc)
    return nc, p


# ---------------- layer-1 replicated matmul ----------------
def _l1_mm(p):
    """Every core computes h1 for ALL nodes -> h1_full (natural order)."""
    nc, tc = p.nc, p.tc
    fi, fo, H = LAYERS[0]
    cols = _aug_cols(0)
    chunks = MM_CHUNKS[0]
    offs = [int(v) for v in np.cumsum([0] + chunks)]
    h_writes = []
    with (
        tc.tile_pool(name="l1w", bufs=1) as wpool,
        tc.tile_pool(name="l1x", bufs=1) as xpool,
        tc.tile_pool(name="l1m", bufs=3) as mpool,
        tc.tile_pool(name="l1p", bufs=2, space="PSUM") as pspool,
    ):
        W_sb = wpool.tile([fi, fo + 2 * H], BF16, tag="W", name="W")
        nc.sync.dma_start(out=W_sb[:], in_=p.W[0][:])
        xf = xpool.tile([fi, N_NODES], BF16, tag="xf", name="xf")
        nc.sync.dma_start(out=xf[:], in_=p.xT[:])
        # local a_d via xTl @ wad1
        xl = xpool.tile([fi, NPC], BF16, tag="xl", name="xl")
        nc.sync.dma_start(out=xl[:], in_=p.xTl[:])
        wad = xpool.tile([fi, H], BF16, tag="wad", name="wad")
        nc.sync.dma_start(out=wad[:], in_=p.wad1[:])
        p.a_d1 = []
        with tc.tile_pool(name="l1adp", bufs=2, space="PSUM") as adps:
            for blk in range(BPC):
                ps = adps.tile([128, H], F32, tag="ad", name="ad")
                nc.tensor.matmul(ps[:], xl[:, blk * 128:(blk + 1) * 128], wad[:],
                                 start=True, stop=True)
                t = p.head_pool.tile([128, H], F32, tag=f"a_d1_{blk}",
                                     name=f"a_d1_{blk}")
                nc.vector.tensor_copy(out=t[:], in_=ps[:])
                p.a_d1.append(t)
        for m in range(NBLK):
            stage = mpool.tile([128, cols], BF16, tag="stage", name="stage",
                               bufs=4)
            for n in range(len(chunks)):
                lo, hi = offs[n], offs[n + 1]
                ps = pspool.tile([128, chunks[n]], F32, tag=f"mp{n}", name=f"mp{n}")
                nc.tensor.matmul(ps[:], xf[:, m * 128:(m + 1) * 128],
                                 W_sb[:, lo:hi], start=True, stop=True)
                if hi <= fo:
                    nc.scalar.copy(out=stage[:, lo:hi], in_=ps[:])
                else:
                    if lo < fo:
                        nc.vector.tensor_copy(out=stage[:, lo:fo],
                                              in_=ps[:, :fo - lo])
                    a_sf = mpool.tile([128, H], F32, tag="a_sf", name="a_sf")
                    nc.vector.tensor_copy(out=a_sf[:], in_=ps[:, fo - lo:fo - lo + H])
                    nc.vector.tensor_copy(
                        out=stage[:, fo:fo + 2 * H], in_=a_sf[:].bitcast(BF16))
            if m < 4:
                oc = fo + 2 * H
                nc.vector.memset(stage[:, oc:oc + 1], 1.0)
                nc.vector.memset(stage[:, oc + 1:cols], 0.0)
            w = nc.sync.dma_start(out=p.h_full[0][m * 128:(m + 1) * 128, :],
                                  in_=stage[:])
            h_writes.append(w)
    # fence: all h1_full writes done
    fence_sb = p.head_pool.tile([1, 2], BF16, tag="fence1", name="fence1")
    fence = nc.sync.dma_start(out=fence_sb[:], in_=p.h_full[0][0:1, 0:2])
    for w in h_writes:
        add_dep_helper(fence.ins, w.ins, reason="h1 fence")
    return fence


# ---------------- message pass for one dst block ----------------
def _mp_block(p, li, blk, pools, a_d_tile, fence, xT_out):
    """Message pass layer li for dst block blk. xT_out: either a list of
    full-width tiles (write cols blk*128..) or None -> allocate per-block
    [128,128] tiles in the mp pool and return them."""
    nc, tc = p.nc, p.tc
    fi, fo, H = LAYERS[li]
    FH = fo // H
    cols = _aug_cols(li)
    gsc = H + fo                     # gs cols: [ex H | h fo]
    chunks = MP_CHUNKS[li]
    offs = [int(v) for v in np.cumsum([0] + chunks)]
    t_off = np.cumsum([0] + p.T_blocks)
    Tb = p.T_blocks[blk]
    t0 = int(t_off[blk])
    n_kT = cdiv(fo, 128)
    mp, mpS = pools["mp"], pools["mpS"]
    eidx = p.eidx_ag if li == 2 else p.eidx_nat

    S_blk = mpS.tile([128, Tb * 128], BF16, tag="Sblk", name="Sblk", bufs=2)
    nc.sync.dma_start(
        out=S_blk[:].rearrange("p (t c) -> p t c", c=128),
        in_=p.S[t0:t0 + Tb].rearrange("t p c -> p t c"))
    ST_blk = mpS.tile([128, Tb * 128], F32, tag="STblk", name="STblk", bufs=2)
    nc.sync.dma_start(
        out=ST_blk[:].rearrange("p (t c) -> p t c", c=128),
        in_=p.ST[t0:t0 + Tb].rearrange("t p c -> p t c"))

    per_tile = True
    rows_bf = mp.tile([128, fo], BF16, tag="rows_bf", name="rows_bf", bufs=2)
    rows_f = mp.tile([128, fo], F32, tag="rows_f", name="rows_f") \
        if li == 0 else rows_bf
    rec = mp.tile([128, H], F32, tag="rec", name="rec", bufs=2)
    if per_tile:
        # per-tile pipeline (H == 1): gather -> score -> scale -> matmuls
        with (
            tc.tile_pool(name=f"ap{li}_{blk}", bufs=1, space="PSUM") as pa,
            tc.tile_pool(name=f"op{li}_{blk}", bufs=1, space="PSUM") as pp,
        ):
            n_ch = len(chunks)
            opsum = [pp.tile([128, chunks[n]], F32, tag=f"op{n}", name=f"op{n}")
                     for n in range(n_ch)]
            for i in range(Tb):
                t = t0 + i
                g_t = mp.tile([128, cols], BF16, tag="g_t", name="g_t", bufs=6)
                gi = nc.gpsimd.indirect_dma_start(
                    out=g_t[:], out_offset=None, in_=p.h_full[li][:],
                    in_offset=bass.IndirectOffsetOnAxis(ap=eidx[:, t:t + 1],
                                                        axis=0))
                add_dep_helper(gi.ins, fence.ins, reason="gather waits h_full")
                adg = pa.tile([128, H], F32, tag="adg", name="adg")
                nc.tensor.matmul(adg[:], ST_blk[:, i * 128:(i + 1) * 128],
                                 a_d_tile[:], start=True, stop=True)
                sc = mp.tile([128, H], F32, tag="sc_t", name="sc_t", bufs=6)
                nc.vector.tensor_tensor(out=sc[:],
                                        in0=g_t[:, fo:fo + 2 * H].bitcast(F32),
                                        in1=adg[:], op=OP.add)
                ex = mp.tile([128, H], F32, tag="ex_t", name="ex_t", bufs=6)
                nc.scalar.activation(ex[:], sc[:], ACT.Prelu, alpha=NEG_SLOPE)
                nc.scalar.activation(ex[:], ex[:], ACT.Exp)
                gs_t = mp.tile([128, gsc], BF16, tag="gs_t", name="gs_t", bufs=4)
                nc.vector.tensor_copy(out=gs_t[:, 0:H], in_=ex[:])
                for h in range(H):
                    eng = nc.vector if h % 2 == 0 else nc.scalar
                    if h % 2 == 0:
                        nc.vector.tensor_scalar(
                            out=gs_t[:, H + h * FH:H + (h + 1) * FH],
                            in0=g_t[:, h * FH:(h + 1) * FH],
                            scalar1=ex[:, h:h + 1], scalar2=None, op0=OP.mult)
                    else:
                        nc.scalar.mul(gs_t[:, H + h * FH:H + (h + 1) * FH],
                                      g_t[:, h * FH:(h + 1) * FH],
                                      ex[:, h:h + 1])
                for n in range(n_ch):
                    nc.tensor.matmul(
                        opsum[n][:], S_blk[:, i * 128:(i + 1) * 128],
                        gs_t[:, offs[n]:offs[n + 1]],
                        start=(i == 0), stop=(i == Tb - 1))
            nc.vector.tensor_scalar(out=rec[:], in0=opsum[0][:, 0:H],
                                    scalar1=1e-16, scalar2=None, op0=OP.add)
            nc.vector.reciprocal(rec[:], rec[:])
            for n in range(n_ch):
                lo, hi = max(offs[n], H), offs[n + 1]
                a, b = lo - H, hi - H
                h0, h1 = a // FH, cdiv(b, FH)
                for h in range(h0, h1):
                    s_lo, s_hi = max(a, h * FH), min(b, (h + 1) * FH)
                    nc.vector.scalar_tensor_tensor(
                        out=rows_f[:, s_lo:s_hi],
                        in0=opsum[n][:, s_lo + H - offs[n]:s_hi + H - offs[n]],
                        scalar=rec[:, h:h + 1], in1=p.br_sb[li][:, s_lo:s_hi],
                        op0=OP.mult, op1=OP.add)
    # activation (batched, row-major)
    if li == 0:
        t1 = mp.tile([128, fo], F32, tag="elu1", name="elu1")
        nc.vector.tensor_scalar(out=t1[:], in0=rows_f[:], scalar1=0.0,
                                scalar2=None, op0=OP.min)
        nc.scalar.activation(t1[:], t1[:], ACT.Exp)
        nc.scalar.activation(rows_f[:], rows_f[:], ACT.Relu)
        nc.vector.scalar_tensor_tensor(out=rows_bf[:], in0=t1[:], scalar=-1.0,
                                       in1=rows_f[:], op0=OP.add, op1=OP.add)
    else:
        nc.scalar.activation(rows_bf[:], rows_bf[:], ACT.Relu)
    # transpose; either into full-width xT_out columns or per-block tiles
    if xT_out is None:
        xtb = [mp.tile([min(128, fo - j * 128), 128], BF16, tag=f"xtb{j}",
                       name=f"xtb{j}", bufs=2) for j in range(n_kT)]
        dst = lambda j: xtb[j][:]
    else:
        dst = lambda j: xT_out[j][:, blk * 128:(blk + 1) * 128]
    with tc.tile_pool(name=f"tp{li}_{blk}", bufs=2, space="PSUM") as ptp:
        for j in range(n_kT):
            kp = min(128, fo - j * 128)
            tp = ptp.tile([kp, 128], BF16, tag="tp", name="tp")
            nc.tensor.transpose(tp[:], rows_bf[:, j * 128:j * 128 + kp],
                                p.ident[:])
            if j % 2 == 0:
                nc.scalar.copy(out=dst(j), in_=tp[:])
            else:
                nc.vector.tensor_copy(out=dst(j), in_=tp[:])
    return None if xT_out is not None else xtb


# ---------------- next-layer matmul for one dst block ----------------
def _mm_block(p, li, blk, W_sb, xtb, pools, a_d_list):
    """x@W for layer li (1 or 2), rows of dst block blk (lhsT tiles xtb);
    writes h_loc rows. Returns the stage-write instr for the AG."""
    nc, tc = p.nc, p.tc
    fi, fo, H = LAYERS[li]
    cols = _aug_cols(li)
    n_k = cdiv(fi, 128)
    chunks = MM_CHUNKS[li]
    offs = [int(v) for v in np.cumsum([0] + chunks)]
    mp = pools["mm"]
    stage = mp.tile([128, cols], BF16, tag=f"stg{li}", name=f"stg{li}", bufs=2)
    with tc.tile_pool(name=f"mmp{li}_{blk}", bufs=4, space="PSUM") as pspool:
        for n in range(len(chunks)):
            lo, hi = offs[n], offs[n + 1]
            ps = pspool.tile([128, chunks[n]], F32, tag="mmps", name="mmps")
            for k in range(n_k):
                nc.tensor.matmul(
                    ps[:], xtb[k], W_sb[k][:, lo:hi],
                    start=(k == 0), stop=(k == n_k - 1))
            if hi <= fo:
                nc.scalar.copy(out=stage[:, lo:hi], in_=ps[:])
            else:
                if lo < fo:
                    nc.scalar.copy(out=stage[:, lo:fo], in_=ps[:, :fo - lo])
                a_sf = mp.tile([128, H], F32, tag="a_sf2", name="a_sf2")
                nc.vector.tensor_copy(out=a_sf[:], in_=ps[:, fo - lo:fo - lo + H])
                a_d = p.head_pool.tile([128, H], F32, tag=f"a_d{li}_{blk}",
                                       name=f"a_d{li}_{blk}")
                nc.vector.tensor_copy(out=a_d[:],
                                      in_=ps[:, fo - lo + H:fo - lo + 2 * H])
                a_d_list.append(a_d)
                nc.vector.tensor_copy(out=stage[:, fo:fo + 2 * H],
                                      in_=a_sf[:].bitcast(BF16))
    oc = fo + 2 * H
    nc.vector.memset(stage[:, oc:oc + 1], 1.0)
    nc.vector.memset(stage[:, oc + 1:cols], 0.0)
    w = nc.sync.dma_start(out=p.h_loc[li][blk * 128:(blk + 1) * 128, :],
                          in_=stage[:])
    return w


def _fire_ag(p, li, half, ws):
    """AllGather of one half of h_loc -> h_full rows (half-AG order)."""
    nc = p.nc
    cc = nc.gpsimd.collective_compute(
        "AllGather", OP.bypass, replica_groups=[list(range(NCORES))],
        ins=[p.h_loc[li][half * 640:(half + 1) * 640, :]],
        outs=[p.h_full[li][half * 5120:(half + 1) * 5120, :]])
    for w in ws:
        add_dep_helper(cc.ins, w.ins, reason="AG waits h_loc writes")
    return cc


# ---------------- GAT orchestration ----------------
def _gat(p):
    nc, tc = p.nc, p.tc
    T_tot = sum(p.T_blocks)

    mpc_cm = tc.tile_pool(name="mpc", bufs=1)
    mpc = mpc_cm.__enter__()
    p.eidx_nat = mpc.tile([128, T_tot], I32, tag="enat", name="enat")
    nc.sync.dma_start(out=p.eidx_nat[:], in_=p.enat[:])
    p.eidx_ag = mpc.tile([128, T_tot], I32, tag="eag", name="eag")
    nc.sync.dma_start(out=p.eidx_ag[:], in_=p.eag[:])
    p.br_sb = []
    for li in range(3):
        t = mpc.tile([128, LAYERS[li][1]], BF16, tag=f"br{li}", name=f"br{li}")
        nc.sync.dma_start(out=t[:], in_=p.br[li][:])
        p.br_sb.append(t)

    fence1 = _l1_mm(p)
    p.cnn["stage1_w"]()     # CNN weight loads early

    # ---- phase 1: MP L1 (all blocks) -> dense mm L2 with mid-pass AG ----
    a_d2 = []
    ccs2 = []
    xT2_cm = tc.tile_pool(name="xT2", bufs=1)
    xT2p = xT2_cm.__enter__()
    fo2 = LAYERS[0][1]
    n_kT2 = cdiv(fo2, 128)
    xT2 = [xT2p.tile([min(128, fo2 - j * 128), NPC], BF16, tag=f"xT2_{j}",
                     name=f"xT2_{j}") for j in range(n_kT2)]
    with (
        tc.tile_pool(name="w2", bufs=1) as w2pool,
        tc.tile_pool(name="mp1", bufs=1) as mp1,
        tc.tile_pool(name="mpS1", bufs=1) as mpS1,
        tc.tile_pool(name="mm1", bufs=1) as mm1,
    ):
        fi2, fo2b, H2 = LAYERS[1]
        W2_sb = []
        for k in range(cdiv(fi2, 128)):
            kp = min(128, fi2 - k * 128)
            t = w2pool.tile([kp, fo2b + 2 * H2], BF16, tag=f"W2_{k}",
                            name=f"W2_{k}")
            nc.sync.dma_start(out=t[:], in_=p.W[1][k * 128:k * 128 + kp, :])
            W2_sb.append(t)
        pools = {"mp": mp1, "mpS": mpS1, "mm": mm1}
        for blk in range(BPC):
            _mp_block(p, 0, blk, pools, p.a_d1[blk], fence1, xT2)
        ws = []
        for blk in range(BPC):
            xts = [xT2[k][:min(128, fo2 - k * 128), blk * 128:(blk + 1) * 128]
                   for k in range(n_kT2)]
            ws.append(_mm_block(p, 1, blk, W2_sb, xts, pools, a_d2))
        cc = nc.gpsimd.collective_compute(
            "AllGather", OP.bypass, replica_groups=[list(range(NCORES))],
            ins=[p.h_loc[1][:]], outs=[p.h_full[1][:]])
        for w in ws:
            add_dep_helper(cc.ins, w.ins, reason="AG2 waits h_loc writes")
        ccs2.append(cc)
        for g in range(8):
            p.cnn["stage1_grp"](g)
        p.cnn["stage2"]()
        p.cnn["stage3"]()
    xT2_cm.__exit__(None, None, None)
    p.cnn["cleanup"]()
    fence2_sb = p.head_pool.tile([1, 2], BF16, tag="fence2", name="fence2")
    fence2 = nc.sync.dma_start(out=fence2_sb[:], in_=p.h_full[1][0:1, 0:2])
    for cc in ccs2:
        add_dep_helper(fence2.ins, cc.ins, reason="h2 fence")

    # ---- phase 2: MP L2 (all blocks) -> dense mm L3 with mid-pass AG ----
    a_d3 = []
    ccs3 = []
    xT3_cm = tc.tile_pool(name="xT3", bufs=1)
    xT3p = xT3_cm.__enter__()
    fo3 = LAYERS[1][1]
    n_kT3 = cdiv(fo3, 128)
    xT3 = [xT3p.tile([min(128, fo3 - j * 128), NPC], BF16, tag=f"xT3_{j}",
                     name=f"xT3_{j}") for j in range(n_kT3)]
    with (
        tc.tile_pool(name="w3", bufs=1) as w3pool,
        tc.tile_pool(name="mp2", bufs=1) as mp2,
        tc.tile_pool(name="mpS2", bufs=1) as mpS2,
        tc.tile_pool(name="mm2", bufs=1) as mm2,
    ):
        fi3, fo3b, H3 = LAYERS[2]
        W3_sb = []
        for k in range(cdiv(fi3, 128)):
            kp = min(128, fi3 - k * 128)
            t = w3pool.tile([kp, fo3b + 2 * H3], BF16, tag=f"W3_{k}",
                            name=f"W3_{k}")
            nc.sync.dma_start(out=t[:], in_=p.W[2][k * 128:k * 128 + kp, :])
            W3_sb.append(t)
        pools = {"mp": mp2, "mpS": mpS2, "mm": mm2}
        for blk in range(BPC):
            _mp_block(p, 1, blk, pools, a_d2[blk], fence2, xT3)
        ws = []
        for blk in range(BPC):
            xts = [xT3[k][:min(128, fo3 - k * 128), blk * 128:(blk + 1) * 128]
                   for k in range(n_kT3)]
            ws.append(_mm_block(p, 2, blk, W3_sb, xts, pools, a_d3))
            if blk == 4:
                ccs3.append(_fire_ag(p, 2, 0, ws))
        ccs3.append(_fire_ag(p, 2, 1, ws[5:]))
    xT3_cm.__exit__(None, None, None)
    fence3_sb = p.head_pool.tile([1, 2], BF16, tag="fence3", name="fence3")
    fence3 = nc.sync.dma_start(out=fence3_sb[:], in_=p.h_full[2][0:1, 0:2])
    for cc in ccs3:
        add_dep_helper(fence3.ins, cc.ins, reason="h3 fence")

    # ---- phase 3: MP L3 (+ CNN stages 2/3) ----
    xT4_cm = tc.tile_pool(name="xT4", bufs=1)
    xT4p = xT4_cm.__enter__()
    fo4 = LAYERS[2][1]
    p.out3T = [xT4p.tile([min(128, fo4 - j * 128), NPC], BF16, tag=f"xT4_{j}",
                         name=f"xT4_{j}") for j in range(cdiv(fo4, 128))]
    with (
        tc.tile_pool(name="mp3", bufs=1) as mp3,
        tc.tile_pool(name="mpS3", bufs=1) as mpS3,
    ):
        pools = {"mp": mp3, "mpS": mpS3, "mm": None}
        for blk in range(BPC):
            _mp_block(p, 2, blk, pools, a_d3[blk], fence3, p.out3T)
    p.gat_cleanup = [xT4_cm, mpc_cm]

    t = p.tap("o3T", [fo4, NPC], BF16)
    if t is not None:
        for j in range(cdiv(fo4, 128)):
            kp = min(128, fo4 - j * 128)
            nc.sync.dma_start(out=t[j * 128:j * 128 + kp, :], in_=p.out3T[j][:])


def _dve_T(nc, dst, src, n):
    """dst[n, 32] = src[32, n].T via DVE 32x32 block transposes (f32)."""
    for i in range(n // 32):
        nc.vector.transpose(out=dst[32 * i:32 * (i + 1), :],
                            in_=src[:, 32 * i:32 * (i + 1)])


# ---------------- graph head ----------------
def _graph_head(p):
    nc, tc = p.nc, p.tc
    n_kT = len(p.out3T)
    with (
        tc.tile_pool(name="gh", bufs=2) as gh,
        tc.tile_pool(name="ghG", bufs=1) as ghG,
        tc.tile_pool(name="ghp", bufs=2, space="PSUM") as ghp,
    ):
        gT = [ghG.tile([min(128, 3120 - j * 128), GPC], BF16, tag=f"gT{j}",
                       name=f"gT{j}") for j in range(n_kT)]
        for j in range(n_kT):
            nc.vector.reduce_max(
                gT[j][:],
                p.out3T[j][:].rearrange("p (g n) -> p g n", n=NPG),
                axis=AX.X)
        g1 = ghG.tile([GPC, 1024], F32, tag="g1", name="g1")
        for n in range(2):
            ps = ghp.tile([GPC, 512], F32, tag="mm", name="mm")
            for j in range(n_kT):
                kp = min(128, 3120 - j * 128)
                w = gh.tile([kp, 512], BF16, tag="fg1w", name="fg1w")
                nc.sync.dma_start(out=w[:], in_=p.fc_g1_w[j * 128:j * 128 + kp,
                                                          n * 512:(n + 1) * 512])
                nc.tensor.matmul(ps[:], gT[j][:], w[:], start=(j == 0),
                                 stop=(j == n_kT - 1))
            nc.vector.tensor_copy(out=g1[:, n * 512:(n + 1) * 512], in_=ps[:])
        bb1 = gh.tile([GPC, 1024], F32, tag="ghbb", name="ghbb")
        nc.sync.dma_start(out=bb1[:], in_=p.fc_g1_b[:])
        nc.vector.tensor_tensor(out=g1[:], in0=g1[:], in1=bb1[:], op=OP.add)
        g1b = ghG.tile([GPC, 1024], F32, tag="g1b", name="g1b")
        nc.scalar.activation(g1b[:], g1[:], ACT.Relu)
        g1T = [ghG.tile([128, GPC], F32, tag=f"g1T{j}", name=f"g1T{j}")
               for j in range(8)]
        for j in range(8):
            _dve_T(nc, g1T[j], g1b[:, j * 128:(j + 1) * 128], 128)
        ps = ghp.tile([GPC, 128], F32, tag="mm", name="mm")
        for j in range(8):
            w = gh.tile([128, 128], F32, tag="fg2w", name="fg2w")
            nc.sync.dma_start(out=w[:], in_=p.fc_g2_w[j * 128:(j + 1) * 128, :])
            nc.tensor.matmul(ps[:], g1T[j][:], w[:], start=(j == 0), stop=(j == 7))
        p.g2 = p.head_pool.tile([GPC, 128], F32, tag="g2", name="g2")
        bb2 = gh.tile([GPC, 128], F32, tag="ghbb2", name="ghbb2")
        nc.sync.dma_start(out=bb2[:], in_=p.fc_g2_b[:])
        nc.vector.tensor_tensor(out=p.g2[:], in0=ps[:], in1=bb2[:], op=OP.add)
        t = p.tap("g2", [GPC, 128])
        if t is not None:
            nc.sync.dma_start(out=t[:], in_=p.g2[:])


# ---------------- CNN branch ----------------
def _cnn_make(p):
    """CNN branch split into stages interleaved into the GAT phases."""
    nc, tc = p.nc, p.tc
    st = {}

    def stage1_w():
        cn_cm = tc.tile_pool(name="cn", bufs=2)
        cnw_cm = tc.tile_pool(name="cnw", bufs=1)
        cny_cm = tc.tile_pool(name="cny", bufs=1)
        cn = cn_cm.__enter__()
        cnw = cnw_cm.__enter__()
        cny = cny_cm.__enter__()
        st["cms"] = [cny_cm, cnw_cm, cn_cm]
        st["cn"], st["cnw"], st["cny"] = cn, cnw, cny
        cw1f_sb = cnw.tile([125, 8, 2, 128], BF16, tag="cw1f", name="cw1f")
        nc.sync.dma_start(out=cw1f_sb[:], in_=p.cw1f[:])
        cw2_sb = cnw.tile([32, 8, 64], BF16, tag="cw2", name="cw2")
        nc.sync.dma_start(out=cw2_sb[:], in_=p.cw2T[:])
        cw3_sb = cnw.tile([64, 8, 96], BF16, tag="cw3", name="cw3")
        nc.sync.dma_start(out=cw3_sb[:], in_=p.cw3T[:])
        cw4_sb = cnw.tile([96, 8, 128], BF16, tag="cw4", name="cw4")
        nc.sync.dma_start(out=cw4_sb[:], in_=p.cw4T[:])
        cb = {}
        for nm, sh in [("cb1", 32), ("cb2", 64), ("cb3", 96), ("cb4", 128)]:
            cb[nm] = cnw.tile([sh, 1], F32, tag=nm, name=nm)
            nc.sync.dma_start(out=cb[nm][:], in_=getattr(p, nm)[:])
        st.update(cw1f=cw1f_sb, cw2=cw2_sb, cw3=cw3_sb, cw4=cw4_sb, cb=cb)
        y1 = cny.tile([32, GPC * 121], BF16, tag="y1", name="y1")
        st["y1"] = y1

    def stage1_grp(grp):
        cn, cb = st["cn"], st["cb"]
        cw1f_sb = st["cw1f"]
        y1 = st["y1"]
        with tc.tile_pool(name=f"cnp1_{grp}", bufs=1, space="PSUM") as cnp:
            pc = [cnp.tile([128, 512], F32, tag=f"pc{k}", name=f"pc{k}")
                  for k in range(2)]
            for sc in range(8):
                E = cn.tile([125, 512], BF16, tag="E", name="E")
                nc.sync.dma_start(out=E[:], in_=p.Eh[grp, sc])
                for ks in range(2):
                    nc.tensor.matmul(pc[ks][:], cw1f_sb[:, sc, ks, :], E[:],
                                     start=(sc == 0), stop=(sc == 7))
            acc = cn.tile([32, 4 * 121], F32, tag="c1acc", name="c1acc", bufs=1)
            accr = acc[:].rearrange("p (b t) -> p b t", b=4)
            firstop = True
            for ks in range(2):
                for kl in range(4):
                    k = ks * 4 + kl
                    src = pc[ks][:].rearrange("p (b j) -> p b j", b=4)[
                        kl * 32:(kl + 1) * 32, :, k:k + 121]
                    if firstop:
                        nc.vector.tensor_copy(out=accr, in_=src)
                        firstop = False
                    else:
                        nc.vector.tensor_tensor(out=accr, in0=accr, in1=src,
                                                op=OP.add)
            nc.scalar.activation(y1[:, grp * 4 * 121:(grp + 1) * 4 * 121],
                                 acc[:], ACT.Relu, bias=cb["cb1"][:32, :1])

    def stage2():
        cn, cny, cb = st["cn"], st["cny"], st["cb"]
        cw2_sb, cw3_sb, cw4_sb = st["cw2"], st["cw3"], st["cw4"]
        y1 = st["y1"]
        with tc.tile_pool(name="cnp2", bufs=2, space="PSUM") as cnp:
            y2 = cny.tile([64, GPC * 114], BF16, tag="y2", name="y2")
            for grp in range(8):
                ps = cnp.tile([64, 4 * 114], F32, tag="pc0", name="pc0")
                for k in range(8):
                    rhs = y1[:].rearrange("p (b t) -> p b t", t=121)[
                        :, grp * 4:(grp + 1) * 4, k:k + 114]
                    nc.tensor.matmul(ps[:], cw2_sb[:, k, :], rhs, start=(k == 0),
                                     stop=(k == 7))
                nc.scalar.activation(y2[:, grp * 4 * 114:(grp + 1) * 4 * 114],
                                     ps[:], ACT.Relu, bias=cb["cb2"][:, :1])
            y3 = cny.tile([96, GPC * 107], BF16, tag="y3", name="y3")
            for grp in range(8):
                ps = cnp.tile([96, 4 * 107], F32, tag="pc0", name="pc0")
                for k in range(8):
                    rhs = y2[:].rearrange("p (b t) -> p b t", t=114)[
                        :, grp * 4:(grp + 1) * 4, k:k + 107]
                    nc.tensor.matmul(ps[:], cw3_sb[:, k, :], rhs, start=(k == 0),
                                     stop=(k == 7))
                nc.scalar.activation(y3[:, grp * 4 * 107:(grp + 1) * 4 * 107],
                                     ps[:], ACT.Relu, bias=cb["cb3"][:, :1])
            yp = cny.tile([128, GPC * 33], BF16, tag="yp", name="yp")
            st["yp"] = yp
            for grp in range(8):
                ps = cnp.tile([128, 4 * 100], F32, tag="pc0", name="pc0")
                for k in range(8):
                    rhs = y3[:].rearrange("p (b t) -> p b t", t=107)[
                        :, grp * 4:(grp + 1) * 4, k:k + 100]
                    nc.tensor.matmul(ps[:], cw4_sb[:, k, :], rhs, start=(k == 0),
                                     stop=(k == 7))
                psr = ps[:].rearrange("p (b t) -> p b t", b=4)
                mx = cn.tile([128, 4 * 33], F32, tag="mx", name="mx")
                mxr = mx[:].rearrange("p (b t) -> p b t", b=4)
                nc.vector.tensor_copy(out=mxr, in_=psr[:, :, 0:99:3])
                nc.vector.tensor_tensor(out=mxr, in0=mxr, in1=psr[:, :, 1:100:3],
                                        op=OP.max)
                nc.vector.tensor_tensor(out=mxr, in0=mxr, in1=psr[:, :, 2:100:3],
                                        op=OP.max)
                nc.scalar.activation(yp[:, grp * 4 * 33:(grp + 1) * 4 * 33],
                                     mx[:], ACT.Relu, bias=cb["cb4"][:, :1])

    def stage3():
        cn, cny = st["cn"], st["cny"]
        yp = st["yp"]
        with tc.tile_pool(name="cnp3", bufs=2, space="PSUM") as cnp:
            xt1 = cny.tile([GPC, 1024], F32, tag="xt1", name="xt1")
            for n in range(2):
                ps = cnp.tile([GPC, 512], F32, tag="pc0", name="pc0")
                for t_ in range(33):
                    w = cn.tile([128, 512], BF16, tag="fx1w", name="fx1w", bufs=2)
                    nc.sync.dma_start(out=w[:],
                                      in_=p.w1xt[t_, :, n * 512:(n + 1) * 512])
                    lhs = yp[:].rearrange("p (b t) -> p t b", t=33)[:, t_, :]
                    nc.tensor.matmul(ps[:], lhs, w[:], start=(t_ == 0),
                                     stop=(t_ == 32))
                nc.vector.tensor_copy(out=xt1[:, n * 512:(n + 1) * 512], in_=ps[:])
            bb = cn.tile([GPC, 1024], F32, tag="fxbb", name="fxbb", bufs=1)
            nc.sync.dma_start(out=bb[:], in_=p.fc1_xt_b[:])
            nc.vector.tensor_tensor(out=xt1[:], in0=xt1[:], in1=bb[:], op=OP.add)
            nc.scalar.activation(xt1[:], xt1[:], ACT.Relu)
            xt1T = [cn.tile([128, GPC], F32, tag=f"xt1T{j}", name=f"xt1T{j}",
                            bufs=1)
                    for j in range(8)]
            for j in range(8):
                _dve_T(nc, xt1T[j], xt1[:, j * 128:(j + 1) * 128], 128)
            ps = cnp.tile([GPC, 128], F32, tag="pc0", name="pc0")
            for j in range(8):
                w = cn.tile([128, 128], F32, tag="fx2w", name="fx2w", bufs=2)
                nc.sync.dma_start(out=w[:], in_=p.fc2_xt_w[j * 128:(j + 1) * 128, :])
                nc.tensor.matmul(ps[:], xt1T[j][:], w[:], start=(j == 0),
                                 stop=(j == 7))
            p.xt2 = p.head_pool.tile([GPC, 128], F32, tag="xt2", name="xt2")
            bb2 = cn.tile([GPC, 128], F32, tag="fxbb2", name="fxbb2", bufs=1)
            nc.sync.dma_start(out=bb2[:], in_=p.fc2_xt_b[:])
            nc.vector.tensor_tensor(out=p.xt2[:], in0=ps[:], in1=bb2[:], op=OP.add)
            t = p.tap("xt2", [GPC, 128])
            if t is not None:
                nc.sync.dma_start(out=t[:], in_=p.xt2[:])

    def cleanup():
        for cm in st["cms"]:
            cm.__exit__(None, None, None)

    return {"stage1_w": stage1_w, "stage1_grp": stage1_grp, "stage2": stage2,
            "stage3": stage3, "cleanup": cleanup}


# ---------------- fusion ----------------
def _fusion(p):
    nc, tc = p.nc, p.tc
    _graph_head(p)
    for cm in p.gat_cleanup:
        cm.__exit__(None, None, None)
    with (
        tc.tile_pool(name="fu", bufs=2) as fu,
        tc.tile_pool(name="fup", bufs=2, space="PSUM") as fup,
    ):
        xcT = []
        for src_ in (p.g2, p.xt2):
            t = fu.tile([128, GPC], F32, tag=f"xcT{len(xcT)}", name=f"xcT{len(xcT)}")
            _dve_T(nc, t, src_[:], 128)
            xcT.append(t)
        c1 = fu.tile([GPC, 1024], F32, tag="c1", name="c1")
        for n in range(2):
            ps = fup.tile([GPC, 512], F32, tag="mm", name="mm")
            for j in range(2):
                w = fu.tile([128, 512], F32, tag="f1w", name="f1w")
                nc.sync.dma_start(out=w[:], in_=p.fc1_w[j * 128:(j + 1) * 128,
                                                        n * 512:(n + 1) * 512])
                nc.tensor.matmul(ps[:], xcT[j][:], w[:], start=(j == 0),
                                 stop=(j == 1))
            nc.vector.tensor_copy(out=c1[:, n * 512:(n + 1) * 512], in_=ps[:])
        bb = fu.tile([GPC, 1024], F32, tag="fbb", name="fbb")
        nc.sync.dma_start(out=bb[:], in_=p.fc1_b[:])
        nc.vector.tensor_tensor(out=c1[:], in0=c1[:], in1=bb[:], op=OP.add)
        c1b = fu.tile([GPC, 1024], F32, tag="c1b", name="c1b")
        nc.scalar.activation(c1b[:], c1[:], ACT.Relu)
        c1T = [fu.tile([128, GPC], F32, tag=f"c1T{j}", name=f"c1T{j}") for j in range(8)]
        for j in range(8):
            _dve_T(nc, c1T[j], c1b[:, j * 128:(j + 1) * 128], 128)
        ps = fup.tile([GPC, 256], F32, tag="mm", name="mm")
        for j in range(8):
            w = fu.tile([128, 256], F32, tag="f2w", name="f2w")
            nc.sync.dma_start(out=w[:], in_=p.fc2_w[j * 128:(j + 1) * 128, :])
            nc.tensor.matmul(ps[:], c1T[j][:], w[:], start=(j == 0), stop=(j == 7))
        c2 = fu.tile([GPC, 256], F32, tag="c2", name="c2")
        bb2 = fu.tile([GPC, 256], F32, tag="fbb2", name="fbb2")
        nc.sync.dma_start(out=bb2[:], in_=p.fc2_b[:])
        nc.vector.tensor_tensor(out=c2[:], in0=ps[:], in1=bb2[:], op=OP.add)
        c2b = fu.tile([GPC, 256], F32, tag="c2b", name="c2b")
        nc.scalar.activation(c2b[:], c2[:], ACT.Relu)
        c2T = []
        for j in range(2):
            t = fu.tile([128, GPC], F32, tag=f"c2T{j}", name=f"c2T{j}")
            _dve_T(nc, t, c2b[:, j * 128:(j + 1) * 128], 128)
            c2T.append(t)
        ow = fu.tile([128, 2], F32, tag="ow", name="ow")
        for j in range(2):
            nc.sync.dma_start(out=ow[:, j:j + 1], in_=p.out_w[j * 128:(j + 1) * 128, :])
        ps = fup.tile([GPC, 1], F32, tag="mm", name="mm")
        for j in range(2):
            nc.tensor.matmul(ps[:], c2T[j][:], ow[:, j:j + 1],
                             start=(j == 0), stop=(j == 1))
        o = fu.tile([GPC, 1], F32, tag="o", name="o")
        nc.vector.tensor_copy(out=o[:], in_=ps[:])
        nc.sync.dma_start(out=p.out[:], in_=o[:])


# ------------------------------------------------------------------ entry
def _build_and_run(inputs, taps=()):
    T_blocks, in_maps, out_b = _host_prep(inputs)
    nc, p = build_program(T_blocks, taps=taps)
    res = run_bass_kernel_spmd(nc, in_maps, list(range(NCORES)))
    return res, out_b, p


def kernel(**inputs) -> np.ndarray:
    res, out_b, _ = _build_and_run(inputs)
    out = np.concatenate([res.results[c]["out"] for c in range(NCORES)], axis=0)
    return (out + out_b).astype(np.float32)


# revision 36
# speedup vs baseline: 1.2198x; 1.1390x over previous
"""GATNet (3x GATConv graph branch + 1D-CNN protein branch + fusion MLP) on 8
Trainium2 NeuronCores via Bass/Tile.

v2 pipeline:
  - L1 x@W replicated on every core (tiny) -> h1_full in natural node order;
    no AllGather for layer 1.
  - Layers 2/3: per-dst-block interleave of [message-pass block m] ->
    [next-layer matmul block m] -> [per-block AllGather of that block's h].
    h2/h3_full use AG-native row order (m*1024 + core*128 + r) with gather
    indices remapped on host, so each 128-row block AllGather lands
    contiguously and communication pipelines behind compute.
  - Message pass per block: per-tile indirect gathers into one SBUF strip,
    block-batched score chain (add / prelu / exp), single broadcast-AP
    multiply for per-head scaling, S^T one-hot matmuls accumulate numerator
    + denominator (ex column first), fused scalar_tensor_tensor epilogue
    (x*rec + bias), batched activation, then transpose tiles for the next
    matmul's lhsT.
  - CNN branch: embedding done on host (E shipped as input); stages
    interleaved into the GAT phases. Large head weights in bf16.

Self-contained: hardcodes all shapes; builds the per-call edge structure into
the traced program, compiles and runs via run_bass_kernel_spmd.
"""
import numpy as np
import ml_dtypes

import concourse.bass as bass
import concourse.mybir as mybir
import concourse.tile as tile
from concourse.bass_utils import run_bass_kernel_spmd
from concourse.masks import make_identity
from concourse.tile import add_dep_helper

NCORES = 8
N_NODES = 10240
N_GRAPHS = 256
NPC = N_NODES // NCORES          # 1280 nodes/core
GPC = N_GRAPHS // NCORES         # 32 graphs/core
NPG = N_NODES // N_GRAPHS        # 40 nodes/graph
BPC = NPC // 128                 # 10 dst blocks/core
NBLK = N_NODES // 128            # 80 global blocks
SEQ = 1000
VOCAB = 26
EMB = 128
NEG_SLOPE = 0.2

F32 = mybir.dt.float32
BF16 = mybir.dt.bfloat16
I32 = mybir.dt.int32
AX = mybir.AxisListType
OP = mybir.AluOpType
ACT = mybir.ActivationFunctionType

# (F_in, F_out, heads)
LAYERS = [(78, 780, 10), (780, 1560, 2), (1560, 3120, 1)]
# x@W psum chunk lists over [fo + 2H] aug cols (h | a_s | a_d)
MM_CHUNKS = [[512, 288], [512, 512, 512, 28], [512] * 6 + [50]]
# message-pass psum chunks over gs cols [ex H | h fo]
MP_CHUNKS = [[512, 278], [512, 512, 512, 26], [512] * 6 + [49]]

bf = lambda a: np.ascontiguousarray(a).astype(ml_dtypes.bfloat16)
f32 = lambda a: np.ascontiguousarray(a, dtype=np.float32)
cdiv = lambda a, b: -(-a // b)


# ------------------------------------------------------------------ walrus patch
def _split_sync_waits(nc, max_keep=1):
    for f in nc.m.functions:
        for bb in f.blocks:
            out, changed = [], False
            for ins in bb.instructions:
                si = ins.sync_info
                waits = list(si.on_wait) if si is not None and si.on_wait else []
                if len(waits) > max_keep:
                    extra, keep = waits[:-max_keep], waits[-max_keep:]
                    for i in range(0, len(extra), max_keep):
                        out.append(mybir.InstNoOp(
                            name=f"WSPLIT-{nc.next_id()}", engine=ins.engine,
                            bass_nofuse=True,
                            sync_info=mybir.SyncInfo(on_wait=extra[i:i + max_keep],
                                                     on_update=[])))
                    si.on_wait = keep
                    changed = True
                out.append(ins)
            if changed:
                bb.instructions[:] = out


# ------------------------------------------------------------------ host prep
def _edge_structure(edge_index):
    src, dst = edge_index[0].astype(np.int64), edge_index[1].astype(np.int64)
    loop = np.arange(N_NODES, dtype=np.int64)
    s_all = np.concatenate([src, loop])
    d_all = np.concatenate([dst, loop])
    order = np.argsort(d_all, kind="stable")
    s_s, d_s = s_all[order], d_all[order]

    bounds = np.searchsorted(d_s, np.arange(0, N_NODES + 1, 128))
    cnt = bounds[1:] - bounds[:-1]
    tiles_needed = -(-cnt // 128)
    T_blocks = [int(tiles_needed.reshape(NCORES, BPC)[:, p].max()) for p in range(BPC)]
    t_off = np.cumsum([0] + T_blocks)
    T_tot = int(t_off[-1])

    src_nat = np.zeros((NCORES, T_tot, 128), np.int32)
    S = np.zeros((NCORES, T_tot, 128, 128), np.float32)
    for c in range(NCORES):
        for p_ in range(BPC):
            blk = c * BPC + p_
            e0, e1 = int(bounds[blk]), int(bounds[blk + 1])
            m = e1 - e0
            ti = np.arange(m) // 128 + t_off[p_]
            ei = np.arange(m) % 128
            src_nat[c, ti, ei] = s_s[e0:e1]
            S[c, ti, ei, d_s[e0:e1] - 128 * blk] = 1.0
    # half-AG row order: half h = loc//640; row = h*5120 + core*640 + loc%640
    sv = src_nat.astype(np.int64)
    loc = sv % NPC
    src_ag = ((loc // 640) * 5120 + (sv // NPC) * 640 + loc % 640).astype(np.int32)
    ST = np.ascontiguousarray(np.swapaxes(S, 2, 3))
    natT = np.ascontiguousarray(np.swapaxes(src_nat, 1, 2))  # [8,128,T_tot]
    agT = np.ascontiguousarray(np.swapaxes(src_ag, 1, 2))
    return T_blocks, natT, agT, bf(S), f32(ST)


def _aug_w(W, a_s, a_d, H):
    """[W | W@as_blk | W@ad_blk] with as_blk[f,h] = a_s[h, f - h*FH]."""
    fi, fo = W.shape
    FH = fo // H
    was = np.zeros((fi, H), np.float32)
    wad = np.zeros((fi, H), np.float32)
    for h in range(H):
        was[:, h] = W[:, h * FH:(h + 1) * FH] @ a_s[h]
        wad[:, h] = W[:, h * FH:(h + 1) * FH] @ a_d[h]
    return np.concatenate([W, was, wad], axis=1), wad


def _host_prep(inputs):
    ii = {k: np.asarray(v) for k, v in inputs.items()}
    T_blocks, natT, agT, S, ST = _edge_structure(ii["edge_index"])

    xT = np.ascontiguousarray(np.swapaxes(f32(ii["x"]), 0, 1))   # [78, 10240]

    W_aug, wads, bias_rep = [], [], []
    for i, (fi, fo, H) in enumerate(LAYERS):
        Wa, wad = _aug_w(f32(ii[f"W{i+1}"]), f32(ii[f"as{i+1}"]),
                         f32(ii[f"ad{i+1}"]), H)
        W_aug.append(bf(Wa))
        wads.append(bf(wad))
        b = f32(ii[f"b{i+1}"]).reshape(1, -1)
        bias_rep.append(bf(np.broadcast_to(b, (128, fo))))

    cw1 = f32(ii["cw1"])
    cw1f = np.zeros((125, 8, 2, 128), np.float32)
    for sc in range(8):
        for ks in range(2):
            blk = cw1[:, sc * 125:(sc + 1) * 125, ks * 4:(ks + 1) * 4]
            cw1f[:, sc, ks, :] = blk.transpose(1, 2, 0).reshape(125, 128)
    cwT = lambda w: np.ascontiguousarray(np.transpose(f32(ii[w]), (1, 2, 0)))

    w1xt = np.ascontiguousarray(
        f32(ii["fc1_xt_w"]).reshape(128, 33, 1024).transpose(1, 0, 2))

    emb = f32(ii["emb_xt"])
    rep = lambda a, n: np.ascontiguousarray(
        np.broadcast_to(f32(a).reshape(1, -1), (n, f32(a).size)))

    shared = {
        "W1": W_aug[0], "W2": W_aug[1], "W3": W_aug[2],
        "br1": bias_rep[0], "br2": bias_rep[1], "br3": bias_rep[2],
        "wad1": wads[0],
        "fc_g1_w": bf(ii["fc_g1_w"]), "fc_g1_b": rep(ii["fc_g1_b"], GPC),
        "fc_g2_w": f32(ii["fc_g2_w"]), "fc_g2_b": rep(ii["fc_g2_b"], GPC),
        "cw1f": bf(cw1f), "cb1": f32(ii["cb1"]).reshape(-1, 1),
        "cw2T": bf(cwT("cw2")), "cb2": f32(ii["cb2"]).reshape(-1, 1),
        "cw3T": bf(cwT("cw3")), "cb3": f32(ii["cb3"]).reshape(-1, 1),
        "cw4T": bf(cwT("cw4")), "cb4": f32(ii["cb4"]).reshape(-1, 1),
        "w1xt": bf(w1xt), "fc1_xt_b": rep(ii["fc1_xt_b"], GPC),
        "fc2_xt_w": f32(ii["fc2_xt_w"]), "fc2_xt_b": rep(ii["fc2_xt_b"], GPC),
        "fc1_w": f32(ii["fc1_w"]), "fc1_b": rep(ii["fc1_b"], GPC),
        "fc2_w": f32(ii["fc2_w"]), "fc2_b": rep(ii["fc2_b"], GPC),
        "out_w": f32(ii["out_w"]),
    }
    in_maps = []
    for c in range(NCORES):
        m = dict(shared)
        m["xT"] = bf(xT)                                  # full, replicated
        m["xTl"] = bf(xT[:, c * NPC:(c + 1) * NPC])       # local slice
        m["enat"] = natT[c]
        m["eag"] = agT[c]
        m["S"] = S[c]
        m["ST"] = ST[c]
        # host-embedded CNN input: E[grp, sc, s, bloc*128+e]
        tgt_c = ii["target"][c * GPC:(c + 1) * GPC].astype(np.int64)  # [32,1000]
        E_full = emb[tgt_c]                               # [32, 1000, 128]
        Eh = E_full.reshape(8, 4, 8, 125, 128).transpose(0, 2, 3, 1, 4)
        m["Eh"] = bf(Eh.reshape(8, 8, 125, 512))
        in_maps.append(m)
    out_b = float(np.asarray(ii["out_b"]).reshape(-1)[0])
    return T_blocks, in_maps, out_b


# ------------------------------------------------------------------ program
class P:
    pass


def _aug_cols(li):
    fo, H = LAYERS[li][1], LAYERS[li][2]
    return fo + 2 * H + 2        # h | a_s(f32 as 2H bf16) | ones | pad


def build_program(T_blocks, taps=()):
    T_tot = sum(T_blocks)
    nc = bass.Bass()
    p = P()
    p.nc = nc
    p.T_blocks = T_blocks
    p.taps = set(taps)
    p.tap_tensors = {}

    dp = lambda name, shape, dt: nc.declare_dram_parameter(name, list(shape), dt,
                                                           isOutput=False)
    p.xT = dp("xT", [78, N_NODES], BF16)
    p.xTl = dp("xTl", [78, NPC], BF16)
    p.W = [dp(f"W{i+1}", [LAYERS[i][0], LAYERS[i][1] + 2 * LAYERS[i][2]], BF16)
           for i in range(3)]
    p.br = [dp(f"br{i+1}", [128, LAYERS[i][1]], BF16) for i in range(3)]
    p.wad1 = dp("wad1", [78, LAYERS[0][2]], BF16)
    p.enat = dp("enat", [128, T_tot], I32)
    p.eag = dp("eag", [128, T_tot], I32)
    p.S = dp("S", [T_tot, 128, 128], BF16)
    p.ST = dp("ST", [T_tot, 128, 128], F32)
    p.fc_g1_w = dp("fc_g1_w", [3120, 1024], BF16)
    p.fc_g1_b = dp("fc_g1_b", [GPC, 1024], F32)
    p.fc_g2_w = dp("fc_g2_w", [1024, 128], F32)
    p.fc_g2_b = dp("fc_g2_b", [GPC, 128], F32)
    p.Eh = dp("Eh", [8, 8, 125, 512], BF16)
    p.cw1f = dp("cw1f", [125, 8, 2, 128], BF16)
    p.cb1 = dp("cb1", [32, 1], F32)
    p.cw2T = dp("cw2T", [32, 8, 64], BF16)
    p.cb2 = dp("cb2", [64, 1], F32)
    p.cw3T = dp("cw3T", [64, 8, 96], BF16)
    p.cb3 = dp("cb3", [96, 1], F32)
    p.cw4T = dp("cw4T", [96, 8, 128], BF16)
    p.cb4 = dp("cb4", [128, 1], F32)
    p.w1xt = dp("w1xt", [33, 128, 1024], BF16)
    p.fc1_xt_b = dp("fc1_xt_b", [GPC, 1024], F32)
    p.fc2_xt_w = dp("fc2_xt_w", [1024, 128], F32)
    p.fc2_xt_b = dp("fc2_xt_b", [GPC, 128], F32)
    p.fc1_w = dp("fc1_w", [256, 1024], F32)
    p.fc1_b = dp("fc1_b", [GPC, 1024], F32)
    p.fc2_w = dp("fc2_w", [1024, 256], F32)
    p.fc2_b = dp("fc2_b", [GPC, 256], F32)
    p.out_w = dp("out_w", [256, 1], F32)
    p.out = nc.declare_dram_parameter("out", [GPC, 1], F32, isOutput=True)

    # h1_full natural order (replicated compute, no AG); h2/h3 AG-native order
    p.h_full = [nc.dram_tensor("h1_full", [N_NODES, _aug_cols(0)], BF16)]
    for i in (1, 2):
        p.h_full.append(nc.dram_tensor(f"h{i+1}_full", [N_NODES, _aug_cols(i)],
                                       BF16, addr_space="Shared"))
    p.h_loc = [None,
               nc.dram_tensor("h2_loc", [NPC, _aug_cols(1)], BF16),
               nc.dram_tensor("h3_loc", [NPC, _aug_cols(2)], BF16)]

    def tap(name, shape, dt=F32):
        if name in p.taps:
            t = nc.declare_dram_parameter("tap_" + name, list(shape), dt,
                                          isOutput=True)
            p.tap_tensors[name] = t
            return t
        return None
    p.tap = tap

    with tile.TileContext(nc) as tc:
        p.tc = tc
        _cp_cm = tc.tile_pool(name="const", bufs=1)
        const_pool = _cp_cm.__enter__()
        p.ident = const_pool.tile([128, 128], BF16)
        make_identity(nc, p.ident[:])
        p.head_pool = const_pool

        p.cnn = _cnn_make(p)
        _gat(p)
        _fusion(p)
        _cp_cm.__exit__(None, None, None)

    _split_sync_waits(nc)
    return nc, p


# ---------------- layer-1 replicated matmul ----------------
def _l1_mm(p):
    """Every core computes h1 for ALL nodes -> h1_full (natural order)."""
    nc, tc = p.nc, p.tc
    fi, fo, H = LAYERS[0]
    cols = _aug_cols(0)
    chunks = MM_CHUNKS[0]
    offs = [int(v) for v in np.cumsum([0] + chunks)]
    h_writes = []
    with (
        tc.tile_pool(name="l1w", bufs=1) as wpool,
        tc.tile_pool(name="l1x", bufs=1) as xpool,
        tc.tile_pool(name="l1m", bufs=3) as mpool,
        tc.tile_pool(name="l1p", bufs=2, space="PSUM") as pspool,
    ):
        W_sb = wpool.tile([fi, fo + 2 * H], BF16, tag="W", name="W")
        nc.sync.dma_start(out=W_sb[:], in_=p.W[0][:])
        xf = xpool.tile([fi, N_NODES], BF16, tag="xf", name="xf")
        nc.sync.dma_start(out=xf[:], in_=p.xT[:])
        # local a_d via xTl @ wad1
        xl = xpool.tile([fi, NPC], BF16, tag="xl", name="xl")
        nc.sync.dma_start(out=xl[:], in_=p.xTl[:])
        wad = xpool.tile([fi, H], BF16, tag="wad", name="wad")
        nc.sync.dma_start(out=wad[:], in_=p.wad1[:])
        p.a_d1 = []
        with tc.tile_pool(name="l1adp", bufs=2, space="PSUM") as adps:
            for blk in range(BPC):
                ps = adps.tile([128, H], F32, tag="ad", name="ad")
                nc.tensor.matmul(ps[:], xl[:, blk * 128:(blk + 1) * 128], wad[:],
                                 start=True, stop=True)
                t = p.head_pool.tile([128, H], F32, tag=f"a_d1_{blk}",
                                     name=f"a_d1_{blk}")
                nc.vector.tensor_copy(out=t[:], in_=ps[:])
                p.a_d1.append(t)
        for m in range(NBLK):
            stage = mpool.tile([128, cols], BF16, tag="stage", name="stage",
                               bufs=4)
            for n in range(len(chunks)):
                lo, hi = offs[n], offs[n + 1]
                ps = pspool.tile([128, chunks[n]], F32, tag=f"mp{n}", name=f"mp{n}")
                nc.tensor.matmul(ps[:], xf[:, m * 128:(m + 1) * 128],
                                 W_sb[:, lo:hi], start=True, stop=True)
                if hi <= fo:
                    nc.scalar.copy(out=stage[:, lo:hi], in_=ps[:])
                else:
                    if lo < fo:
                        nc.vector.tensor_copy(out=stage[:, lo:fo],
                                              in_=ps[:, :fo - lo])
                    a_sf = mpool.tile([128, H], F32, tag="a_sf", name="a_sf")
                    nc.vector.tensor_copy(out=a_sf[:], in_=ps[:, fo - lo:fo - lo + H])
                    nc.vector.tensor_copy(
                        out=stage[:, fo:fo + 2 * H], in_=a_sf[:].bitcast(BF16))
            if m < 4:
                oc = fo + 2 * H
                nc.vector.memset(stage[:, oc:oc + 1], 1.0)
                nc.vector.memset(stage[:, oc + 1:cols], 0.0)
            w = nc.sync.dma_start(out=p.h_full[0][m * 128:(m + 1) * 128, :],
                                  in_=stage[:])
            h_writes.append(w)
    # fence: all h1_full writes done
    fence_sb = p.head_pool.tile([1, 2], BF16, tag="fence1", name="fence1")
    fence = nc.sync.dma_start(out=fence_sb[:], in_=p.h_full[0][0:1, 0:2])
    for w in h_writes:
        add_dep_helper(fence.ins, w.ins, reason="h1 fence")
    return fence


# ---------------- message pass for one dst block ----------------
def _mp_block(p, li, blk, pools, a_d_tile, fence, xT_out):
    """Message pass layer li for dst block blk. xT_out: either a list of
    full-width tiles (write cols blk*128..) or None -> allocate per-block
    [128,128] tiles in the mp pool and return them."""
    nc, tc = p.nc, p.tc
    fi, fo, H = LAYERS[li]
    FH = fo // H
    cols = _aug_cols(li)
    gsc = H + fo                     # gs cols: [ex H | h fo]
    chunks = MP_CHUNKS[li]
    offs = [int(v) for v in np.cumsum([0] + chunks)]
    t_off = np.cumsum([0] + p.T_blocks)
    Tb = p.T_blocks[blk]
    t0 = int(t_off[blk])
    n_kT = cdiv(fo, 128)
    mp, mpS = pools["mp"], pools["mpS"]
    eidx = p.eidx_ag if li == 2 else p.eidx_nat

    S_blk = mpS.tile([128, Tb * 128], BF16, tag="Sblk", name="Sblk", bufs=2)
    nc.sync.dma_start(
        out=S_blk[:].rearrange("p (t c) -> p t c", c=128),
        in_=p.S[t0:t0 + Tb].rearrange("t p c -> p t c"))
    ST_blk = mpS.tile([128, Tb * 128], F32, tag="STblk", name="STblk", bufs=2)
    nc.sync.dma_start(
        out=ST_blk[:].rearrange("p (t c) -> p t c", c=128),
        in_=p.ST[t0:t0 + Tb].rearrange("t p c -> p t c"))

    per_tile = True
    rows_bf = mp.tile([128, fo], BF16, tag="rows_bf", name="rows_bf", bufs=2)
    rows_f = mp.tile([128, fo], F32, tag="rows_f", name="rows_f") \
        if li == 0 else rows_bf
    rec = mp.tile([128, H], F32, tag="rec", name="rec", bufs=2)
    if per_tile:
        # per-tile pipeline (H == 1): gather -> score -> scale -> matmuls
        with (
            tc.tile_pool(name=f"ap{li}_{blk}", bufs=1, space="PSUM") as pa,
            tc.tile_pool(name=f"op{li}_{blk}", bufs=1, space="PSUM") as pp,
        ):
            n_ch = len(chunks)
            opsum = [pp.tile([128, chunks[n]], F32, tag=f"op{n}", name=f"op{n}")
                     for n in range(n_ch)]
            for i in range(Tb):
                t = t0 + i
                g_t = mp.tile([128, cols], BF16, tag="g_t", name="g_t", bufs=6)
                gi = nc.gpsimd.indirect_dma_start(
                    out=g_t[:], out_offset=None, in_=p.h_full[li][:],
                    in_offset=bass.IndirectOffsetOnAxis(ap=eidx[:, t:t + 1],
                                                        axis=0))
                add_dep_helper(gi.ins, fence.ins, reason="gather waits h_full")
                adg = pa.tile([128, H], F32, tag="adg", name="adg")
                nc.tensor.matmul(adg[:], ST_blk[:, i * 128:(i + 1) * 128],
                                 a_d_tile[:], start=True, stop=True)
                sc = mp.tile([128, H], F32, tag="sc_t", name="sc_t", bufs=6)
                nc.vector.tensor_tensor(out=sc[:],
                                        in0=g_t[:, fo:fo + 2 * H].bitcast(F32),
                                        in1=adg[:], op=OP.add)
                ex = mp.tile([128, H], F32, tag="ex_t", name="ex_t", bufs=6)
                nc.scalar.activation(ex[:], sc[:], ACT.Prelu, alpha=NEG_SLOPE)
                nc.scalar.activation(ex[:], ex[:], ACT.Exp)
                gs_t = mp.tile([128, gsc], BF16, tag="gs_t", name="gs_t", bufs=4)
                nc.vector.tensor_copy(out=gs_t[:, 0:H], in_=ex[:])
                for h in range(H):
                    eng = nc.vector if h % 2 == 0 else nc.scalar
                    if h % 2 == 0:
                        nc.vector.tensor_scalar(
                            out=gs_t[:, H + h * FH:H + (h + 1) * FH],
                            in0=g_t[:, h * FH:(h + 1) * FH],
                            scalar1=ex[:, h:h + 1], scalar2=None, op0=OP.mult)
                    else:
                        nc.scalar.mul(gs_t[:, H + h * FH:H + (h + 1) * FH],
                                      g_t[:, h * FH:(h + 1) * FH],
                                      ex[:, h:h + 1])
                for n in range(n_ch):
                    nc.tensor.matmul(
                        opsum[n][:], S_blk[:, i * 128:(i + 1) * 128],
                        gs_t[:, offs[n]:offs[n + 1]],
                        start=(i == 0), stop=(i == Tb - 1))
            nc.vector.tensor_scalar(out=rec[:], in0=opsum[0][:, 0:H],
                                    scalar1=1e-16, scalar2=None, op0=OP.add)
            nc.vector.reciprocal(rec[:], rec[:])
            for n in range(n_ch):
                lo, hi = max(offs[n], H), offs[n + 1]
                a, b = lo - H, hi - H
                h0, h1 = a // FH, cdiv(b, FH)
                for h in range(h0, h1):
                    s_lo, s_hi = max(a, h * FH), min(b, (h + 1) * FH)
                    nc.vector.scalar_tensor_tensor(
                        out=rows_f[:, s_lo:s_hi],
                        in0=opsum[n][:, s_lo + H - offs[n]:s_hi + H - offs[n]],
                        scalar=rec[:, h:h + 1], in1=p.br_sb[li][:, s_lo:s_hi],
                        op0=OP.mult, op1=OP.add)
    # activation (batched, row-major)
    if li == 0:
        t1 = mp.tile([128, fo], F32, tag="elu1", name="elu1")
        nc.vector.tensor_scalar(out=t1[:], in0=rows_f[:], scalar1=0.0,
                                scalar2=None, op0=OP.min)
        nc.scalar.activation(t1[:], t1[:], ACT.Exp)
        nc.scalar.activation(rows_f[:], rows_f[:], ACT.Relu)
        nc.vector.scalar_tensor_tensor(out=rows_bf[:], in0=t1[:], scalar=-1.0,
                                       in1=rows_f[:], op0=OP.add, op1=OP.add)
    else:
        nc.scalar.activation(rows_bf[:], rows_bf[:], ACT.Relu)
    # transpose; either into full-width xT_out columns or per-block tiles
    if xT_out is None:
        xtb = [mp.tile([min(128, fo - j * 128), 128], BF16, tag=f"xtb{j}",
                       name=f"xtb{j}", bufs=2) for j in range(n_kT)]
        dst = lambda j: xtb[j][:]
    else:
        dst = lambda j: xT_out[j][:, blk * 128:(blk + 1) * 128]
    with tc.tile_pool(name=f"tp{li}_{blk}", bufs=2, space="PSUM") as ptp:
        for j in range(n_kT):
            kp = min(128, fo - j * 128)
            tp = ptp.tile([kp, 128], BF16, tag="tp", name="tp")
            nc.tensor.transpose(tp[:], rows_bf[:, j * 128:j * 128 + kp],
                                p.ident[:])
            if j % 2 == 0:
                nc.scalar.copy(out=dst(j), in_=tp[:])
            else:
                nc.vector.tensor_copy(out=dst(j), in_=tp[:])
    return None if xT_out is not None else xtb


# ---------------- next-layer matmul for one dst block ----------------
def _mm_block(p, li, blk, W_sb, xtb, pools, a_d_list):
    """x@W for layer li (1 or 2), rows of dst block blk (lhsT tiles xtb);
    writes h_loc rows. Returns the stage-write instr for the AG."""
    nc, tc = p.nc, p.tc
    fi, fo, H = LAYERS[li]
    cols = _aug_cols(li)
    n_k = cdiv(fi, 128)
    chunks = MM_CHUNKS[li]
    offs = [int(v) for v in np.cumsum([0] + chunks)]
    mp = pools["mm"]
    stage = mp.tile([128, cols], BF16, tag=f"stg{li}", name=f"stg{li}", bufs=2)
    with tc.tile_pool(name=f"mmp{li}_{blk}", bufs=4, space="PSUM") as pspool:
        for n in range(len(chunks)):
            lo, hi = offs[n], offs[n + 1]
            ps = pspool.tile([128, chunks[n]], F32, tag="mmps", name="mmps")
            for k in range(n_k):
                nc.tensor.matmul(
                    ps[:], xtb[k], W_sb[k][:, lo:hi],
                    start=(k == 0), stop=(k == n_k - 1))
            if hi <= fo:
                nc.scalar.copy(out=stage[:, lo:hi], in_=ps[:])
            else:
                if lo < fo:
                    nc.scalar.copy(out=stage[:, lo:fo], in_=ps[:, :fo - lo])
                a_sf = mp.tile([128, H], F32, tag="a_sf2", name="a_sf2")
                nc.vector.tensor_copy(out=a_sf[:], in_=ps[:, fo - lo:fo - lo + H])
                a_d = p.head_pool.tile([128, H], F32, tag=f"a_d{li}_{blk}",
                                       name=f"a_d{li}_{blk}")
                nc.vector.tensor_copy(out=a_d[:],
                                      in_=ps[:, fo - lo + H:fo - lo + 2 * H])
                a_d_list.append(a_d)
                nc.vector.tensor_copy(out=stage[:, fo:fo + 2 * H],
                                      in_=a_sf[:].bitcast(BF16))
    oc = fo + 2 * H
    nc.vector.memset(stage[:, oc:oc + 1], 1.0)
    nc.vector.memset(stage[:, oc + 1:cols], 0.0)
    w = nc.sync.dma_start(out=p.h_loc[li][blk * 128:(blk + 1) * 128, :],
                          in_=stage[:])
    return w


def _fire_ag(p, li, half, ws):
    """AllGather of one half of h_loc -> h_full rows (half-AG order)."""
    nc = p.nc
    cc = nc.gpsimd.collective_compute(
        "AllGather", OP.bypass, replica_groups=[list(range(NCORES))],
        ins=[p.h_loc[li][half * 640:(half + 1) * 640, :]],
        outs=[p.h_full[li][half * 5120:(half + 1) * 5120, :]])
    for w in ws:
        add_dep_helper(cc.ins, w.ins, reason="AG waits h_loc writes")
    return cc


# ---------------- GAT orchestration ----------------
def _gat(p):
    nc, tc = p.nc, p.tc
    T_tot = sum(p.T_blocks)

    mpc_cm = tc.tile_pool(name="mpc", bufs=1)
    mpc = mpc_cm.__enter__()
    p.eidx_nat = mpc.tile([128, T_tot], I32, tag="enat", name="enat")
    nc.sync.dma_start(out=p.eidx_nat[:], in_=p.enat[:])
    p.eidx_ag = mpc.tile([128, T_tot], I32, tag="eag", name="eag")
    nc.sync.dma_start(out=p.eidx_ag[:], in_=p.eag[:])
    p.br_sb = []
    for li in range(3):
        t = mpc.tile([128, LAYERS[li][1]], BF16, tag=f"br{li}", name=f"br{li}")
        nc.sync.dma_start(out=t[:], in_=p.br[li][:])
        p.br_sb.append(t)

    fence1 = _l1_mm(p)
    p.cnn["stage1_w"]()     # CNN weight loads early

    # ---- phase 1: MP L1 (all blocks) -> dense mm L2 with mid-pass AG ----
    a_d2 = []
    ccs2 = []
    xT2_cm = tc.tile_pool(name="xT2", bufs=1)
    xT2p = xT2_cm.__enter__()
    fo2 = LAYERS[0][1]
    n_kT2 = cdiv(fo2, 128)
    xT2 = [xT2p.tile([min(128, fo2 - j * 128), NPC], BF16, tag=f"xT2_{j}",
                     name=f"xT2_{j}") for j in range(n_kT2)]
    with (
        tc.tile_pool(name="w2", bufs=1) as w2pool,
        tc.tile_pool(name="mp1", bufs=1) as mp1,
        tc.tile_pool(name="mpS1", bufs=1) as mpS1,
        tc.tile_pool(name="mm1", bufs=1) as mm1,
    ):
        fi2, fo2b, H2 = LAYERS[1]
        W2_sb = []
        for k in range(cdiv(fi2, 128)):
            kp = min(128, fi2 - k * 128)
            t = w2pool.tile([kp, fo2b + 2 * H2], BF16, tag=f"W2_{k}",
                            name=f"W2_{k}")
            nc.sync.dma_start(out=t[:], in_=p.W[1][k * 128:k * 128 + kp, :])
            W2_sb.append(t)
        pools = {"mp": mp1, "mpS": mpS1, "mm": mm1}
        for blk in range(BPC):
            _mp_block(p, 0, blk, pools, p.a_d1[blk], fence1, xT2)
        ws = []
        for blk in range(BPC):
            xts = [xT2[k][:min(128, fo2 - k * 128), blk * 128:(blk + 1) * 128]
                   for k in range(n_kT2)]
            ws.append(_mm_block(p, 1, blk, W2_sb, xts, pools, a_d2))
        cc = nc.gpsimd.collective_compute(
            "AllGather", OP.bypass, replica_groups=[list(range(NCORES))],
            ins=[p.h_loc[1][:]], outs=[p.h_full[1][:]])
        for w in ws:
            add_dep_helper(cc.ins, w.ins, reason="AG2 waits h_loc writes")
        ccs2.append(cc)
        for g in range(8):
            p.cnn["stage1_grp"](g)
        p.cnn["stage2"]()
        p.cnn["stage3"]()
    xT2_cm.__exit__(None, None, None)
    p.cnn["cleanup"]()
    fence2_sb = p.head_pool.tile([1, 2], BF16, tag="fence2", name="fence2")
    fence2 = nc.sync.dma_start(out=fence2_sb[:], in_=p.h_full[1][0:1, 0:2])
    for cc in ccs2:
        add_dep_helper(fence2.ins, cc.ins, reason="h2 fence")

    # ---- phase 2: MP L2 (all blocks) -> dense mm L3 with mid-pass AG ----
    a_d3 = []
    ccs3 = []
    xT3_cm = tc.tile_pool(name="xT3", bufs=1)
    xT3p = xT3_cm.__enter__()
    fo3 = LAYERS[1][1]
    n_kT3 = cdiv(fo3, 128)
    xT3 = [xT3p.tile([min(128, fo3 - j * 128), NPC], BF16, tag=f"xT3_{j}",
                     name=f"xT3_{j}") for j in range(n_kT3)]
    with (
        tc.tile_pool(name="w3", bufs=1) as w3pool,
        tc.tile_pool(name="mp2", bufs=1) as mp2,
        tc.tile_pool(name="mpS2", bufs=1) as mpS2,
        tc.tile_pool(name="mm2", bufs=1) as mm2,
    ):
        fi3, fo3b, H3 = LAYERS[2]
        W3_sb = []
        for k in range(cdiv(fi3, 128)):
            kp = min(128, fi3 - k * 128)
            t = w3pool.tile([kp, fo3b + 2 * H3], BF16, tag=f"W3_{k}",
                            name=f"W3_{k}")
            nc.sync.dma_start(out=t[:], in_=p.W[2][k * 128:k * 128 + kp, :])
            W3_sb.append(t)
        pools = {"mp": mp2, "mpS": mpS2, "mm": mm2}
        for blk in range(BPC):
            _mp_block(p, 1, blk, pools, a_d2[blk], fence2, xT3)
        ws = []
        for blk in range(BPC):
            xts = [xT3[k][:min(128, fo3 - k * 128), blk * 128:(blk + 1) * 128]
                   for k in range(n_kT3)]
            ws.append(_mm_block(p, 2, blk, W3_sb, xts, pools, a_d3))
            if blk == 4:
                ccs3.append(_fire_ag(p, 2, 0, ws))
        ccs3.append(_fire_ag(p, 2, 1, ws[5:]))
    xT3_cm.__exit__(None, None, None)
    fence3_sb = p.head_pool.tile([1, 2], BF16, tag="fence3", name="fence3")
    fence3 = nc.sync.dma_start(out=fence3_sb[:], in_=p.h_full[2][0:1, 0:2])
    for cc in ccs3:
        add_dep_helper(fence3.ins, cc.ins, reason="h3 fence")

    # ---- phase 3: MP L3 (+ CNN stages 2/3) ----
    xT4_cm = tc.tile_pool(name="xT4", bufs=1)
    xT4p = xT4_cm.__enter__()
    fo4 = LAYERS[2][1]
    p.out3T = [xT4p.tile([min(128, fo4 - j * 128), NPC], BF16, tag=f"xT4_{j}",
                         name=f"xT4_{j}") for j in range(cdiv(fo4, 128))]
    with (
        tc.tile_pool(name="mp3", bufs=1) as mp3,
        tc.tile_pool(name="mpS3", bufs=1) as mpS3,
    ):
        pools = {"mp": mp3, "mpS": mpS3, "mm": None}
        for blk in range(BPC):
            _mp_block(p, 2, blk, pools, a_d3[blk], fence3, p.out3T)
    p.gat_cleanup = [xT4_cm, mpc_cm]

    t = p.tap("o3T", [fo4, NPC], BF16)
    if t is not None:
        for j in range(cdiv(fo4, 128)):
            kp = min(128, fo4 - j * 128)
            nc.sync.dma_start(out=t[j * 128:j * 128 + kp, :], in_=p.out3T[j][:])


def _dve_T(nc, dst, src, n):
    """dst[n, 32] = src[32, n].T via DVE 32x32 block transposes (f32)."""
    for i in range(n // 32):
        nc.vector.transpose(out=dst[32 * i:32 * (i + 1), :],
                            in_=src[:, 32 * i:32 * (i + 1)])


# ---------------- graph head ----------------
def _graph_head(p):
    nc, tc = p.nc, p.tc
    n_kT = len(p.out3T)
    with (
        tc.tile_pool(name="gh", bufs=2) as gh,
        tc.tile_pool(name="ghG", bufs=1) as ghG,
        tc.tile_pool(name="ghp", bufs=2, space="PSUM") as ghp,
    ):
        gT = [ghG.tile([min(128, 3120 - j * 128), GPC], BF16, tag=f"gT{j}",
                       name=f"gT{j}") for j in range(n_kT)]
        for j in range(n_kT):
            nc.vector.reduce_max(
                gT[j][:],
                p.out3T[j][:].rearrange("p (g n) -> p g n", n=NPG),
                axis=AX.X)
        g1 = ghG.tile([GPC, 1024], F32, tag="g1", name="g1")
        for n in range(2):
            ps = ghp.tile([GPC, 512], F32, tag="mm", name="mm")
            for j in range(n_kT):
                kp = min(128, 3120 - j * 128)
                w = gh.tile([kp, 512], BF16, tag="fg1w", name="fg1w")
                nc.sync.dma_start(out=w[:], in_=p.fc_g1_w[j * 128:j * 128 + kp,
                                                          n * 512:(n + 1) * 512])
                nc.tensor.matmul(ps[:], gT[j][:], w[:], start=(j == 0),
                                 stop=(j == n_kT - 1))
            nc.vector.tensor_copy(out=g1[:, n * 512:(n + 1) * 512], in_=ps[:])
        bb1 = gh.tile([GPC, 1024], F32, tag="ghbb", name="ghbb")
        nc.sync.dma_start(out=bb1[:], in_=p.fc_g1_b[:])
        nc.vector.tensor_tensor(out=g1[:], in0=g1[:], in1=bb1[:], op=OP.add)
        g1b = ghG.tile([GPC, 1024], F32, tag="g1b", name="g1b")
        nc.scalar.activation(g1b[:], g1[:], ACT.Relu)
        g1T = [ghG.tile([128, GPC], F32, tag=f"g1T{j}", name=f"g1T{j}")
               for j in range(8)]
        for j in range(8):
            _dve_T(nc, g1T[j], g1b[:, j * 128:(j + 1) * 128], 128)
        ps = ghp.tile([GPC, 128], F32, tag="mm", name="mm")
        for j in range(8):
            w = gh.tile([128, 128], F32, tag="fg2w", name="fg2w")
            nc.sync.dma_start(out=w[:], in_=p.fc_g2_w[j * 128:(j + 1) * 128, :])
            nc.tensor.matmul(ps[:], g1T[j][:], w[:], start=(j == 0), stop=(j == 7))
        p.g2 = p.head_pool.tile([GPC, 128], F32, tag="g2", name="g2")
        bb2 = gh.tile([GPC, 128], F32, tag="ghbb2", name="ghbb2")
        nc.sync.dma_start(out=bb2[:], in_=p.fc_g2_b[:])
        nc.vector.tensor_tensor(out=p.g2[:], in0=ps[:], in1=bb2[:], op=OP.add)
        t = p.tap("g2", [GPC, 128])
        if t is not None:
            nc.sync.dma_start(out=t[:], in_=p.g2[:])


# ---------------- CNN branch ----------------
def _cnn_make(p):
    """CNN branch split into stages interleaved into the GAT phases."""
    nc, tc = p.nc, p.tc
    st = {}

    def stage1_w():
        cn_cm = tc.tile_pool(name="cn", bufs=2)
        cnw_cm = tc.tile_pool(name="cnw", bufs=1)
        cny_cm = tc.tile_pool(name="cny", bufs=1)
        cn = cn_cm.__enter__()
        cnw = cnw_cm.__enter__()
        cny = cny_cm.__enter__()
        st["cms"] = [cny_cm, cnw_cm, cn_cm]
        st["cn"], st["cnw"], st["cny"] = cn, cnw, cny
        cw1f_sb = cnw.tile([125, 8, 2, 128], BF16, tag="cw1f", name="cw1f")
        nc.sync.dma_start(out=cw1f_sb[:], in_=p.cw1f[:])
        cw2_sb = cnw.tile([32, 8, 64], BF16, tag="cw2", name="cw2")
        nc.sync.dma_start(out=cw2_sb[:], in_=p.cw2T[:])
        cw3_sb = cnw.tile([64, 8, 96], BF16, tag="cw3", name="cw3")
        nc.sync.dma_start(out=cw3_sb[:], in_=p.cw3T[:])
        cw4_sb = cnw.tile([96, 8, 128], BF16, tag="cw4", name="cw4")
        nc.sync.dma_start(out=cw4_sb[:], in_=p.cw4T[:])
        cb = {}
        for nm, sh in [("cb1", 32), ("cb2", 64), ("cb3", 96), ("cb4", 128)]:
            cb[nm] = cnw.tile([sh, 1], F32, tag=nm, name=nm)
            nc.sync.dma_start(out=cb[nm][:], in_=getattr(p, nm)[:])
        st.update(cw1f=cw1f_sb, cw2=cw2_sb, cw3=cw3_sb, cw4=cw4_sb, cb=cb)
        y1 = cny.tile([32, GPC * 121], BF16, tag="y1", name="y1")
        st["y1"] = y1
        E_all = cnw.tile([125, 64 * 512], BF16, tag="E_all", name="E_all")
        st["E_all"] = E_all
        for grp in range(8):
            for sc in range(8):
                nc.sync.dma_start(
                    out=E_all[:, (grp * 8 + sc) * 512:(grp * 8 + sc + 1) * 512],
                    in_=p.Eh[grp, sc])

    def stage1_grp(grp):
        cn, cb = st["cn"], st["cb"]
        cw1f_sb = st["cw1f"]
        y1 = st["y1"]
        with tc.tile_pool(name=f"cnp1_{grp}", bufs=1, space="PSUM") as cnp:
            pc = [cnp.tile([128, 512], F32, tag=f"pc{k}", name=f"pc{k}")
                  for k in range(2)]
            E_all = st["E_all"]
            for sc in range(8):
                j = (grp * 8 + sc) * 512
                for ks in range(2):
                    nc.tensor.matmul(pc[ks][:], cw1f_sb[:, sc, ks, :],
                                     E_all[:, j:j + 512],
                                     start=(sc == 0), stop=(sc == 7))
            acc = cn.tile([32, 4 * 121], F32, tag="c1acc", name="c1acc", bufs=1)
            accr = acc[:].rearrange("p (b t) -> p b t", b=4)
            firstop = True
            for ks in range(2):
                for kl in range(4):
                    k = ks * 4 + kl
                    src = pc[ks][:].rearrange("p (b j) -> p b j", b=4)[
                        kl * 32:(kl + 1) * 32, :, k:k + 121]
                    if firstop:
                        nc.vector.tensor_copy(out=accr, in_=src)
                        firstop = False
                    else:
                        nc.vector.tensor_tensor(out=accr, in0=accr, in1=src,
                                                op=OP.add)
            nc.scalar.activation(y1[:, grp * 4 * 121:(grp + 1) * 4 * 121],
                                 acc[:], ACT.Relu, bias=cb["cb1"][:32, :1])

    def stage2():
        cn, cny, cb = st["cn"], st["cny"], st["cb"]
        cw2_sb, cw3_sb, cw4_sb = st["cw2"], st["cw3"], st["cw4"]
        y1 = st["y1"]
        with tc.tile_pool(name="cnp2", bufs=2, space="PSUM") as cnp:
            y2 = cny.tile([64, GPC * 114], BF16, tag="y2", name="y2")
            for grp in range(8):
                ps = cnp.tile([64, 4 * 114], F32, tag="pc0", name="pc0")
                for k in range(8):
                    rhs = y1[:].rearrange("p (b t) -> p b t", t=121)[
                        :, grp * 4:(grp + 1) * 4, k:k + 114]
                    nc.tensor.matmul(ps[:], cw2_sb[:, k, :], rhs, start=(k == 0),
                                     stop=(k == 7))
                nc.scalar.activation(y2[:, grp * 4 * 114:(grp + 1) * 4 * 114],
                                     ps[:], ACT.Relu, bias=cb["cb2"][:, :1])
            y3 = cny.tile([96, GPC * 107], BF16, tag="y3", name="y3")
            for grp in range(8):
                ps = cnp.tile([96, 4 * 107], F32, tag="pc0", name="pc0")
                for k in range(8):
                    rhs = y2[:].rearrange("p (b t) -> p b t", t=114)[
                        :, grp * 4:(grp + 1) * 4, k:k + 107]
                    nc.tensor.matmul(ps[:], cw3_sb[:, k, :], rhs, start=(k == 0),
                                     stop=(k == 7))
                nc.scalar.activation(y3[:, grp * 4 * 107:(grp + 1) * 4 * 107],
                                     ps[:], ACT.Relu, bias=cb["cb3"][:, :1])
            yp = cny.tile([128, GPC * 33], BF16, tag="yp", name="yp")
            st["yp"] = yp
            for grp in range(8):
                ps = cnp.tile([128, 4 * 100], F32, tag="pc0", name="pc0")
                for k in range(8):
                    rhs = y3[:].rearrange("p (b t) -> p b t", t=107)[
                        :, grp * 4:(grp + 1) * 4, k:k + 100]
                    nc.tensor.matmul(ps[:], cw4_sb[:, k, :], rhs, start=(k == 0),
                                     stop=(k == 7))
                psr = ps[:].rearrange("p (b t) -> p b t", b=4)
                mx = cn.tile([128, 4 * 33], F32, tag="mx", name="mx")
                mxr = mx[:].rearrange("p (b t) -> p b t", b=4)
                nc.vector.tensor_copy(out=mxr, in_=psr[:, :, 0:99:3])
                nc.vector.tensor_tensor(out=mxr, in0=mxr, in1=psr[:, :, 1:100:3],
                                        op=OP.max)
                nc.vector.tensor_tensor(out=mxr, in0=mxr, in1=psr[:, :, 2:100:3],
                                        op=OP.max)
                nc.scalar.activation(yp[:, grp * 4 * 33:(grp + 1) * 4 * 33],
                                     mx[:], ACT.Relu, bias=cb["cb4"][:, :1])

    def stage3():
        cn, cny = st["cn"], st["cny"]
        yp = st["yp"]
        with tc.tile_pool(name="cnp3", bufs=2, space="PSUM") as cnp:
            xt1 = cny.tile([GPC, 1024], F32, tag="xt1", name="xt1")
            for n in range(2):
                ps = cnp.tile([GPC, 512], F32, tag="pc0", name="pc0")
                for t_ in range(33):
                    w = cn.tile([128, 512], BF16, tag="fx1w", name="fx1w", bufs=2)
                    nc.sync.dma_start(out=w[:],
                                      in_=p.w1xt[t_, :, n * 512:(n + 1) * 512])
                    lhs = yp[:].rearrange("p (b t) -> p t b", t=33)[:, t_, :]
                    nc.tensor.matmul(ps[:], lhs, w[:], start=(t_ == 0),
                                     stop=(t_ == 32))
                nc.vector.tensor_copy(out=xt1[:, n * 512:(n + 1) * 512], in_=ps[:])
            bb = cn.tile([GPC, 1024], F32, tag="fxbb", name="fxbb", bufs=1)
            nc.sync.dma_start(out=bb[:], in_=p.fc1_xt_b[:])
            nc.vector.tensor_tensor(out=xt1[:], in0=xt1[:], in1=bb[:], op=OP.add)
            nc.scalar.activation(xt1[:], xt1[:], ACT.Relu)
            xt1T = [cn.tile([128, GPC], F32, tag=f"xt1T{j}", name=f"xt1T{j}",
                            bufs=1)
                    for j in range(8)]
            for j in range(8):
                _dve_T(nc, xt1T[j], xt1[:, j * 128:(j + 1) * 128], 128)
            ps = cnp.tile([GPC, 128], F32, tag="pc0", name="pc0")
            for j in range(8):
                w = cn.tile([128, 128], F32, tag="fx2w", name="fx2w", bufs=2)
                nc.sync.dma_start(out=w[:], in_=p.fc2_xt_w[j * 128:(j + 1) * 128, :])
                nc.tensor.matmul(ps[:], xt1T[j][:], w[:], start=(j == 0),
                                 stop=(j == 7))
            p.xt2 = p.head_pool.tile([GPC, 128], F32, tag="xt2", name="xt2")
            bb2 = cn.tile([GPC, 128], F32, tag="fxbb2", name="fxbb2", bufs=1)
            nc.sync.dma_start(out=bb2[:], in_=p.fc2_xt_b[:])
            nc.vector.tensor_tensor(out=p.xt2[:], in0=ps[:], in1=bb2[:], op=OP.add)
            t = p.tap("xt2", [GPC, 128])
            if t is not None:
                nc.sync.dma_start(out=t[:], in_=p.xt2[:])

    def cleanup():
        for cm in st["cms"]:
            cm.__exit__(None, None, None)

    return {"stage1_w": stage1_w, "stage1_grp": stage1_grp, "stage2": stage2,
            "stage3": stage3, "cleanup": cleanup}


# ---------------- fusion ----------------
def _fusion(p):
    nc, tc = p.nc, p.tc
    _graph_head(p)
    for cm in p.gat_cleanup:
        cm.__exit__(None, None, None)
    with (
        tc.tile_pool(name="fu", bufs=2) as fu,
        tc.tile_pool(name="fup", bufs=2, space="PSUM") as fup,
    ):
        xcT = []
        for src_ in (p.g2, p.xt2):
            t = fu.tile([128, GPC], F32, tag=f"xcT{len(xcT)}", name=f"xcT{len(xcT)}")
            _dve_T(nc, t, src_[:], 128)
            xcT.append(t)
        c1 = fu.tile([GPC, 1024], F32, tag="c1", name="c1")
        for n in range(2):
            ps = fup.tile([GPC, 512], F32, tag="mm", name="mm")
            for j in range(2):
                w = fu.tile([128, 512], F32, tag="f1w", name="f1w")
                nc.sync.dma_start(out=w[:], in_=p.fc1_w[j * 128:(j + 1) * 128,
                                                        n * 512:(n + 1) * 512])
                nc.tensor.matmul(ps[:], xcT[j][:], w[:], start=(j == 0),
                                 stop=(j == 1))
            nc.vector.tensor_copy(out=c1[:, n * 512:(n + 1) * 512], in_=ps[:])
        bb = fu.tile([GPC, 1024], F32, tag="fbb", name="fbb")
        nc.sync.dma_start(out=bb[:], in_=p.fc1_b[:])
        nc.vector.tensor_tensor(out=c1[:], in0=c1[:], in1=bb[:], op=OP.add)
        c1b = fu.tile([GPC, 1024], F32, tag="c1b", name="c1b")
        nc.scalar.activation(c1b[:], c1[:], ACT.Relu)
        c1T = [fu.tile([128, GPC], F32, tag=f"c1T{j}", name=f"c1T{j}") for j in range(8)]
        for j in range(8):
            _dve_T(nc, c1T[j], c1b[:, j * 128:(j + 1) * 128], 128)
        ps = fup.tile([GPC, 256], F32, tag="mm", name="mm")
        for j in range(8):
            w = fu.tile([128, 256], F32, tag="f2w", name="f2w")
            nc.sync.dma_start(out=w[:], in_=p.fc2_w[j * 128:(j + 1) * 128, :])
            nc.tensor.matmul(ps[:], c1T[j][:], w[:], start=(j == 0), stop=(j == 7))
        c2 = fu.tile([GPC, 256], F32, tag="c2", name="c2")
        bb2 = fu.tile([GPC, 256], F32, tag="fbb2", name="fbb2")
        nc.sync.dma_start(out=bb2[:], in_=p.fc2_b[:])
        nc.vector.tensor_tensor(out=c2[:], in0=ps[:], in1=bb2[:], op=OP.add)
        c2b = fu.tile([GPC, 256], F32, tag="c2b", name="c2b")
        nc.scalar.activation(c2b[:], c2[:], ACT.Relu)
        c2T = []
        for j in range(2):
            t = fu.tile([128, GPC], F32, tag=f"c2T{j}", name=f"c2T{j}")
            _dve_T(nc, t, c2b[:, j * 128:(j + 1) * 128], 128)
            c2T.append(t)
        ow = fu.tile([128, 2], F32, tag="ow", name="ow")
        for j in range(2):
            nc.sync.dma_start(out=ow[:, j:j + 1], in_=p.out_w[j * 128:(j + 1) * 128, :])
        ps = fup.tile([GPC, 1], F32, tag="mm", name="mm")
        for j in range(2):
            nc.tensor.matmul(ps[:], c2T[j][:], ow[:, j:j + 1],
                             start=(j == 0), stop=(j == 1))
        o = fu.tile([GPC, 1], F32, tag="o", name="o")
        nc.vector.tensor_copy(out=o[:], in_=ps[:])
        nc.sync.dma_start(out=p.out[:], in_=o[:])


# ------------------------------------------------------------------ entry
def _build_and_run(inputs, taps=()):
    T_blocks, in_maps, out_b = _host_prep(inputs)
    nc, p = build_program(T_blocks, taps=taps)
    res = run_bass_kernel_spmd(nc, in_maps, list(range(NCORES)))
    return res, out_b, p


def kernel(**inputs) -> np.ndarray:
    res, out_b, _ = _build_and_run(inputs)
    out = np.concatenate([res.results[c]["out"] for c in range(NCORES)], axis=0)
    return (out + out_b).astype(np.float32)


# revision 42
# speedup vs baseline: 1.2304x; 1.0087x over previous
"""GATNet (3x GATConv graph branch + 1D-CNN protein branch + fusion MLP) on 8
Trainium2 NeuronCores via Bass/Tile.

v2 pipeline:
  - L1 x@W replicated on every core (tiny) -> h1_full in natural node order;
    no AllGather for layer 1.
  - Layers 2/3: per-dst-block interleave of [message-pass block m] ->
    [next-layer matmul block m] -> [per-block AllGather of that block's h].
    h2/h3_full use AG-native row order (m*1024 + core*128 + r) with gather
    indices remapped on host, so each 128-row block AllGather lands
    contiguously and communication pipelines behind compute.
  - Message pass per block: per-tile indirect gathers into one SBUF strip,
    block-batched score chain (add / prelu / exp), single broadcast-AP
    multiply for per-head scaling, S^T one-hot matmuls accumulate numerator
    + denominator (ex column first), fused scalar_tensor_tensor epilogue
    (x*rec + bias), batched activation, then transpose tiles for the next
    matmul's lhsT.
  - CNN branch: embedding done on host (E shipped as input); stages
    interleaved into the GAT phases. Large head weights in bf16.

Self-contained: hardcodes all shapes; builds the per-call edge structure into
the traced program, compiles and runs via run_bass_kernel_spmd.
"""
import numpy as np
import ml_dtypes

import concourse.bass as bass
import concourse.mybir as mybir
import concourse.tile as tile
from concourse.bass_utils import run_bass_kernel_spmd
from concourse.masks import make_identity
from concourse.tile import add_dep_helper

NCORES = 8
N_NODES = 10240
N_GRAPHS = 256
NPC = N_NODES // NCORES          # 1280 nodes/core
GPC = N_GRAPHS // NCORES         # 32 graphs/core
NPG = N_NODES // N_GRAPHS        # 40 nodes/graph
BPC = NPC // 128                 # 10 dst blocks/core
NBLK = N_NODES // 128            # 80 global blocks
SEQ = 1000
VOCAB = 26
EMB = 128
NEG_SLOPE = 0.2

F32 = mybir.dt.float32
BF16 = mybir.dt.bfloat16
I32 = mybir.dt.int32
AX = mybir.AxisListType
OP = mybir.AluOpType
ACT = mybir.ActivationFunctionType

# (F_in, F_out, heads)
LAYERS = [(78, 780, 10), (780, 1560, 2), (1560, 3120, 1)]
# x@W psum chunk lists over [fo + 2H] aug cols (h | a_s | a_d)
MM_CHUNKS = [[512, 288], [512, 512, 512, 28], [512] * 6 + [50]]
# message-pass psum chunks over gs cols [ex H | h fo]
MP_CHUNKS = [[512, 278], [512, 512, 512, 26], [512] * 6 + [49]]

bf = lambda a: np.ascontiguousarray(a).astype(ml_dtypes.bfloat16)
f32 = lambda a: np.ascontiguousarray(a, dtype=np.float32)
cdiv = lambda a, b: -(-a // b)


# ------------------------------------------------------------------ walrus patch
def _split_sync_waits(nc, max_keep=1):
    for f in nc.m.functions:
        for bb in f.blocks:
            out, changed = [], False
            for ins in bb.instructions:
                si = ins.sync_info
                waits = list(si.on_wait) if si is not None and si.on_wait else []
                if len(waits) > max_keep:
                    extra, keep = waits[:-max_keep], waits[-max_keep:]
                    for i in range(0, len(extra), max_keep):
                        out.append(mybir.InstNoOp(
                            name=f"WSPLIT-{nc.next_id()}", engine=ins.engine,
                            bass_nofuse=True,
                            sync_info=mybir.SyncInfo(on_wait=extra[i:i + max_keep],
                                                     on_update=[])))
                    si.on_wait = keep
                    changed = True
                out.append(ins)
            if changed:
                bb.instructions[:] = out


# ------------------------------------------------------------------ host prep
def _edge_structure(edge_index):
    src, dst = edge_index[0].astype(np.int64), edge_index[1].astype(np.int64)
    loop = np.arange(N_NODES, dtype=np.int64)
    s_all = np.concatenate([src, loop])
    d_all = np.concatenate([dst, loop])
    order = np.argsort(d_all, kind="stable")
    s_s, d_s = s_all[order], d_all[order]

    bounds = np.searchsorted(d_s, np.arange(0, N_NODES + 1, 128))
    cnt = bounds[1:] - bounds[:-1]
    tiles_needed = -(-cnt // 128)
    T_blocks = [int(tiles_needed.reshape(NCORES, BPC)[:, p].max()) for p in range(BPC)]
    t_off = np.cumsum([0] + T_blocks)
    T_tot = int(t_off[-1])

    src_nat = np.zeros((NCORES, T_tot, 128), np.int32)
    S = np.zeros((NCORES, T_tot, 128, 128), np.float32)
    for c in range(NCORES):
        for p_ in range(BPC):
            blk = c * BPC + p_
            e0, e1 = int(bounds[blk]), int(bounds[blk + 1])
            m = e1 - e0
            ti = np.arange(m) // 128 + t_off[p_]
            ei = np.arange(m) % 128
            src_nat[c, ti, ei] = s_s[e0:e1]
            S[c, ti, ei, d_s[e0:e1] - 128 * blk] = 1.0
    # half-AG row order: half h = loc//640; row = h*5120 + core*640 + loc%640
    sv = src_nat.astype(np.int64)
    loc = sv % NPC
    src_ag = ((loc // 640) * 5120 + (sv // NPC) * 640 + loc % 640).astype(np.int32)
    ST = np.ascontiguousarray(np.swapaxes(S, 2, 3))
    natT = np.ascontiguousarray(np.swapaxes(src_nat, 1, 2))  # [8,128,T_tot]
    agT = np.ascontiguousarray(np.swapaxes(src_ag, 1, 2))
    return T_blocks, natT, agT, bf(S), f32(ST)


def _aug_w(W, a_s, a_d, H):
    """[W | W@as_blk | W@ad_blk] with as_blk[f,h] = a_s[h, f - h*FH]."""
    fi, fo = W.shape
    FH = fo // H
    was = np.zeros((fi, H), np.float32)
    wad = np.zeros((fi, H), np.float32)
    for h in range(H):
        was[:, h] = W[:, h * FH:(h + 1) * FH] @ a_s[h]
        wad[:, h] = W[:, h * FH:(h + 1) * FH] @ a_d[h]
    return np.concatenate([W, was, wad], axis=1), wad


def _host_prep(inputs):
    ii = {k: np.asarray(v) for k, v in inputs.items()}
    T_blocks, natT, agT, S, ST = _edge_structure(ii["edge_index"])

    xT = np.ascontiguousarray(np.swapaxes(f32(ii["x"]), 0, 1))   # [78, 10240]

    W_aug, wads, bias_rep = [], [], []
    for i, (fi, fo, H) in enumerate(LAYERS):
        Wa, wad = _aug_w(f32(ii[f"W{i+1}"]), f32(ii[f"as{i+1}"]),
                         f32(ii[f"ad{i+1}"]), H)
        W_aug.append(bf(Wa))
        wads.append(bf(wad))
        b = f32(ii[f"b{i+1}"]).reshape(1, -1)
        bias_rep.append(bf(np.broadcast_to(b, (128, fo))))

    cw1 = f32(ii["cw1"])
    cw1f = np.zeros((125, 8, 2, 128), np.float32)
    for sc in range(8):
        for ks in range(2):
            blk = cw1[:, sc * 125:(sc + 1) * 125, ks * 4:(ks + 1) * 4]
            cw1f[:, sc, ks, :] = blk.transpose(1, 2, 0).reshape(125, 128)
    cwT = lambda w: np.ascontiguousarray(np.transpose(f32(ii[w]), (1, 2, 0)))

    w1xt = np.ascontiguousarray(
        f32(ii["fc1_xt_w"]).reshape(128, 33, 1024).transpose(1, 0, 2))

    emb = f32(ii["emb_xt"])
    rep = lambda a, n: np.ascontiguousarray(
        np.broadcast_to(f32(a).reshape(1, -1), (n, f32(a).size)))

    shared = {
        "W1": W_aug[0], "W2": W_aug[1], "W3": W_aug[2],
        "br1": bias_rep[0], "br2": bias_rep[1], "br3": bias_rep[2],
        "wad1": wads[0],
        "fc_g1_w": bf(ii["fc_g1_w"]), "fc_g1_b": rep(ii["fc_g1_b"], GPC),
        "fc_g2_w": f32(ii["fc_g2_w"]), "fc_g2_b": rep(ii["fc_g2_b"], GPC),
        "cw1f": bf(cw1f), "cb1": f32(ii["cb1"]).reshape(-1, 1),
        "cw2T": bf(cwT("cw2")), "cb2": f32(ii["cb2"]).reshape(-1, 1),
        "cw3T": bf(cwT("cw3")), "cb3": f32(ii["cb3"]).reshape(-1, 1),
        "cw4T": bf(cwT("cw4")), "cb4": f32(ii["cb4"]).reshape(-1, 1),
        "w1xt": bf(w1xt), "fc1_xt_b": rep(ii["fc1_xt_b"], GPC),
        "fc2_xt_w": f32(ii["fc2_xt_w"]), "fc2_xt_b": rep(ii["fc2_xt_b"], GPC),
        "fc1_w": f32(ii["fc1_w"]), "fc1_b": rep(ii["fc1_b"], GPC),
        "fc2_w": f32(ii["fc2_w"]), "fc2_b": rep(ii["fc2_b"], GPC),
        "out_w": f32(ii["out_w"]),
    }
    in_maps = []
    for c in range(NCORES):
        m = dict(shared)
        m["xT"] = bf(xT)                                  # full, replicated
        m["xTl"] = bf(xT[:, c * NPC:(c + 1) * NPC])       # local slice
        m["enat"] = natT[c]
        m["eag"] = agT[c]
        m["S"] = S[c]
        m["ST"] = ST[c]
        # host-embedded CNN input: E[grp, sc, s, bloc*128+e]
        tgt_c = ii["target"][c * GPC:(c + 1) * GPC].astype(np.int64)  # [32,1000]
        E_full = emb[tgt_c]                               # [32, 1000, 128]
        Eh = E_full.reshape(8, 4, 8, 125, 128).transpose(0, 2, 3, 1, 4)
        m["Eh"] = bf(Eh.reshape(8, 8, 125, 512))
        in_maps.append(m)
    out_b = float(np.asarray(ii["out_b"]).reshape(-1)[0])
    return T_blocks, in_maps, out_b


# ------------------------------------------------------------------ program
class P:
    pass


def _aug_cols(li):
    fo, H = LAYERS[li][1], LAYERS[li][2]
    return fo + 2 * H + 2        # h | a_s(f32 as 2H bf16) | ones | pad


def build_program(T_blocks, taps=()):
    T_tot = sum(T_blocks)
    nc = bass.Bass()
    p = P()
    p.nc = nc
    p.T_blocks = T_blocks
    p.taps = set(taps)
    p.tap_tensors = {}

    dp = lambda name, shape, dt: nc.declare_dram_parameter(name, list(shape), dt,
                                                           isOutput=False)
    p.xT = dp("xT", [78, N_NODES], BF16)
    p.xTl = dp("xTl", [78, NPC], BF16)
    p.W = [dp(f"W{i+1}", [LAYERS[i][0], LAYERS[i][1] + 2 * LAYERS[i][2]], BF16)
           for i in range(3)]
    p.br = [dp(f"br{i+1}", [128, LAYERS[i][1]], BF16) for i in range(3)]
    p.wad1 = dp("wad1", [78, LAYERS[0][2]], BF16)
    p.enat = dp("enat", [128, T_tot], I32)
    p.eag = dp("eag", [128, T_tot], I32)
    p.S = dp("S", [T_tot, 128, 128], BF16)
    p.ST = dp("ST", [T_tot, 128, 128], F32)
    p.fc_g1_w = dp("fc_g1_w", [3120, 1024], BF16)
    p.fc_g1_b = dp("fc_g1_b", [GPC, 1024], F32)
    p.fc_g2_w = dp("fc_g2_w", [1024, 128], F32)
    p.fc_g2_b = dp("fc_g2_b", [GPC, 128], F32)
    p.Eh = dp("Eh", [8, 8, 125, 512], BF16)
    p.cw1f = dp("cw1f", [125, 8, 2, 128], BF16)
    p.cb1 = dp("cb1", [32, 1], F32)
    p.cw2T = dp("cw2T", [32, 8, 64], BF16)
    p.cb2 = dp("cb2", [64, 1], F32)
    p.cw3T = dp("cw3T", [64, 8, 96], BF16)
    p.cb3 = dp("cb3", [96, 1], F32)
    p.cw4T = dp("cw4T", [96, 8, 128], BF16)
    p.cb4 = dp("cb4", [128, 1], F32)
    p.w1xt = dp("w1xt", [33, 128, 1024], BF16)
    p.fc1_xt_b = dp("fc1_xt_b", [GPC, 1024], F32)
    p.fc2_xt_w = dp("fc2_xt_w", [1024, 128], F32)
    p.fc2_xt_b = dp("fc2_xt_b", [GPC, 128], F32)
    p.fc1_w = dp("fc1_w", [256, 1024], F32)
    p.fc1_b = dp("fc1_b", [GPC, 1024], F32)
    p.fc2_w = dp("fc2_w", [1024, 256], F32)
    p.fc2_b = dp("fc2_b", [GPC, 256], F32)
    p.out_w = dp("out_w", [256, 1], F32)
    p.out = nc.declare_dram_parameter("out", [GPC, 1], F32, isOutput=True)

    # h1_full natural order (replicated compute, no AG); h2/h3 AG-native order
    p.h_full = [nc.dram_tensor("h1_full", [N_NODES, _aug_cols(0)], BF16)]
    for i in (1, 2):
        p.h_full.append(nc.dram_tensor(f"h{i+1}_full", [N_NODES, _aug_cols(i)],
                                       BF16, addr_space="Shared"))
    p.h_loc = [None,
               nc.dram_tensor("h2_loc", [NPC, _aug_cols(1)], BF16),
               nc.dram_tensor("h3_loc", [NPC, _aug_cols(2)], BF16)]

    def tap(name, shape, dt=F32):
        if name in p.taps:
            t = nc.declare_dram_parameter("tap_" + name, list(shape), dt,
                                          isOutput=True)
            p.tap_tensors[name] = t
            return t
        return None
    p.tap = tap

    with tile.TileContext(nc) as tc:
        p.tc = tc
        _cp_cm = tc.tile_pool(name="const", bufs=1)
        const_pool = _cp_cm.__enter__()
        p.ident = const_pool.tile([128, 128], BF16)
        make_identity(nc, p.ident[:])
        p.head_pool = const_pool

        p.cnn = _cnn_make(p)
        _gat(p)
        _fusion(p)
        _cp_cm.__exit__(None, None, None)

    _split_sync_waits(nc)
    return nc, p


# ---------------- layer-1 replicated matmul ----------------
def _l1_mm(p):
    """Every core computes h1 for ALL nodes -> h1_full (natural order)."""
    nc, tc = p.nc, p.tc
    fi, fo, H = LAYERS[0]
    cols = _aug_cols(0)
    chunks = MM_CHUNKS[0]
    offs = [int(v) for v in np.cumsum([0] + chunks)]
    h_writes = []
    with (
        tc.tile_pool(name="l1w", bufs=1) as wpool,
        tc.tile_pool(name="l1x", bufs=1) as xpool,
        tc.tile_pool(name="l1m", bufs=3) as mpool,
        tc.tile_pool(name="l1p", bufs=2, space="PSUM") as pspool,
    ):
        W_sb = wpool.tile([fi, fo + 2 * H], BF16, tag="W", name="W")
        nc.sync.dma_start(out=W_sb[:], in_=p.W[0][:])
        xf = xpool.tile([fi, N_NODES], BF16, tag="xf", name="xf")
        nc.sync.dma_start(out=xf[:], in_=p.xT[:])
        # local a_d via xTl @ wad1
        xl = xpool.tile([fi, NPC], BF16, tag="xl", name="xl")
        nc.sync.dma_start(out=xl[:], in_=p.xTl[:])
        wad = xpool.tile([fi, H], BF16, tag="wad", name="wad")
        nc.sync.dma_start(out=wad[:], in_=p.wad1[:])
        p.a_d1 = []
        with tc.tile_pool(name="l1adp", bufs=2, space="PSUM") as adps:
            for blk in range(BPC):
                ps = adps.tile([128, H], F32, tag="ad", name="ad")
                nc.tensor.matmul(ps[:], xl[:, blk * 128:(blk + 1) * 128], wad[:],
                                 start=True, stop=True)
                t = p.head_pool.tile([128, H], F32, tag=f"a_d1_{blk}",
                                     name=f"a_d1_{blk}")
                nc.vector.tensor_copy(out=t[:], in_=ps[:])
                p.a_d1.append(t)
        for m in range(NBLK):
            stage = mpool.tile([128, cols], BF16, tag="stage", name="stage",
                               bufs=4)
            for n in range(len(chunks)):
                lo, hi = offs[n], offs[n + 1]
                ps = pspool.tile([128, chunks[n]], F32, tag=f"mp{n}", name=f"mp{n}")
                nc.tensor.matmul(ps[:], xf[:, m * 128:(m + 1) * 128],
                                 W_sb[:, lo:hi], start=True, stop=True)
                if hi <= fo:
                    nc.scalar.copy(out=stage[:, lo:hi], in_=ps[:])
                else:
                    if lo < fo:
                        nc.vector.tensor_copy(out=stage[:, lo:fo],
                                              in_=ps[:, :fo - lo])
                    a_sf = mpool.tile([128, H], F32, tag="a_sf", name="a_sf")
                    nc.vector.tensor_copy(out=a_sf[:], in_=ps[:, fo - lo:fo - lo + H])
                    nc.vector.tensor_copy(
                        out=stage[:, fo:fo + 2 * H], in_=a_sf[:].bitcast(BF16))
            if m < 4:
                oc = fo + 2 * H
                nc.vector.memset(stage[:, oc:oc + 1], 1.0)
                nc.vector.memset(stage[:, oc + 1:cols], 0.0)
            w = nc.sync.dma_start(out=p.h_full[0][m * 128:(m + 1) * 128, :],
                                  in_=stage[:])
            h_writes.append(w)
    # fence: all h1_full writes done
    fence_sb = p.head_pool.tile([1, 2], BF16, tag="fence1", name="fence1")
    fence = nc.sync.dma_start(out=fence_sb[:], in_=p.h_full[0][0:1, 0:2])
    for w in h_writes:
        add_dep_helper(fence.ins, w.ins, reason="h1 fence")
    return fence


# ---------------- message pass for one dst block ----------------
def _mp_block(p, li, blk, pools, a_d_tile, fence, xT_out):
    """Message pass layer li for dst block blk. xT_out: either a list of
    full-width tiles (write cols blk*128..) or None -> allocate per-block
    [128,128] tiles in the mp pool and return them."""
    nc, tc = p.nc, p.tc
    fi, fo, H = LAYERS[li]
    FH = fo // H
    cols = _aug_cols(li)
    gsc = H + fo                     # gs cols: [ex H | h fo]
    chunks = MP_CHUNKS[li]
    offs = [int(v) for v in np.cumsum([0] + chunks)]
    t_off = np.cumsum([0] + p.T_blocks)
    Tb = p.T_blocks[blk]
    t0 = int(t_off[blk])
    n_kT = cdiv(fo, 128)
    mp, mpS = pools["mp"], pools["mpS"]
    eidx = p.eidx_ag if li == 2 else p.eidx_nat

    S_blk = mpS.tile([128, Tb * 128], BF16, tag="Sblk", name="Sblk", bufs=2)
    nc.sync.dma_start(
        out=S_blk[:].rearrange("p (t c) -> p t c", c=128),
        in_=p.S[t0:t0 + Tb].rearrange("t p c -> p t c"))
    ST_blk = mpS.tile([128, Tb * 128], F32, tag="STblk", name="STblk", bufs=2)
    nc.sync.dma_start(
        out=ST_blk[:].rearrange("p (t c) -> p t c", c=128),
        in_=p.ST[t0:t0 + Tb].rearrange("t p c -> p t c"))

    per_tile = True
    rows_bf = mp.tile([128, fo], BF16, tag="rows_bf", name="rows_bf", bufs=2)
    rows_f = mp.tile([128, fo], F32, tag="rows_f", name="rows_f") \
        if li == 0 else rows_bf
    rec = mp.tile([128, H], F32, tag="rec", name="rec", bufs=2)
    if per_tile:
        # per-tile pipeline (H == 1): gather -> score -> scale -> matmuls
        with (
            tc.tile_pool(name=f"ap{li}_{blk}", bufs=1, space="PSUM") as pa,
            tc.tile_pool(name=f"op{li}_{blk}", bufs=1, space="PSUM") as pp,
        ):
            n_ch = len(chunks)
            opsum = [pp.tile([128, chunks[n]], F32, tag=f"op{n}", name=f"op{n}")
                     for n in range(n_ch)]
            for i in range(Tb):
                t = t0 + i
                g_t = mp.tile([128, cols], BF16, tag="g_t", name="g_t", bufs=6)
                gi = nc.gpsimd.indirect_dma_start(
                    out=g_t[:], out_offset=None, in_=p.h_full[li][:],
                    in_offset=bass.IndirectOffsetOnAxis(ap=eidx[:, t:t + 1],
                                                        axis=0))
                add_dep_helper(gi.ins, fence.ins, reason="gather waits h_full")
                adg = pa.tile([128, H], F32, tag="adg", name="adg")
                nc.tensor.matmul(adg[:], ST_blk[:, i * 128:(i + 1) * 128],
                                 a_d_tile[:], start=True, stop=True)
                sc = mp.tile([128, H], F32, tag="sc_t", name="sc_t", bufs=6)
                nc.vector.tensor_tensor(out=sc[:],
                                        in0=g_t[:, fo:fo + 2 * H].bitcast(F32),
                                        in1=adg[:], op=OP.add)
                ex = mp.tile([128, H], F32, tag="ex_t", name="ex_t", bufs=6)
                nc.scalar.activation(ex[:], sc[:], ACT.Prelu, alpha=NEG_SLOPE)
                nc.scalar.activation(ex[:], ex[:], ACT.Exp)
                gs_t = mp.tile([128, gsc], BF16, tag="gs_t", name="gs_t", bufs=4)
                nc.vector.tensor_copy(out=gs_t[:, 0:H], in_=ex[:])
                for h in range(H):
                    eng = nc.vector if h % 2 == 0 else nc.scalar
                    if h % 2 == 0:
                        nc.vector.tensor_scalar(
                            out=gs_t[:, H + h * FH:H + (h + 1) * FH],
                            in0=g_t[:, h * FH:(h + 1) * FH],
                            scalar1=ex[:, h:h + 1], scalar2=None, op0=OP.mult)
                    else:
                        nc.scalar.mul(gs_t[:, H + h * FH:H + (h + 1) * FH],
                                      g_t[:, h * FH:(h + 1) * FH],
                                      ex[:, h:h + 1])
                for n in range(n_ch):
                    nc.tensor.matmul(
                        opsum[n][:], S_blk[:, i * 128:(i + 1) * 128],
                        gs_t[:, offs[n]:offs[n + 1]],
                        start=(i == 0), stop=(i == Tb - 1))
            nc.vector.tensor_scalar(out=rec[:], in0=opsum[0][:, 0:H],
                                    scalar1=1e-16, scalar2=None, op0=OP.add)
            nc.vector.reciprocal(rec[:], rec[:])
            for n in range(n_ch):
                lo, hi = max(offs[n], H), offs[n + 1]
                a, b = lo - H, hi - H
                h0, h1 = a // FH, cdiv(b, FH)
                for h in range(h0, h1):
                    s_lo, s_hi = max(a, h * FH), min(b, (h + 1) * FH)
                    nc.vector.scalar_tensor_tensor(
                        out=rows_f[:, s_lo:s_hi],
                        in0=opsum[n][:, s_lo + H - offs[n]:s_hi + H - offs[n]],
                        scalar=rec[:, h:h + 1], in1=p.br_sb[li][:, s_lo:s_hi],
                        op0=OP.mult, op1=OP.add)
    # activation (batched, row-major)
    if li == 0:
        t1 = mp.tile([128, fo], F32, tag="elu1", name="elu1")
        nc.vector.tensor_scalar(out=t1[:], in0=rows_f[:], scalar1=0.0,
                                scalar2=None, op0=OP.min)
        nc.scalar.activation(t1[:], t1[:], ACT.Exp)
        nc.scalar.activation(rows_f[:], rows_f[:], ACT.Relu)
        nc.vector.scalar_tensor_tensor(out=rows_bf[:], in0=t1[:], scalar=-1.0,
                                       in1=rows_f[:], op0=OP.add, op1=OP.add)
    else:
        nc.scalar.activation(rows_bf[:], rows_bf[:], ACT.Relu)
    # transpose; either into full-width xT_out columns or per-block tiles
    if xT_out is None:
        xtb = [mp.tile([min(128, fo - j * 128), 128], BF16, tag=f"xtb{j}",
                       name=f"xtb{j}", bufs=2) for j in range(n_kT)]
        dst = lambda j: xtb[j][:]
    else:
        dst = lambda j: xT_out[j][:, blk * 128:(blk + 1) * 128]
    with tc.tile_pool(name=f"tp{li}_{blk}", bufs=2, space="PSUM") as ptp:
        for j in range(n_kT):
            kp = min(128, fo - j * 128)
            tp = ptp.tile([kp, 128], BF16, tag="tp", name="tp")
            nc.tensor.transpose(tp[:], rows_bf[:, j * 128:j * 128 + kp],
                                p.ident[:])
            if j % 2 == 0:
                nc.scalar.copy(out=dst(j), in_=tp[:])
            else:
                nc.vector.tensor_copy(out=dst(j), in_=tp[:])
    return None if xT_out is not None else xtb


# ---------------- next-layer matmul for one dst block ----------------
def _mm_block(p, li, blk, W_sb, xtb, pools, a_d_list):
    """x@W for layer li (1 or 2), rows of dst block blk (lhsT tiles xtb);
    writes h_loc rows. Returns the stage-write instr for the AG."""
    nc, tc = p.nc, p.tc
    fi, fo, H = LAYERS[li]
    cols = _aug_cols(li)
    n_k = cdiv(fi, 128)
    chunks = MM_CHUNKS[li]
    offs = [int(v) for v in np.cumsum([0] + chunks)]
    mp = pools["mm"]
    stage = mp.tile([128, cols], BF16, tag=f"stg{li}", name=f"stg{li}", bufs=2)
    with tc.tile_pool(name=f"mmp{li}_{blk}", bufs=4, space="PSUM") as pspool:
        for n in range(len(chunks)):
            lo, hi = offs[n], offs[n + 1]
            ps = pspool.tile([128, chunks[n]], F32, tag="mmps", name="mmps")
            for k in range(n_k):
                nc.tensor.matmul(
                    ps[:], xtb[k], W_sb[k][:, lo:hi],
                    start=(k == 0), stop=(k == n_k - 1))
            if hi <= fo:
                nc.scalar.copy(out=stage[:, lo:hi], in_=ps[:])
            else:
                if lo < fo:
                    nc.scalar.copy(out=stage[:, lo:fo], in_=ps[:, :fo - lo])
                a_sf = mp.tile([128, H], F32, tag="a_sf2", name="a_sf2")
                nc.vector.tensor_copy(out=a_sf[:], in_=ps[:, fo - lo:fo - lo + H])
                a_d = p.head_pool.tile([128, H], F32, tag=f"a_d{li}_{blk}",
                                       name=f"a_d{li}_{blk}")
                nc.vector.tensor_copy(out=a_d[:],
                                      in_=ps[:, fo - lo + H:fo - lo + 2 * H])
                a_d_list.append(a_d)
                nc.vector.tensor_copy(out=stage[:, fo:fo + 2 * H],
                                      in_=a_sf[:].bitcast(BF16))
    oc = fo + 2 * H
    nc.vector.memset(stage[:, oc:oc + 1], 1.0)
    nc.vector.memset(stage[:, oc + 1:cols], 0.0)
    w = nc.sync.dma_start(out=p.h_loc[li][blk * 128:(blk + 1) * 128, :],
                          in_=stage[:])
    return w


def _fire_ag(p, li, half, ws):
    """AllGather of one half of h_loc -> h_full rows (half-AG order)."""
    nc = p.nc
    cc = nc.gpsimd.collective_compute(
        "AllGather", OP.bypass, replica_groups=[list(range(NCORES))],
        ins=[p.h_loc[li][half * 640:(half + 1) * 640, :]],
        outs=[p.h_full[li][half * 5120:(half + 1) * 5120, :]])
    for w in ws:
        add_dep_helper(cc.ins, w.ins, reason="AG waits h_loc writes")
    return cc


# ---------------- GAT orchestration ----------------
def _gat(p):
    nc, tc = p.nc, p.tc
    T_tot = sum(p.T_blocks)

    mpc_cm = tc.tile_pool(name="mpc", bufs=1)
    mpc = mpc_cm.__enter__()
    p.eidx_nat = mpc.tile([128, T_tot], I32, tag="enat", name="enat")
    nc.sync.dma_start(out=p.eidx_nat[:], in_=p.enat[:])
    p.eidx_ag = mpc.tile([128, T_tot], I32, tag="eag", name="eag")
    nc.sync.dma_start(out=p.eidx_ag[:], in_=p.eag[:])
    p.br_sb = []
    for li in range(3):
        t = mpc.tile([128, LAYERS[li][1]], BF16, tag=f"br{li}", name=f"br{li}")
        nc.sync.dma_start(out=t[:], in_=p.br[li][:])
        p.br_sb.append(t)

    fence1 = _l1_mm(p)
    p.cnn["stage1_w"]()     # CNN weight loads early

    # ---- phase 1: MP L1 (all blocks) -> dense mm L2 with mid-pass AG ----
    a_d2 = []
    ccs2 = []
    xT2_cm = tc.tile_pool(name="xT2", bufs=1)
    xT2p = xT2_cm.__enter__()
    fo2 = LAYERS[0][1]
    n_kT2 = cdiv(fo2, 128)
    xT2 = [xT2p.tile([min(128, fo2 - j * 128), NPC], BF16, tag=f"xT2_{j}",
                     name=f"xT2_{j}") for j in range(n_kT2)]
    with (
        tc.tile_pool(name="w2", bufs=1) as w2pool,
        tc.tile_pool(name="mp1", bufs=1) as mp1,
        tc.tile_pool(name="mpS1", bufs=1) as mpS1,
        tc.tile_pool(name="mm1", bufs=1) as mm1,
    ):
        fi2, fo2b, H2 = LAYERS[1]
        W2_sb = []
        for k in range(cdiv(fi2, 128)):
            kp = min(128, fi2 - k * 128)
            t = w2pool.tile([kp, fo2b + 2 * H2], BF16, tag=f"W2_{k}",
                            name=f"W2_{k}")
            nc.sync.dma_start(out=t[:], in_=p.W[1][k * 128:k * 128 + kp, :])
            W2_sb.append(t)
        pools = {"mp": mp1, "mpS": mpS1, "mm": mm1}
        for blk in range(BPC):
            _mp_block(p, 0, blk, pools, p.a_d1[blk], fence1, xT2)
        ws = []
        for blk in range(BPC):
            xts = [xT2[k][:min(128, fo2 - k * 128), blk * 128:(blk + 1) * 128]
                   for k in range(n_kT2)]
            ws.append(_mm_block(p, 1, blk, W2_sb, xts, pools, a_d2))
        cc = nc.gpsimd.collective_compute(
            "AllGather", OP.bypass, replica_groups=[list(range(NCORES))],
            ins=[p.h_loc[1][:]], outs=[p.h_full[1][:]])
        for w in ws:
            add_dep_helper(cc.ins, w.ins, reason="AG2 waits h_loc writes")
        ccs2.append(cc)
        for g in range(8):
            p.cnn["stage1_grp"](g)
        p.cnn["stage2"]()
        p.cnn["stage3"]()
    xT2_cm.__exit__(None, None, None)
    p.cnn["close_E"]()
    p.cnn["cleanup"]()
    fence2_sb = p.head_pool.tile([1, 2], BF16, tag="fence2", name="fence2")
    fence2 = nc.sync.dma_start(out=fence2_sb[:], in_=p.h_full[1][0:1, 0:2])
    for cc in ccs2:
        add_dep_helper(fence2.ins, cc.ins, reason="h2 fence")

    # ---- phase 2: MP L2 (all blocks) -> dense mm L3 with mid-pass AG ----
    a_d3 = []
    ccs3 = []
    xT3_cm = tc.tile_pool(name="xT3", bufs=1)
    xT3p = xT3_cm.__enter__()
    fo3 = LAYERS[1][1]
    n_kT3 = cdiv(fo3, 128)
    xT3 = [xT3p.tile([min(128, fo3 - j * 128), NPC], BF16, tag=f"xT3_{j}",
                     name=f"xT3_{j}") for j in range(n_kT3)]
    with (
        tc.tile_pool(name="w3", bufs=1) as w3pool,
        tc.tile_pool(name="mp2", bufs=1) as mp2,
        tc.tile_pool(name="mpS2", bufs=1) as mpS2,
        tc.tile_pool(name="mm2", bufs=1) as mm2,
    ):
        fi3, fo3b, H3 = LAYERS[2]
        W3_sb = []
        for k in range(cdiv(fi3, 128)):
            kp = min(128, fi3 - k * 128)
            t = w3pool.tile([kp, fo3b + 2 * H3], BF16, tag=f"W3_{k}",
                            name=f"W3_{k}")
            nc.sync.dma_start(out=t[:], in_=p.W[2][k * 128:k * 128 + kp, :])
            W3_sb.append(t)
        pools = {"mp": mp2, "mpS": mpS2, "mm": mm2}
        for blk in range(BPC):
            _mp_block(p, 1, blk, pools, a_d2[blk], fence2, xT3)
        ws = []
        for blk in range(BPC):
            xts = [xT3[k][:min(128, fo3 - k * 128), blk * 128:(blk + 1) * 128]
                   for k in range(n_kT3)]
            ws.append(_mm_block(p, 2, blk, W3_sb, xts, pools, a_d3))
            if blk == 4:
                ccs3.append(_fire_ag(p, 2, 0, ws))
        ccs3.append(_fire_ag(p, 2, 1, ws[5:]))
    xT3_cm.__exit__(None, None, None)
    fence3_sb = p.head_pool.tile([1, 2], BF16, tag="fence3", name="fence3")
    fence3 = nc.sync.dma_start(out=fence3_sb[:], in_=p.h_full[2][0:1, 0:2])
    for cc in ccs3:
        add_dep_helper(fence3.ins, cc.ins, reason="h3 fence")

    # ---- phase 3: MP L3 (+ CNN stages 2/3) ----
    xT4_cm = tc.tile_pool(name="xT4", bufs=1)
    xT4p = xT4_cm.__enter__()
    fo4 = LAYERS[2][1]
    p.out3T = [xT4p.tile([min(128, fo4 - j * 128), NPC], BF16, tag=f"xT4_{j}",
                         name=f"xT4_{j}") for j in range(cdiv(fo4, 128))]
    with (
        tc.tile_pool(name="mp3", bufs=1) as mp3,
        tc.tile_pool(name="mpS3", bufs=1) as mpS3,
    ):
        pools = {"mp": mp3, "mpS": mpS3, "mm": None}
        for blk in range(BPC):
            _mp_block(p, 2, blk, pools, a_d3[blk], fence3, p.out3T)
    p.gat_cleanup = [xT4_cm, mpc_cm]

    t = p.tap("o3T", [fo4, NPC], BF16)
    if t is not None:
        for j in range(cdiv(fo4, 128)):
            kp = min(128, fo4 - j * 128)
            nc.sync.dma_start(out=t[j * 128:j * 128 + kp, :], in_=p.out3T[j][:])


def _dve_T(nc, dst, src, n):
    """dst[n, 32] = src[32, n].T via DVE 32x32 block transposes (f32)."""
    for i in range(n // 32):
        nc.vector.transpose(out=dst[32 * i:32 * (i + 1), :],
                            in_=src[:, 32 * i:32 * (i + 1)])


# ---------------- graph head ----------------
def _graph_head(p):
    nc, tc = p.nc, p.tc
    n_kT = len(p.out3T)
    with (
        tc.tile_pool(name="gh", bufs=2) as gh,
        tc.tile_pool(name="ghG", bufs=1) as ghG,
        tc.tile_pool(name="ghp", bufs=2, space="PSUM") as ghp,
    ):
        gT = [ghG.tile([min(128, 3120 - j * 128), GPC], BF16, tag=f"gT{j}",
                       name=f"gT{j}") for j in range(n_kT)]
        for j in range(n_kT):
            nc.vector.reduce_max(
                gT[j][:],
                p.out3T[j][:].rearrange("p (g n) -> p g n", n=NPG),
                axis=AX.X)
        g1 = ghG.tile([GPC, 1024], F32, tag="g1", name="g1")
        for n in range(2):
            ps = ghp.tile([GPC, 512], F32, tag="mm", name="mm")
            for j in range(n_kT):
                kp = min(128, 3120 - j * 128)
                w = gh.tile([kp, 512], BF16, tag="fg1w", name="fg1w")
                nc.sync.dma_start(out=w[:], in_=p.fc_g1_w[j * 128:j * 128 + kp,
                                                          n * 512:(n + 1) * 512])
                nc.tensor.matmul(ps[:], gT[j][:], w[:], start=(j == 0),
                                 stop=(j == n_kT - 1))
            nc.vector.tensor_copy(out=g1[:, n * 512:(n + 1) * 512], in_=ps[:])
        bb1 = gh.tile([GPC, 1024], F32, tag="ghbb", name="ghbb")
        nc.sync.dma_start(out=bb1[:], in_=p.fc_g1_b[:])
        nc.vector.tensor_tensor(out=g1[:], in0=g1[:], in1=bb1[:], op=OP.add)
        g1b = ghG.tile([GPC, 1024], F32, tag="g1b", name="g1b")
        nc.scalar.activation(g1b[:], g1[:], ACT.Relu)
        g1T = [ghG.tile([128, GPC], F32, tag=f"g1T{j}", name=f"g1T{j}")
               for j in range(8)]
        for j in range(8):
            _dve_T(nc, g1T[j], g1b[:, j * 128:(j + 1) * 128], 128)
        ps = ghp.tile([GPC, 128], F32, tag="mm", name="mm")
        for j in range(8):
            w = gh.tile([128, 128], F32, tag="fg2w", name="fg2w")
            nc.sync.dma_start(out=w[:], in_=p.fc_g2_w[j * 128:(j + 1) * 128, :])
            nc.tensor.matmul(ps[:], g1T[j][:], w[:], start=(j == 0), stop=(j == 7))
        p.g2 = p.head_pool.tile([GPC, 128], F32, tag="g2", name="g2")
        bb2 = gh.tile([GPC, 128], F32, tag="ghbb2", name="ghbb2")
        nc.sync.dma_start(out=bb2[:], in_=p.fc_g2_b[:])
        nc.vector.tensor_tensor(out=p.g2[:], in0=ps[:], in1=bb2[:], op=OP.add)
        t = p.tap("g2", [GPC, 128])
        if t is not None:
            nc.sync.dma_start(out=t[:], in_=p.g2[:])


# ---------------- CNN branch ----------------
def _cnn_make(p):
    """CNN branch split into stages interleaved into the GAT phases."""
    nc, tc = p.nc, p.tc
    st = {}

    def stage1_w():
        cn_cm = tc.tile_pool(name="cn", bufs=2)
        cnw_cm = tc.tile_pool(name="cnw", bufs=1)
        cny_cm = tc.tile_pool(name="cny", bufs=1)
        cn = cn_cm.__enter__()
        cnw = cnw_cm.__enter__()
        cny = cny_cm.__enter__()
        st["cms"] = [cny_cm, cnw_cm, cn_cm]
        st["cn"], st["cnw"], st["cny"] = cn, cnw, cny
        cw1f_sb = cnw.tile([125, 8, 2, 128], BF16, tag="cw1f", name="cw1f")
        nc.sync.dma_start(out=cw1f_sb[:], in_=p.cw1f[:])
        cw2_sb = cnw.tile([32, 8, 64], BF16, tag="cw2", name="cw2")
        nc.sync.dma_start(out=cw2_sb[:], in_=p.cw2T[:])
        cw3_sb = cnw.tile([64, 8, 96], BF16, tag="cw3", name="cw3")
        nc.sync.dma_start(out=cw3_sb[:], in_=p.cw3T[:])
        cw4_sb = cnw.tile([96, 8, 128], BF16, tag="cw4", name="cw4")
        nc.sync.dma_start(out=cw4_sb[:], in_=p.cw4T[:])
        cb = {}
        for nm, sh in [("cb1", 32), ("cb2", 64), ("cb3", 96), ("cb4", 128)]:
            cb[nm] = cnw.tile([sh, 1], F32, tag=nm, name=nm)
            nc.sync.dma_start(out=cb[nm][:], in_=getattr(p, nm)[:])
        st.update(cw1f=cw1f_sb, cw2=cw2_sb, cw3=cw3_sb, cw4=cw4_sb, cb=cb)
        y1 = cny.tile([32, GPC * 121], BF16, tag="y1", name="y1")
        st["y1"] = y1
        E_cm = tc.tile_pool(name="Epool", bufs=1)
        Ep = E_cm.__enter__()
        st["E_cm"] = E_cm
        E_all = Ep.tile([125, 64 * 512], BF16, tag="E_all", name="E_all")
        st["E_all"] = E_all
        for grp in range(8):
            for sc in range(8):
                nc.sync.dma_start(
                    out=E_all[:, (grp * 8 + sc) * 512:(grp * 8 + sc + 1) * 512],
                    in_=p.Eh[grp, sc])

    def stage1_grp(grp):
        cn, cb = st["cn"], st["cb"]
        cw1f_sb = st["cw1f"]
        y1 = st["y1"]
        with tc.tile_pool(name=f"cnp1_{grp}", bufs=1, space="PSUM") as cnp:
            pc = [cnp.tile([128, 512], F32, tag=f"pc{k}", name=f"pc{k}")
                  for k in range(2)]
            E_all = st["E_all"]
            for sc in range(8):
                j = (grp * 8 + sc) * 512
                for ks in range(2):
                    nc.tensor.matmul(pc[ks][:], cw1f_sb[:, sc, ks, :],
                                     E_all[:, j:j + 512],
                                     start=(sc == 0), stop=(sc == 7))
            acc = cn.tile([32, 4 * 121], F32, tag="c1acc", name="c1acc", bufs=1)
            accr = acc[:].rearrange("p (b t) -> p b t", b=4)
            firstop = True
            for ks in range(2):
                for kl in range(4):
                    k = ks * 4 + kl
                    src = pc[ks][:].rearrange("p (b j) -> p b j", b=4)[
                        kl * 32:(kl + 1) * 32, :, k:k + 121]
                    if firstop:
                        nc.vector.tensor_copy(out=accr, in_=src)
                        firstop = False
                    else:
                        nc.vector.tensor_tensor(out=accr, in0=accr, in1=src,
                                                op=OP.add)
            nc.scalar.activation(y1[:, grp * 4 * 121:(grp + 1) * 4 * 121],
                                 acc[:], ACT.Relu, bias=cb["cb1"][:32, :1])

    def stage2():
        cn, cny, cb = st["cn"], st["cny"], st["cb"]
        cw2_sb, cw3_sb, cw4_sb = st["cw2"], st["cw3"], st["cw4"]
        y1 = st["y1"]
        with tc.tile_pool(name="cnp2", bufs=2, space="PSUM") as cnp:
            y2 = cny.tile([64, GPC * 114], BF16, tag="y2", name="y2")
            for grp in range(8):
                ps = cnp.tile([64, 4 * 114], F32, tag="pc0", name="pc0")
                for k in range(8):
                    rhs = y1[:].rearrange("p (b t) -> p b t", t=121)[
                        :, grp * 4:(grp + 1) * 4, k:k + 114]
                    nc.tensor.matmul(ps[:], cw2_sb[:, k, :], rhs, start=(k == 0),
                                     stop=(k == 7))
                nc.scalar.activation(y2[:, grp * 4 * 114:(grp + 1) * 4 * 114],
                                     ps[:], ACT.Relu, bias=cb["cb2"][:, :1])
            y3 = cny.tile([96, GPC * 107], BF16, tag="y3", name="y3")
            for grp in range(8):
                ps = cnp.tile([96, 4 * 107], F32, tag="pc0", name="pc0")
                for k in range(8):
                    rhs = y2[:].rearrange("p (b t) -> p b t", t=114)[
                        :, grp * 4:(grp + 1) * 4, k:k + 107]
                    nc.tensor.matmul(ps[:], cw3_sb[:, k, :], rhs, start=(k == 0),
                                     stop=(k == 7))
                nc.scalar.activation(y3[:, grp * 4 * 107:(grp + 1) * 4 * 107],
                                     ps[:], ACT.Relu, bias=cb["cb3"][:, :1])
            yp = cny.tile([128, GPC * 33], BF16, tag="yp", name="yp")
            st["yp"] = yp
            for grp in range(8):
                ps = cnp.tile([128, 4 * 100], F32, tag="pc0", name="pc0")
                for k in range(8):
                    rhs = y3[:].rearrange("p (b t) -> p b t", t=107)[
                        :, grp * 4:(grp + 1) * 4, k:k + 100]
                    nc.tensor.matmul(ps[:], cw4_sb[:, k, :], rhs, start=(k == 0),
                                     stop=(k == 7))
                psr = ps[:].rearrange("p (b t) -> p b t", b=4)
                mx = cn.tile([128, 4 * 33], F32, tag="mx", name="mx")
                mxr = mx[:].rearrange("p (b t) -> p b t", b=4)
                nc.vector.tensor_copy(out=mxr, in_=psr[:, :, 0:99:3])
                nc.vector.tensor_tensor(out=mxr, in0=mxr, in1=psr[:, :, 1:100:3],
                                        op=OP.max)
                nc.vector.tensor_tensor(out=mxr, in0=mxr, in1=psr[:, :, 2:100:3],
                                        op=OP.max)
                nc.scalar.activation(yp[:, grp * 4 * 33:(grp + 1) * 4 * 33],
                                     mx[:], ACT.Relu, bias=cb["cb4"][:, :1])

    def stage3():
        cn, cny = st["cn"], st["cny"]
        yp = st["yp"]
        with tc.tile_pool(name="cnp3", bufs=2, space="PSUM") as cnp:
            xt1 = cny.tile([GPC, 1024], F32, tag="xt1", name="xt1")
            for n in range(2):
                ps = cnp.tile([GPC, 512], F32, tag="pc0", name="pc0")
                for t_ in range(33):
                    w = cn.tile([128, 512], BF16, tag="fx1w", name="fx1w", bufs=2)
                    nc.sync.dma_start(out=w[:],
                                      in_=p.w1xt[t_, :, n * 512:(n + 1) * 512])
                    lhs = yp[:].rearrange("p (b t) -> p t b", t=33)[:, t_, :]
                    nc.tensor.matmul(ps[:], lhs, w[:], start=(t_ == 0),
                                     stop=(t_ == 32))
                nc.vector.tensor_copy(out=xt1[:, n * 512:(n + 1) * 512], in_=ps[:])
            bb = cn.tile([GPC, 1024], F32, tag="fxbb", name="fxbb", bufs=1)
            nc.sync.dma_start(out=bb[:], in_=p.fc1_xt_b[:])
            nc.vector.tensor_tensor(out=xt1[:], in0=xt1[:], in1=bb[:], op=OP.add)
            nc.scalar.activation(xt1[:], xt1[:], ACT.Relu)
            xt1T = [cn.tile([128, GPC], F32, tag=f"xt1T{j}", name=f"xt1T{j}",
                            bufs=1)
                    for j in range(8)]
            for j in range(8):
                _dve_T(nc, xt1T[j], xt1[:, j * 128:(j + 1) * 128], 128)
            ps = cnp.tile([GPC, 128], F32, tag="pc0", name="pc0")
            for j in range(8):
                w = cn.tile([128, 128], F32, tag="fx2w", name="fx2w", bufs=2)
                nc.sync.dma_start(out=w[:], in_=p.fc2_xt_w[j * 128:(j + 1) * 128, :])
                nc.tensor.matmul(ps[:], xt1T[j][:], w[:], start=(j == 0),
                                 stop=(j == 7))
            p.xt2 = p.head_pool.tile([GPC, 128], F32, tag="xt2", name="xt2")
            bb2 = cn.tile([GPC, 128], F32, tag="fxbb2", name="fxbb2", bufs=1)
            nc.sync.dma_start(out=bb2[:], in_=p.fc2_xt_b[:])
            nc.vector.tensor_tensor(out=p.xt2[:], in0=ps[:], in1=bb2[:], op=OP.add)
            t = p.tap("xt2", [GPC, 128])
            if t is not None:
                nc.sync.dma_start(out=t[:], in_=p.xt2[:])

    def close_E():
        st["E_cm"].__exit__(None, None, None)

    def cleanup():
        for cm in st["cms"]:
            cm.__exit__(None, None, None)

    return {"stage1_w": stage1_w, "stage1_grp": stage1_grp, "stage2": stage2,
            "stage3": stage3, "cleanup": cleanup, "close_E": close_E}


# ---------------- fusion ----------------
def _fusion(p):
    nc, tc = p.nc, p.tc
    _graph_head(p)
    for cm in p.gat_cleanup:
        cm.__exit__(None, None, None)
    with (
        tc.tile_pool(name="fu", bufs=2) as fu,
        tc.tile_pool(name="fup", bufs=2, space="PSUM") as fup,
    ):
        xcT = []
        for src_ in (p.g2, p.xt2):
            t = fu.tile([128, GPC], F32, tag=f"xcT{len(xcT)}", name=f"xcT{len(xcT)}")
            _dve_T(nc, t, src_[:], 128)
            xcT.append(t)
        c1 = fu.tile([GPC, 1024], F32, tag="c1", name="c1")
        for n in range(2):
            ps = fup.tile([GPC, 512], F32, tag="mm", name="mm")
            for j in range(2):
                w = fu.tile([128, 512], F32, tag="f1w", name="f1w")
                nc.sync.dma_start(out=w[:], in_=p.fc1_w[j * 128:(j + 1) * 128,
                                                        n * 512:(n + 1) * 512])
                nc.tensor.matmul(ps[:], xcT[j][:], w[:], start=(j == 0),
                                 stop=(j == 1))
            nc.vector.tensor_copy(out=c1[:, n * 512:(n + 1) * 512], in_=ps[:])
        bb = fu.tile([GPC, 1024], F32, tag="fbb", name="fbb")
        nc.sync.dma_start(out=bb[:], in_=p.fc1_b[:])
        nc.vector.tensor_tensor(out=c1[:], in0=c1[:], in1=bb[:], op=OP.add)
        c1b = fu.tile([GPC, 1024], F32, tag="c1b", name="c1b")
        nc.scalar.activation(c1b[:], c1[:], ACT.Relu)
        c1T = [fu.tile([128, GPC], F32, tag=f"c1T{j}", name=f"c1T{j}") for j in range(8)]
        for j in range(8):
            _dve_T(nc, c1T[j], c1b[:, j * 128:(j + 1) * 128], 128)
        ps = fup.tile([GPC, 256], F32, tag="mm", name="mm")
        for j in range(8):
            w = fu.tile([128, 256], F32, tag="f2w", name="f2w")
            nc.sync.dma_start(out=w[:], in_=p.fc2_w[j * 128:(j + 1) * 128, :])
            nc.tensor.matmul(ps[:], c1T[j][:], w[:], start=(j == 0), stop=(j == 7))
        c2 = fu.tile([GPC, 256], F32, tag="c2", name="c2")
        bb2 = fu.tile([GPC, 256], F32, tag="fbb2", name="fbb2")
        nc.sync.dma_start(out=bb2[:], in_=p.fc2_b[:])
        nc.vector.tensor_tensor(out=c2[:], in0=ps[:], in1=bb2[:], op=OP.add)
        c2b = fu.tile([GPC, 256], F32, tag="c2b", name="c2b")
        nc.scalar.activation(c2b[:], c2[:], ACT.Relu)
        c2T = []
        for j in range(2):
            t = fu.tile([128, GPC], F32, tag=f"c2T{j}", name=f"c2T{j}")
            _dve_T(nc, t, c2b[:, j * 128:(j + 1) * 128], 128)
            c2T.append(t)
        ow = fu.tile([128, 2], F32, tag="ow", name="ow")
        for j in range(2):
            nc.sync.dma_start(out=ow[:, j:j + 1], in_=p.out_w[j * 128:(j + 1) * 128, :])
        ps = fup.tile([GPC, 1], F32, tag="mm", name="mm")
        for j in range(2):
            nc.tensor.matmul(ps[:], c2T[j][:], ow[:, j:j + 1],
                             start=(j == 0), stop=(j == 1))
        o = fu.tile([GPC, 1], F32, tag="o", name="o")
        nc.vector.tensor_copy(out=o[:], in_=ps[:])
        nc.sync.dma_start(out=p.out[:], in_=o[:])


# ------------------------------------------------------------------ entry
def _build_and_run(inputs, taps=()):
    T_blocks, in_maps, out_b = _host_prep(inputs)
    nc, p = build_program(T_blocks, taps=taps)
    res = run_bass_kernel_spmd(nc, in_maps, list(range(NCORES)))
    return res, out_b, p


def kernel(**inputs) -> np.ndarray:
    res, out_b, _ = _build_and_run(inputs)
    out = np.concatenate([res.results[c]["out"] for c in range(NCORES)], axis=0)
    return (out + out_b).astype(np.float32)
